# revision 2
# baseline (speedup 1.0000x reference)
"""Causal self-attention (B=4, T=2048, C=1024, H=16, D=64) on 8 TRN2 NeuronCores.

Sharding: core = (batch b, head-group g) with b = core // 2, g = core % 2.
Each core computes heads [8g, 8g+8) of batch b and produces the partial
out-projection (C, T) for its head group; the host sums the two head-group
partials per batch and adds the output bias.

v2: fp8e4m3 DoubleRow matmuls for qk-projection, v-projection, scores and
att@V (paired k-tiles / s-tiles; scores use a zeroed second slot over the
d=64 contraction). The out-projection and the first t-tile's attention
(t < 512, where softmax averaging is too weak to wash out fp8 noise) stay
fp16. Output DMA goes straight from PSUM; softmax normalization uses a
0-stride broadcast AP instead of a gpsimd partition_broadcast; the causal
mask select only touches the 128-wide diagonal slab.
"""

import numpy as np

B, T, C = 4, 2048, 1024
H, D = 16, 64
N_CORES = 8
HPG = H // 2            # heads per core (group)
NCHUNK = 4              # head-pair chunks per core
KT = 8                  # k-tiles of 128 over C
KT_AUG = 9              # + bias/ones k-tile
TT = 4                  # t-tiles of 512 over T
NT = 512                # t tile (matmul N)
VS = 66                 # v column stride per head (64 dims + ones + pad)
VW = HPG * VS           # 528 v columns per k-chunk block
ROPE_BASE = 10000.0

_CACHE = {}


def _build_nc():
    import concourse.bass as bass  # noqa: F401
    import concourse.tile as tile
    from concourse import bacc, mybir
    from contextlib import ExitStack

    f16 = mybir.dt.float16
    f32 = mybir.dt.float32
    f8 = mybir.dt.float8e4
    DR = mybir.MatmulPerfMode.DoubleRow

    nc = bacc.Bacc(
        "TRN2",
        target_bir_lowering=False,
        debug=False,
        enable_asserts=True,
        num_devices=N_CORES,
    )

    xt8_d = nc.dram_tensor("xt8", (KT_AUG * 128, T), f8, kind="ExternalInput").ap()
    xt16_d = nc.dram_tensor("xt16", (KT_AUG * 128, NT), f16, kind="ExternalInput").ap()
    wqk_d = nc.dram_tensor("wqk", (128, KT * 1024), f8, kind="ExternalInput").ap()
    wv8_d = nc.dram_tensor("wv8", (128, KT_AUG * VW), f8, kind="ExternalInput").ap()
    wv16_d = nc.dram_tensor("wv16", (128, KT_AUG * VW), f16, kind="ExternalInput").ap()
    wo_d = nc.dram_tensor("wo", (128, NCHUNK * 1024), f16, kind="ExternalInput").ap()
    bqk_d = nc.dram_tensor("bqk", (128, 16), f32, kind="ExternalInput").ap()
    cs_d = nc.dram_tensor("cs", (128, T), f16, kind="ExternalInput").ap()
    css_d = nc.dram_tensor("css", (128, T), f16, kind="ExternalInput").ap()
    ot_d = nc.dram_tensor("ot", (1024, T), f32, kind="ExternalOutput").ap()

    SHUF = list(range(16, 32)) + list(range(0, 16))

    with tile.TileContext(nc) as tc:
        with ExitStack() as ctx, nc.allow_low_precision("fp8/fp16 attention pipeline"):
            consts = ctx.enter_context(tc.tile_pool(name="consts", bufs=1))
            rtmp = ctx.enter_context(tc.tile_pool(name="rtmp", bufs=4))
            ep_pool = ctx.enter_context(tc.tile_pool(name="ep", bufs=13))
            ed_pool = ctx.enter_context(tc.tile_pool(name="ed", bufs=14))
            ed16_pool = ctx.enter_context(tc.tile_pool(name="ed16", bufs=5))
            small = ctx.enter_context(tc.tile_pool(name="small", bufs=3))
            osb = ctx.enter_context(tc.tile_pool(name="osb", bufs=4))
            ps_q = ctx.enter_context(tc.tile_pool(name="psq", bufs=2, space="PSUM"))
            ps_s = ctx.enter_context(tc.tile_pool(name="pss", bufs=2, space="PSUM"))
            ps_y = ctx.enter_context(tc.tile_pool(name="psy", bufs=1, space="PSUM"))

            # ---- resident tiles + input DMA ----
            # Fused input DMA (SP-issued, ~13 transfers): xt8 t-block 0 and the
            # chunk-0 qk weights land first so RoPE + first scores start early.
            xt8 = consts.tile([128, KT_AUG * T], f8)
            wqk = consts.tile([128, KT * 1024], f8)
            xt16 = consts.tile([128, KT_AUG * NT], f16)
            wv16 = consts.tile([128, KT_AUG * VW], f16)
            wv8 = consts.tile([128, KT_AUG * VW], f8)
            wo = consts.tile([128, NCHUNK * 1024], f16)
            bqk = consts.tile([128, 16], f32)
            cs = consts.tile([128, T], f16)
            css = consts.tile([128, T], f16)

            xt8_r = xt8[:].rearrange("p (k t) -> p k t", k=KT_AUG)
            xt16_r3 = xt16[:].rearrange("p (k t) -> p k t", k=KT_AUG)

            def dma_xt(dst3, dram, nk, width, c0, w):
                # one transfer covering kc 0..nk of dram rows, cols [c0, c0+w)
                nc.sync.dma_start(
                    dst3[:, 0:nk, c0:c0 + w],
                    bass.AP(tensor=dram.tensor, offset=dram.offset + c0,
                            ap=[[width, 128], [128 * width, nk], [1, w]]))

            dma_xt(xt8_r, xt8_d, KT, T, 0, NT)                      # t-block 0
            nc.sync.dma_start(wqk[:, 0:2048], wqk_d[:, 0:2048])     # chunk 0
            nc.sync.dma_start(bqk[:], bqk_d[:])
            nc.sync.dma_start(cs[:], cs_d[:])
            nc.sync.dma_start(css[:], css_d[:])
            dma_xt(xt8_r, xt8_d, KT_AUG, T, NT, T - NT)             # t-blocks 1-3
            nc.sync.dma_start(xt8_r[:, 8:9, 0:NT],
                              bass.AP(tensor=xt8_d.tensor, offset=xt8_d.offset + 8 * 128 * T,
                                      ap=[[T, 128], [1, NT]]))      # aug t-block 0
            dma_xt(xt16_r3, xt16_d, KT_AUG, NT, 0, NT)
            nc.sync.dma_start(wv16[:], wv16_d[:])
            nc.sync.dma_start(wv8[:], wv8_d[:])
            nc.sync.dma_start(wqk[:, 2048:8192], wqk_d[:, 2048:8192])
            nc.sync.dma_start(wo[:], wo_d[:])

            b56 = consts.tile([128, 1], f32)
            nc.gpsimd.memset(b56[:], 56.5)
            v8 = consts.tile([128, 16 * VW], f8)
            v16 = consts.tile([128, 4 * VW], f16)
            y_all = consts.tile([128, NCHUNK * T], f16)

            # double-buffered q/k tiles: cols [0:T) = rope output (slot A),
            # cols [T:2T) = zeros (slot B of the DoubleRow zero-slot trick)
            rqb = [consts.tile([128, 2 * T], f8, name=f"rq{i}") for i in range(2)]
            rkb = [consts.tile([128, 2 * T], f8, name=f"rk{i}") for i in range(2)]
            for t_ in rqb + rkb:
                nc.gpsimd.memset(t_[:, T:2 * T], 0.0)

            wqk_r = wqk[:].rearrange("p (c k u) -> p c k u", c=NCHUNK, k=KT)
            wv8_r = wv8[:].rearrange("p (k w) -> p k w", k=KT_AUG)
            wv16_r = wv16[:].rearrange("p (k w) -> p k w", k=KT_AUG)
            v8_r = v8[:].rearrange("p (m w) -> p m w", m=16)
            v16_r = v16[:].rearrange("p (m w) -> p m w", m=4)

            def emit_vproj8(m_lo, m_hi):
                with nc.named_scope("vproj8"):
                    for m in range(m_lo, m_hi):  # 128-row t-slices
                        psa = ps_q.tile([128, 512], f32, tag="q")
                        psb = ps_s.tile([128, 1024], f32, tag="s")
                        for kp in range(4):
                            lhsT = xt8_r[:, 2 * kp:2 * kp + 2, m * 128:(m + 1) * 128]
                            nc.tensor.matmul(psa[:], lhsT, wv8_r[:, 2 * kp:2 * kp + 2, 0:512],
                                             start=(kp == 0), stop=False, perf_mode=DR)
                            nc.tensor.matmul(psb[:, 0:16], lhsT,
                                             wv8_r[:, 2 * kp:2 * kp + 2, 512:528],
                                             start=(kp == 0), stop=False, perf_mode=DR)
                        lhs8 = xt8_r[:, 8:9, m * 128:(m + 1) * 128]
                        nc.tensor.matmul(psa[:], lhs8, wv8_r[:, 8:9, 0:512],
                                         start=False, stop=True)
                        nc.tensor.matmul(psb[:, 0:16], lhs8, wv8_r[:, 8:9, 512:528],
                                         start=False, stop=True)
                        nc.vector.tensor_copy(v8[:, m * VW: m * VW + 512], psa[:])
                        nc.vector.tensor_copy(v8[:, m * VW + 512:(m + 1) * VW], psb[:, 0:16])

            def emit_vproj16():
                # accurate fp16 V for s < 512 (feeds the t<512 attention)
                with nc.named_scope("vproj16"):
                    for m in range(4):
                        psa = ps_q.tile([128, 512], f32, tag="q")
                        psb = ps_s.tile([128, 1024], f32, tag="s")
                        for kc in range(KT_AUG):
                            lhs = xt16[:, kc * NT + m * 128: kc * NT + (m + 1) * 128]
                            nc.tensor.matmul(psa[:], lhs, wv16_r[:, kc:kc + 1, 0:512],
                                             start=(kc == 0), stop=(kc == KT_AUG - 1))
                            nc.tensor.matmul(psb[:, 0:16], lhs, wv16_r[:, kc:kc + 1, 512:528],
                                             start=(kc == 0), stop=(kc == KT_AUG - 1))
                        nc.vector.tensor_copy(v16[:, m * VW: m * VW + 512], psa[:])
                        nc.vector.tensor_copy(v16[:, m * VW + 512:(m + 1) * VW], psb[:, 0:16])

            def emit_rope(c, tt):
                # q/k projection + RoPE for heads (2c, 2c+1), t-window tt
                rq = rqb[c % 2]
                rk = rkb[c % 2]
                with nc.named_scope("qkrope"):
                    if True:
                        t0 = tt * NT
                        for which, dst in ((0, rq), (1, rk)):
                            ps = ps_q.tile([128, 512], f32, tag="q")
                            u0 = which * 128
                            for kp in range(4):
                                nc.tensor.matmul(
                                    ps[:],
                                    wqk_r[:, c, 2 * kp:2 * kp + 2, u0:u0 + 128],
                                    xt8_r[:, 2 * kp:2 * kp + 2, t0:t0 + NT],
                                    start=(kp == 0), stop=(kp == 3), perf_mode=DR)
                            bcol = bqk[:, c * 4 + which * 2: c * 4 + which * 2 + 1]
                            bswp = bqk[:, c * 4 + which * 2 + 1: c * 4 + which * 2 + 2]
                            s_t = rtmp.tile([128, 512], f32, tag="st")
                            nc.vector.stream_shuffle(s_t[:], ps[:], SHUF)
                            x1 = rtmp.tile([128, 512], f16, tag="x1")
                            nc.vector.scalar_tensor_tensor(
                                out=x1[:], in0=ps[:], scalar=bcol, in1=cs[:, t0:t0 + NT],
                                op0=mybir.AluOpType.add, op1=mybir.AluOpType.mult)
                            x2 = rtmp.tile([128, 512], f16, tag="x2")
                            nc.vector.scalar_tensor_tensor(
                                out=x2[:], in0=s_t[:], scalar=bswp, in1=css[:, t0:t0 + NT],
                                op0=mybir.AluOpType.add, op1=mybir.AluOpType.mult)
                            nc.gpsimd.tensor_add(dst[:, t0:t0 + NT], x1[:], x2[:])

            def emit_scores(c, tt):
                # scores + exp (+ diagonal mask) for window (c, tt).
                # Returns the e-tiles for emit_attv.
                rq_r = rqb[c % 2][:].rearrange("p (s t) -> p s t", s=2)
                rk_r = rkb[c % 2][:].rearrange("p (s t) -> p s t", s=2)
                t0 = tt * NT
                n_full = t0 // 128
                epairs, ediags = [], []
                with nc.named_scope("scores"):
                    for ip in range(n_full // 2):
                        epair = ep_pool.tile([128, 2048], f8)
                        for j in range(2):
                            sc = 2 * ip + j
                            s0 = sc * 128
                            sp = ps_s.tile([128, 1024], f32, tag="s")
                            for h in range(2):
                                nc.tensor.matmul(
                                    sp[:, h * NT:(h + 1) * NT],
                                    rk_r[h * 64:(h + 1) * 64, :, s0:s0 + 128],
                                    rq_r[h * 64:(h + 1) * 64, :, t0:t0 + NT],
                                    start=True, stop=True, perf_mode=DR)
                            if tt == 3 and ip >= 4:
                                # fp8e4m3 bit grid is linear in log2, so one
                                # DVE op emits exp(s/8) bits: b = 1.4427*s+56.5
                                nc.vector.scalar_tensor_tensor(
                                    out=epair[:, j * 1024:(j + 1) * 1024].bitcast(mybir.dt.int8),
                                    in0=sp[:], scalar=1.4426950,
                                    in1=b56[:, 0:1].broadcast_to([128, 1024]),
                                    op0=mybir.AluOpType.mult,
                                    op1=mybir.AluOpType.add)
                            else:
                                nc.scalar.activation(
                                    epair[:, j * 1024:(j + 1) * 1024], sp[:],
                                    mybir.ActivationFunctionType.Exp,
                                    bias=0.0, scale=0.125)
                        epairs.append(epair)
                    for d in range(4):
                        sc = n_full + d
                        s0 = sc * 128
                        dlt = d * 128
                        sp = ps_s.tile([128, 1024], f32, tag="s")
                        for h in range(2):
                            nc.tensor.matmul(
                                sp[:, h * NT + dlt:(h + 1) * NT],
                                rk_r[h * 64:(h + 1) * 64, :, s0:s0 + 128],
                                rq_r[h * 64:(h + 1) * 64, :, t0 + dlt:t0 + NT],
                                start=True, stop=True, perf_mode=DR)
                        e_t = (ed16_pool.tile([128, 1024], f16, name="e16")
                               if tt == 0 else ed_pool.tile([128, 1024], f8, name="e8"))
                        s3 = sp[:].rearrange("p (a b) -> p a b", a=2)[:, :, dlt:]
                        e3 = e_t[:].rearrange("p (a b) -> p a b", a=2)[:, :, dlt:]
                        nc.scalar.activation(e3, s3, mybir.ActivationFunctionType.Exp,
                                             bias=0.0, scale=0.125)
                        # causal mask on the 128-wide diagonal slab only:
                        # within the slab keep iff col >= partition
                        nc.gpsimd.affine_select(
                            out=e3[:, :, 0:128], in_=e3[:, :, 0:128],
                            compare_op=mybir.AluOpType.is_ge,
                            fill=0.0, base=0,
                            pattern=[[0, 2], [1, 128]], channel_multiplier=-1)
                        ediags.append(e_t)
                return epairs, ediags

            def emit_attv(c, tt, epairs, ediags, tail=False):
                # att@V accumulation + normalization for window (c, tt)
                t0 = tt * NT
                n_full = t0 // 128
                with nc.named_scope("attv"):
                    yp = ps_y.tile([65, 1024], f32, tag="y")
                    for ip, epair in enumerate(epairs):
                        epair_r = epair[:].rearrange("p (s x) -> p s x", s=2)
                        for h in range(2):
                            vcol = VS * (2 * c + h)
                            nc.tensor.matmul(
                                yp[:, h * NT:(h + 1) * NT],
                                v8_r[:, 2 * ip:2 * ip + 2, vcol:vcol + 65],
                                epair_r[:, :, h * NT:(h + 1) * NT],
                                start=(ip == 0), stop=False, perf_mode=DR,
                                skip_group_check=True)
                    vsrc = v16_r if tt == 0 else v8_r
                    for d, e_t in enumerate(ediags):
                        sc = n_full + d
                        dlt = d * 128
                        for h in range(2):
                            vcol = VS * (2 * c + h)
                            nc.tensor.matmul(
                                yp[:, h * NT + dlt:(h + 1) * NT],
                                vsrc[:, sc:sc + 1, vcol:vcol + 65],
                                e_t[:, h * NT + dlt:(h + 1) * NT],
                                start=(tt == 0 and d == 0), stop=(d == 3),
                                skip_group_check=True)

                    # normalization: y *= 1/den. The denominator row is
                    # reshaped to [128,8] by DMA so reciprocal costs free=8,
                    # then reshaped back and partition-broadcast by DMA.
                    # (tail=True keeps the short direct chain for the final
                    # window, where DMA latency would sit on the critical path)
                    yc = small.tile([65, 1024], f16, tag="yc")
                    nc.vector.tensor_copy(yc[:], yp[:])
                    rd = small.tile([1, 1024], f16, tag="rd")
                    if tail:
                        nc.vector.reciprocal(rd[:], yc[64:65, :])
                    else:
                        dT = small.tile([128, 8], f16, tag="dT")
                        nc.sync.dma_start(out=dT[:], in_=yc[64:65, :])
                        rdT = small.tile([128, 8], f16, tag="rdT")
                        nc.vector.reciprocal(rdT[:], dT[:])
                        nc.sync.dma_start(out=rd[:], in_=rdT[:])
                    dbc = small.tile([64, 1024], f16, tag="dbc")
                    rda = rd[0:1, :]
                    nc.sync.dma_start(out=dbc[:], in_=bass.AP(
                        tensor=rda.tensor, offset=rda.offset,
                        ap=[list(rda.ap)[0], [0, 64], [1, 1024]]))
                    for h in range(2):
                        nc.vector.tensor_mul(
                            y_all[h * 64:(h + 1) * 64, c * T + t0: c * T + t0 + NT],
                            yc[0:64, h * NT:(h + 1) * NT],
                            dbc[:, h * NT:(h + 1) * NT])

            def emit_oproj(tt, cs_=tuple(range(NCHUNK)), dram=None, act_copy=False,
                           wide=False):
                # output projection for t-window tt (all head-chunks of core)
                t0 = tt * NT
                dram = ot_d if dram is None else dram
                with nc.named_scope("oproj"):
                    for ct in range(8):
                        if wide and ct % 2 == 1:
                            po = ps_s.tile([128, 512], f32, tag="s", name="po")
                        else:
                            po = ps_q.tile([128, 512], f32, tag="q", name="po")
                        for c in cs_:
                            nc.tensor.matmul(po[:], wo[:, c * 1024 + ct * 128: c * 1024 + ct * 128 + 128],
                                             y_all[:, c * T + t0: c * T + t0 + NT],
                                             start=(c == cs_[0]), stop=(c == cs_[-1]))
                        ob = osb.tile([128, 512], f32)
                        if act_copy:
                            nc.scalar.activation(ob[:], po[:],
                                                 mybir.ActivationFunctionType.Copy)
                        else:
                            nc.vector.tensor_copy(ob[:], po[:])
                        oq = nc.sync
                        oq.dma_start(ot_d[ct * 128:(ct + 1) * 128, t0:t0 + NT], ob[:])

            # software pipeline over windows w=(c,tt): rope of windows[i+1]
            # and scores+exp of windows[i] stream on PE/Act while attV+norm of
            # windows[i-1] ride behind; v/out-projections interleave early/late.
            # Chunk 3 runs tt descending so the final oproj tail is smallest.
            windows = [(c, tt) for c in range(NCHUNK - 1) for tt in range(TT)]
            windows += [(NCHUNK - 1, tt) for tt in (3, 2, 1, 0)]
            emit_rope(*windows[0])
            pend = None
            for i, (c, tt) in enumerate(windows):
                if i + 1 < len(windows):
                    emit_rope(*windows[i + 1])
                et = emit_scores(c, tt)
                if i == 1:
                    emit_vproj16()
                elif i == 2:
                    emit_vproj8(0, 8)
                elif i == 3:
                    emit_vproj8(8, 16)
                if pend is not None:
                    pc, ptt, pet = pend
                    emit_attv(pc, ptt, *pet)
                    if pc == NCHUNK - 1:
                        emit_oproj(ptt, act_copy=(ptt <= 1))
                pend = (c, tt, et)
            pc, ptt, pet = pend
            emit_attv(pc, ptt, *pet, tail=True)
            emit_oproj(ptt, act_copy=True, wide=True)

    nc.compile()
    return nc


def _prep_inputs(x, qkv_w, qkv_b):
    """Build the 8 per-core input maps (all host-side numpy)."""
    import ml_dtypes
    f8 = ml_dtypes.float8_e4m3

    x = np.asarray(x, dtype=np.float32)
    qkv_w = np.asarray(qkv_w, dtype=np.float32)
    qkv_b = np.asarray(qkv_b, dtype=np.float32)

    # xt per batch: (KT_AUG*128, T) with row 1024 = ones, rest of aug block 0
    xt8s, xt16s = [], []
    for b in range(B):
        xa = np.zeros((KT_AUG * 128, T), dtype=np.float32)
        xa[:C] = x[b].T
        xa[C] = 1.0
        xt8s.append(xa.astype(f8))
        xt16s.append(xa[:, :NT].astype(np.float16))

    r = np.arange(64)
    d_r = 2 * ((r // 32) * 16 + (r % 16)) + ((r % 32) >= 16)  # row -> head dim
    p = np.arange(128)
    f_p = ((p // 32) % 2) * 16 + (p % 16)

    ins_g = []
    for g in range(2):
        # wqk: [p, c*2048 + kc*256 + which*128 + m] (chunk-major)
        wqk = np.empty((128, KT * 1024), dtype=f8)
        bqk = np.empty((128, 16), dtype=np.float32)
        for c in range(NCHUNK):
            for which in range(2):  # 0=q, 1=k
                rows = np.concatenate([
                    which * C + (8 * g + 2 * c + hh) * 64 + d_r for hh in range(2)
                ])  # 128 feature rows
                blk = qkv_w[rows, :]          # (128 feat, 1024 k)
                for kc in range(KT):
                    sl = slice(c * 2048 + kc * 256 + which * 128,
                               c * 2048 + kc * 256 + which * 128 + 128)
                    wqk[:, sl] = blk[:, kc * 128:(kc + 1) * 128].T.astype(f8)
                bc = qkv_b[rows].astype(np.float32)
                bqk[:, c * 4 + which * 2] = bc
                bqk[:, c * 4 + which * 2 + 1] = bc[p ^ 16]
        # wv: [p, kc*VW + col], col = VS*h + j
        wva = np.zeros((KT_AUG * 128, VW), dtype=np.float32)
        for h in range(HPG):
            rows = 2 * C + (8 * g + h) * 64 + np.arange(64)
            wva[:C, VS * h: VS * h + 64] = qkv_w[rows, :].T
            wva[C, VS * h: VS * h + 64] = qkv_b[rows]
            wva[C, VS * h + 64] = 1.0
        wv8 = np.empty((128, KT_AUG * VW), dtype=f8)
        wv16 = np.empty((128, KT_AUG * VW), dtype=np.float16)
        for kc in range(KT_AUG):
            wv8[:, kc * VW:(kc + 1) * VW] = wva[kc * 128:(kc + 1) * 128].astype(f8)
            wv16[:, kc * VW:(kc + 1) * VW] = wva[kc * 128:(kc + 1) * 128].astype(np.float16)
        ins_g.append((wqk, bqk, wv8, wv16))

    # rope tables
    inv_freq = (1.0 / (ROPE_BASE ** (np.arange(0, D, 2) / D))).astype(np.float64)
    t = np.arange(T, dtype=np.float64)
    ang = t[None, :] * inv_freq[f_p][:, None]          # (128, T)
    cs = np.cos(ang).astype(np.float16)
    sgn = np.where((p % 32) < 16, -1.0, 1.0)[:, None]
    css = (sgn * np.sin(ang)).astype(np.float16)

    return xt8s, xt16s, ins_g, cs, css


def _prep_wo(out_w, g):
    out_w = np.asarray(out_w, dtype=np.float32)
    wo = np.empty((128, NCHUNK * 1024), dtype=np.float16)
    for c in range(NCHUNK):
        rows = np.concatenate([(8 * g + 2 * c + hh) * 64 + np.arange(64) for hh in range(2)])
        wo[:, c * 1024:(c + 1) * 1024] = out_w[:, rows].astype(np.float16).T
    return wo


def _in_maps(x, qkv_w, qkv_b, out_w):
    xt8s, xt16s, ins_g, cs, css = _prep_inputs(x, qkv_w, qkv_b)
    wos = [_prep_wo(out_w, g) for g in range(2)]
    in_maps = []
    for core in range(N_CORES):
        b, g = core // 2, core % 2
        wqk, bqk, wv8, wv16 = ins_g[g]
        in_maps.append({
            "xt8": xt8s[b], "xt16": xt16s[b], "wqk": wqk,
            "wv8": wv8, "wv16": wv16, "wo": wos[g],
            "bqk": bqk, "cs": cs, "css": css,
        })
    return in_maps


def kernel(x, qkv_w, qkv_b, out_w, out_b):
    from concourse.bass_utils import run_bass_kernel_spmd

    if "nc" not in _CACHE:
        _CACHE["nc"] = _build_nc()
    nc = _CACHE["nc"]

    in_maps = _in_maps(x, qkv_w, qkv_b, out_w)
    out_b = np.asarray(out_b, dtype=np.float32)

    try:
        res = run_bass_kernel_spmd(nc, in_maps, core_ids=list(range(N_CORES)))
    except ModuleNotFoundError:
        # BASS_TRACE set but the NTFF profile hook isn't importable here
        import os
        os.environ["BASS_NEVER_TRACE"] = "1"
        res = run_bass_kernel_spmd(nc, in_maps, core_ids=list(range(N_CORES)))

    out = np.empty((B, T, C), dtype=np.float32)
    for b in range(B):
        pt = res.results[2 * b]["ot"] + res.results[2 * b + 1]["ot"]  # (C, T)
        out[b] = pt.T + out_b[None, :]
    return out


# revision 3
# speedup vs baseline: 1.0036x; 1.0036x over previous
"""Causal self-attention (B=4, T=2048, C=1024, H=16, D=64) on 8 TRN2 NeuronCores.

Sharding: core = (batch b, head-group g) with b = core // 2, g = core % 2.
Each core computes heads [8g, 8g+8) of batch b and produces the partial
out-projection (C, T) for its head group; the host sums the two head-group
partials per batch and adds the output bias.

v2 performance structure:
- fp8e4m3 DoubleRow matmuls (2 contraction k-tiles per instruction) for the
  qk-projection, v-projection, scores and att@V. Scores contract over d=64
  only, so their second DoubleRow slot is a zeroed region of the rq/rk
  tiles; att@V pairs adjacent s-tiles. The out-projection and the first
  t-window's attention (t < 512, where softmax averaging is too weak to
  wash out fp8 noise) stay fp16.
- Software pipeline over windows w=(c, tt): rope of windows[i+1] and
  scores+exp of windows[i] stream on PE/Act while attV+norm of windows[i-1]
  ride behind, so the in-order engines never head-of-line block the exp
  stream. V/out-projections are interleaved where they fit; chunk 3 runs
  tt descending so the final out-projection tail is the smallest window.
- exp for the late (most softmax-diffuse) windows is computed on DVE as a
  single scalar_tensor_tensor emitting fp8e4m3 BIT PATTERNS directly
  (the fp8 bit grid is linear in log2, so bits = 1.4427*score + 56.5).
- Softmax denominator: reciprocal on a DMA-transposed [128,8] layout
  (free-dim cost 8 instead of 1024), then DMA partition-broadcast.
- Causal masking only touches the 128-wide diagonal slab of each e-tile
  (gpsimd affine_select); fully-masked tiles are never computed.
- All input DMA is fused into ~13 large multi-dim transfers issued from the
  idle SP sequencer, ordered so RoPE/scores inputs land first.
"""

import numpy as np

B, T, C = 4, 2048, 1024
H, D = 16, 64
N_CORES = 8
HPG = H // 2            # heads per core (group)
NCHUNK = 4              # head-pair chunks per core
KT = 8                  # k-tiles of 128 over C
KT_AUG = 9              # + bias/ones k-tile
TT = 4                  # t-tiles of 512 over T
NT = 512                # t tile (matmul N)
VS = 66                 # v column stride per head (64 dims + ones + pad)
VW = HPG * VS           # 528 v columns per k-chunk block
ROPE_BASE = 10000.0

_CACHE = {}


def _build_nc():
    import concourse.bass as bass  # noqa: F401
    import concourse.tile as tile
    from concourse import bacc, mybir
    from contextlib import ExitStack

    f16 = mybir.dt.float16
    f32 = mybir.dt.float32
    f8 = mybir.dt.float8e4
    DR = mybir.MatmulPerfMode.DoubleRow

    nc = bacc.Bacc(
        "TRN2",
        target_bir_lowering=False,
        debug=False,
        enable_asserts=True,
        num_devices=N_CORES,
    )

    xt8_d = nc.dram_tensor("xt8", (KT_AUG * 128, T), f8, kind="ExternalInput").ap()
    xt16_d = nc.dram_tensor("xt16", (KT_AUG * 128, NT), f16, kind="ExternalInput").ap()
    wqk_d = nc.dram_tensor("wqk", (128, KT * 1024), f8, kind="ExternalInput").ap()
    wv8_d = nc.dram_tensor("wv8", (128, KT_AUG * VW), f8, kind="ExternalInput").ap()
    wv16_d = nc.dram_tensor("wv16", (128, KT_AUG * VW), f16, kind="ExternalInput").ap()
    wo_d = nc.dram_tensor("wo", (128, NCHUNK * 1024), f16, kind="ExternalInput").ap()
    bqk_d = nc.dram_tensor("bqk", (128, 16), f32, kind="ExternalInput").ap()
    cs_d = nc.dram_tensor("cs", (128, T), f16, kind="ExternalInput").ap()
    css_d = nc.dram_tensor("css", (128, T), f16, kind="ExternalInput").ap()
    ot_d = nc.dram_tensor("ot", (1024, T), f32, kind="ExternalOutput").ap()

    SHUF = list(range(16, 32)) + list(range(0, 16))

    with tile.TileContext(nc) as tc:
        with ExitStack() as ctx, nc.allow_low_precision("fp8/fp16 attention pipeline"):
            consts = ctx.enter_context(tc.tile_pool(name="consts", bufs=1))
            rtmp = ctx.enter_context(tc.tile_pool(name="rtmp", bufs=4))
            ep_pool = ctx.enter_context(tc.tile_pool(name="ep", bufs=13))
            ed_pool = ctx.enter_context(tc.tile_pool(name="ed", bufs=14))
            ed16_pool = ctx.enter_context(tc.tile_pool(name="ed16", bufs=5))
            small = ctx.enter_context(tc.tile_pool(name="small", bufs=3))
            osb = ctx.enter_context(tc.tile_pool(name="osb", bufs=4))
            ps_q = ctx.enter_context(tc.tile_pool(name="psq", bufs=2, space="PSUM"))
            ps_s = ctx.enter_context(tc.tile_pool(name="pss", bufs=2, space="PSUM"))
            ps_y = ctx.enter_context(tc.tile_pool(name="psy", bufs=1, space="PSUM"))

            # ---- resident tiles + input DMA ----
            # Fused input DMA (SP-issued, ~13 transfers): xt8 t-block 0 and the
            # chunk-0 qk weights land first so RoPE + first scores start early.
            xt8 = consts.tile([128, KT_AUG * T], f8)
            wqk = consts.tile([128, KT * 1024], f8)
            xt16 = consts.tile([128, KT_AUG * NT], f16)
            wv16 = consts.tile([128, KT_AUG * VW], f16)
            wv8 = consts.tile([128, KT_AUG * VW], f8)
            wo = consts.tile([128, NCHUNK * 1024], f16)
            bqk = consts.tile([128, 16], f32)
            cs = consts.tile([128, T], f16)
            css = consts.tile([128, T], f16)

            xt8_r = xt8[:].rearrange("p (k t) -> p k t", k=KT_AUG)
            xt16_r3 = xt16[:].rearrange("p (k t) -> p k t", k=KT_AUG)

            def dma_xt(dst3, dram, nk, width, c0, w):
                # one transfer covering kc 0..nk of dram rows, cols [c0, c0+w)
                nc.sync.dma_start(
                    dst3[:, 0:nk, c0:c0 + w],
                    bass.AP(tensor=dram.tensor, offset=dram.offset + c0,
                            ap=[[width, 128], [128 * width, nk], [1, w]]))

            dma_xt(xt8_r, xt8_d, KT, T, 0, NT)                      # t-block 0
            nc.sync.dma_start(wqk[:, 0:2048], wqk_d[:, 0:2048])     # chunk 0
            nc.sync.dma_start(bqk[:], bqk_d[:])
            nc.sync.dma_start(cs[:], cs_d[:])
            nc.sync.dma_start(css[:], css_d[:])
            dma_xt(xt8_r, xt8_d, KT_AUG, T, NT, T - NT)             # t-blocks 1-3
            nc.sync.dma_start(xt8_r[:, 8:9, 0:NT],
                              bass.AP(tensor=xt8_d.tensor, offset=xt8_d.offset + 8 * 128 * T,
                                      ap=[[T, 128], [1, NT]]))      # aug t-block 0
            dma_xt(xt16_r3, xt16_d, KT_AUG, NT, 0, NT)
            nc.sync.dma_start(wv16[:], wv16_d[:])
            nc.sync.dma_start(wv8[:], wv8_d[:])
            nc.sync.dma_start(wqk[:, 2048:8192], wqk_d[:, 2048:8192])
            nc.sync.dma_start(wo[:], wo_d[:])

            b56 = consts.tile([128, 1], f32)
            nc.gpsimd.memset(b56[:], 56.5)
            v8 = consts.tile([128, 16 * VW], f8)
            v16 = consts.tile([128, 4 * VW], f16)
            y_all = consts.tile([128, NCHUNK * T], f16)

            # double-buffered q/k tiles: cols [0:T) = rope output (slot A),
            # cols [T:2T) = zeros (slot B of the DoubleRow zero-slot trick)
            rqb = [consts.tile([128, 2 * T], f8, name=f"rq{i}") for i in range(2)]
            rkb = [consts.tile([128, 2 * T], f8, name=f"rk{i}") for i in range(2)]
            for t_ in rqb + rkb:
                nc.gpsimd.memset(t_[:, T:2 * T], 0.0)

            wqk_r = wqk[:].rearrange("p (c k u) -> p c k u", c=NCHUNK, k=KT)
            wv8_r = wv8[:].rearrange("p (k w) -> p k w", k=KT_AUG)
            wv16_r = wv16[:].rearrange("p (k w) -> p k w", k=KT_AUG)
            v8_r = v8[:].rearrange("p (m w) -> p m w", m=16)
            v16_r = v16[:].rearrange("p (m w) -> p m w", m=4)

            def emit_vproj8(m_lo, m_hi):
                with nc.named_scope("vproj8"):
                    for m in range(m_lo, m_hi):  # 128-row t-slices
                        psa = ps_q.tile([128, 512], f32, tag="q")
                        psb = ps_s.tile([128, 1024], f32, tag="s")
                        for kp in range(4):
                            lhsT = xt8_r[:, 2 * kp:2 * kp + 2, m * 128:(m + 1) * 128]
                            nc.tensor.matmul(psa[:], lhsT, wv8_r[:, 2 * kp:2 * kp + 2, 0:512],
                                             start=(kp == 0), stop=False, perf_mode=DR)
                            nc.tensor.matmul(psb[:, 0:16], lhsT,
                                             wv8_r[:, 2 * kp:2 * kp + 2, 512:528],
                                             start=(kp == 0), stop=False, perf_mode=DR)
                        lhs8 = xt8_r[:, 8:9, m * 128:(m + 1) * 128]
                        nc.tensor.matmul(psa[:], lhs8, wv8_r[:, 8:9, 0:512],
                                         start=False, stop=True)
                        nc.tensor.matmul(psb[:, 0:16], lhs8, wv8_r[:, 8:9, 512:528],
                                         start=False, stop=True)
                        nc.vector.tensor_copy(v8[:, m * VW: m * VW + 512], psa[:])
                        nc.vector.tensor_copy(v8[:, m * VW + 512:(m + 1) * VW], psb[:, 0:16])

            def emit_vproj16():
                # accurate fp16 V for s < 512 (feeds the t<512 attention)
                with nc.named_scope("vproj16"):
                    for m in range(4):
                        psa = ps_q.tile([128, 512], f32, tag="q")
                        psb = ps_s.tile([128, 1024], f32, tag="s")
                        for kc in range(KT_AUG):
                            lhs = xt16[:, kc * NT + m * 128: kc * NT + (m + 1) * 128]
                            nc.tensor.matmul(psa[:], lhs, wv16_r[:, kc:kc + 1, 0:512],
                                             start=(kc == 0), stop=(kc == KT_AUG - 1))
                            nc.tensor.matmul(psb[:, 0:16], lhs, wv16_r[:, kc:kc + 1, 512:528],
                                             start=(kc == 0), stop=(kc == KT_AUG - 1))
                        nc.vector.tensor_copy(v16[:, m * VW: m * VW + 512], psa[:])
                        nc.vector.tensor_copy(v16[:, m * VW + 512:(m + 1) * VW], psb[:, 0:16])

            def emit_rope(c, tt):
                # q/k projection + RoPE for heads (2c, 2c+1), t-window tt
                rq = rqb[c % 2]
                rk = rkb[c % 2]
                with nc.named_scope("qkrope"):
                    if True:
                        t0 = tt * NT
                        for which, dst in ((0, rq), (1, rk)):
                            ps = ps_q.tile([128, 512], f32, tag="q")
                            u0 = which * 128
                            for kp in range(4):
                                nc.tensor.matmul(
                                    ps[:],
                                    wqk_r[:, c, 2 * kp:2 * kp + 2, u0:u0 + 128],
                                    xt8_r[:, 2 * kp:2 * kp + 2, t0:t0 + NT],
                                    start=(kp == 0), stop=(kp == 3), perf_mode=DR)
                            bcol = bqk[:, c * 4 + which * 2: c * 4 + which * 2 + 1]
                            bswp = bqk[:, c * 4 + which * 2 + 1: c * 4 + which * 2 + 2]
                            s_t = rtmp.tile([128, 512], f32, tag="st")
                            nc.vector.stream_shuffle(s_t[:], ps[:], SHUF)
                            x1 = rtmp.tile([128, 512], f16, tag="x1")
                            nc.vector.scalar_tensor_tensor(
                                out=x1[:], in0=ps[:], scalar=bcol, in1=cs[:, t0:t0 + NT],
                                op0=mybir.AluOpType.add, op1=mybir.AluOpType.mult)
                            x2 = rtmp.tile([128, 512], f16, tag="x2")
                            nc.vector.scalar_tensor_tensor(
                                out=x2[:], in0=s_t[:], scalar=bswp, in1=css[:, t0:t0 + NT],
                                op0=mybir.AluOpType.add, op1=mybir.AluOpType.mult)
                            nc.gpsimd.tensor_add(dst[:, t0:t0 + NT], x1[:], x2[:])

            def emit_scores(c, tt):
                # scores + exp (+ diagonal mask) for window (c, tt).
                # Returns the e-tiles for emit_attv.
                rq_r = rqb[c % 2][:].rearrange("p (s t) -> p s t", s=2)
                rk_r = rkb[c % 2][:].rearrange("p (s t) -> p s t", s=2)
                t0 = tt * NT
                n_full = t0 // 128
                epairs, ediags = [], []
                with nc.named_scope("scores"):
                    for ip in range(n_full // 2):
                        epair = ep_pool.tile([128, 2048], f8)
                        for j in range(2):
                            sc = 2 * ip + j
                            s0 = sc * 128
                            sp = ps_s.tile([128, 1024], f32, tag="s")
                            for h in range(2):
                                nc.tensor.matmul(
                                    sp[:, h * NT:(h + 1) * NT],
                                    rk_r[h * 64:(h + 1) * 64, :, s0:s0 + 128],
                                    rq_r[h * 64:(h + 1) * 64, :, t0:t0 + NT],
                                    start=True, stop=True, perf_mode=DR)
                            if tt == 3 and ip >= 4:
                                # fp8e4m3 bit grid is linear in log2, so one
                                # DVE op emits exp(s/8) bits: b = 1.4427*s+56.5
                                nc.vector.scalar_tensor_tensor(
                                    out=epair[:, j * 1024:(j + 1) * 1024].bitcast(mybir.dt.int8),
                                    in0=sp[:], scalar=1.4426950,
                                    in1=b56[:, 0:1].broadcast_to([128, 1024]),
                                    op0=mybir.AluOpType.mult,
                                    op1=mybir.AluOpType.add)
                            else:
                                nc.scalar.activation(
                                    epair[:, j * 1024:(j + 1) * 1024], sp[:],
                                    mybir.ActivationFunctionType.Exp,
                                    bias=0.0, scale=0.125)
                        epairs.append(epair)
                    for d in range(4):
                        sc = n_full + d
                        s0 = sc * 128
                        dlt = d * 128
                        sp = ps_s.tile([128, 1024], f32, tag="s")
                        for h in range(2):
                            nc.tensor.matmul(
                                sp[:, h * NT + dlt:(h + 1) * NT],
                                rk_r[h * 64:(h + 1) * 64, :, s0:s0 + 128],
                                rq_r[h * 64:(h + 1) * 64, :, t0 + dlt:t0 + NT],
                                start=True, stop=True, perf_mode=DR)
                        e_t = (ed16_pool.tile([128, 1024], f16, name="e16")
                               if tt == 0 else ed_pool.tile([128, 1024], f8, name="e8"))
                        s3 = sp[:].rearrange("p (a b) -> p a b", a=2)[:, :, dlt:]
                        e3 = e_t[:].rearrange("p (a b) -> p a b", a=2)[:, :, dlt:]
                        nc.scalar.activation(e3, s3, mybir.ActivationFunctionType.Exp,
                                             bias=0.0, scale=0.125)
                        # causal mask on the 128-wide diagonal slab only:
                        # within the slab keep iff col >= partition
                        nc.gpsimd.affine_select(
                            out=e3[:, :, 0:128], in_=e3[:, :, 0:128],
                            compare_op=mybir.AluOpType.is_ge,
                            fill=0.0, base=0,
                            pattern=[[0, 2], [1, 128]], channel_multiplier=-1)
                        ediags.append(e_t)
                return epairs, ediags

            def emit_attv(c, tt, epairs, ediags, tail=False):
                # att@V accumulation + normalization for window (c, tt)
                t0 = tt * NT
                n_full = t0 // 128
                with nc.named_scope("attv"):
                    yp = ps_y.tile([65, 1024], f32, tag="y")
                    for ip, epair in enumerate(epairs):
                        epair_r = epair[:].rearrange("p (s x) -> p s x", s=2)
                        for h in range(2):
                            vcol = VS * (2 * c + h)
                            nc.tensor.matmul(
                                yp[:, h * NT:(h + 1) * NT],
                                v8_r[:, 2 * ip:2 * ip + 2, vcol:vcol + 65],
                                epair_r[:, :, h * NT:(h + 1) * NT],
                                start=(ip == 0), stop=False, perf_mode=DR,
                                skip_group_check=True)
                    vsrc = v16_r if tt == 0 else v8_r
                    for d, e_t in enumerate(ediags):
                        sc = n_full + d
                        dlt = d * 128
                        for h in range(2):
                            vcol = VS * (2 * c + h)
                            nc.tensor.matmul(
                                yp[:, h * NT + dlt:(h + 1) * NT],
                                vsrc[:, sc:sc + 1, vcol:vcol + 65],
                                e_t[:, h * NT + dlt:(h + 1) * NT],
                                start=(tt == 0 and d == 0), stop=(d == 3),
                                skip_group_check=True)

                    # normalization: y *= 1/den. The denominator row is
                    # reshaped to [128,8] by DMA so reciprocal costs free=8,
                    # then reshaped back and partition-broadcast by DMA.
                    # (tail=True keeps the short direct chain for the final
                    # window, where DMA latency would sit on the critical path)
                    yc = small.tile([65, 1024], f16, tag="yc")
                    nc.vector.tensor_copy(yc[:], yp[:])
                    rd = small.tile([1, 1024], f16, tag="rd")
                    if tail:
                        nc.vector.reciprocal(rd[:], yc[64:65, :])
                    else:
                        dT = small.tile([128, 8], f16, tag="dT")
                        nc.sync.dma_start(out=dT[:], in_=yc[64:65, :])
                        rdT = small.tile([128, 8], f16, tag="rdT")
                        nc.vector.reciprocal(rdT[:], dT[:])
                        nc.sync.dma_start(out=rd[:], in_=rdT[:])
                    dbc = small.tile([64, 1024], f16, tag="dbc")
                    rda = rd[0:1, :]
                    nc.sync.dma_start(out=dbc[:], in_=bass.AP(
                        tensor=rda.tensor, offset=rda.offset,
                        ap=[list(rda.ap)[0], [0, 64], [1, 1024]]))
                    for h in range(2):
                        nc.vector.tensor_mul(
                            y_all[h * 64:(h + 1) * 64, c * T + t0: c * T + t0 + NT],
                            yc[0:64, h * NT:(h + 1) * NT],
                            dbc[:, h * NT:(h + 1) * NT])

            def emit_oproj(tt, cs_=tuple(range(NCHUNK)), dram=None, act_copy=False,
                           wide=False):
                # output projection for t-window tt (all head-chunks of core)
                t0 = tt * NT
                dram = ot_d if dram is None else dram
                with nc.named_scope("oproj"):
                    for ct in range(8):
                        if wide and ct % 2 == 1:
                            po = ps_s.tile([128, 512], f32, tag="s", name="po")
                        else:
                            po = ps_q.tile([128, 512], f32, tag="q", name="po")
                        for c in cs_:
                            nc.tensor.matmul(po[:], wo[:, c * 1024 + ct * 128: c * 1024 + ct * 128 + 128],
                                             y_all[:, c * T + t0: c * T + t0 + NT],
                                             start=(c == cs_[0]), stop=(c == cs_[-1]))
                        ob = osb.tile([128, 512], f32)
                        if act_copy:
                            nc.scalar.activation(ob[:], po[:],
                                                 mybir.ActivationFunctionType.Copy)
                        else:
                            nc.vector.tensor_copy(ob[:], po[:])
                        oq = nc.sync
                        oq.dma_start(ot_d[ct * 128:(ct + 1) * 128, t0:t0 + NT], ob[:])

            # software pipeline over windows w=(c,tt): rope of windows[i+1]
            # and scores+exp of windows[i] stream on PE/Act while attV+norm of
            # windows[i-1] ride behind; v/out-projections interleave early/late.
            # Chunk 3 runs tt descending so the final oproj tail is smallest.
            windows = [(c, tt) for c in range(NCHUNK - 1) for tt in range(TT)]
            windows += [(NCHUNK - 1, tt) for tt in (3, 2, 1, 0)]
            emit_rope(*windows[0])
            pend = None
            for i, (c, tt) in enumerate(windows):
                if i + 1 < len(windows):
                    emit_rope(*windows[i + 1])
                et = emit_scores(c, tt)
                if i == 1:
                    emit_vproj16()
                elif i == 2:
                    emit_vproj8(0, 8)
                elif i == 3:
                    emit_vproj8(8, 16)
                if pend is not None:
                    pc, ptt, pet = pend
                    emit_attv(pc, ptt, *pet)
                    if pc == NCHUNK - 1:
                        emit_oproj(ptt, act_copy=(ptt <= 1))
                pend = (c, tt, et)
            pc, ptt, pet = pend
            emit_attv(pc, ptt, *pet, tail=True)
            emit_oproj(ptt, act_copy=True, wide=True)

    nc.compile()
    return nc


def _prep_inputs(x, qkv_w, qkv_b):
    """Build the 8 per-core input maps (all host-side numpy)."""
    import ml_dtypes
    f8 = ml_dtypes.float8_e4m3

    x = np.asarray(x, dtype=np.float32)
    qkv_w = np.asarray(qkv_w, dtype=np.float32)
    qkv_b = np.asarray(qkv_b, dtype=np.float32)

    # xt per batch: (KT_AUG*128, T) with row 1024 = ones, rest of aug block 0
    xt8s, xt16s = [], []
    for b in range(B):
        xa = np.zeros((KT_AUG * 128, T), dtype=np.float32)
        xa[:C] = x[b].T
        xa[C] = 1.0
        xt8s.append(xa.astype(f8))
        xt16s.append(xa[:, :NT].astype(np.float16))

    r = np.arange(64)
    d_r = 2 * ((r // 32) * 16 + (r % 16)) + ((r % 32) >= 16)  # row -> head dim
    p = np.arange(128)
    f_p = ((p // 32) % 2) * 16 + (p % 16)

    ins_g = []
    for g in range(2):
        # wqk: [p, c*2048 + kc*256 + which*128 + m] (chunk-major)
        wqk = np.empty((128, KT * 1024), dtype=f8)
        bqk = np.empty((128, 16), dtype=np.float32)
        for c in range(NCHUNK):
            for which in range(2):  # 0=q, 1=k
                rows = np.concatenate([
                    which * C + (8 * g + 2 * c + hh) * 64 + d_r for hh in range(2)
                ])  # 128 feature rows
                blk = qkv_w[rows, :]          # (128 feat, 1024 k)
                for kc in range(KT):
                    sl = slice(c * 2048 + kc * 256 + which * 128,
                               c * 2048 + kc * 256 + which * 128 + 128)
                    wqk[:, sl] = blk[:, kc * 128:(kc + 1) * 128].T.astype(f8)
                bc = qkv_b[rows].astype(np.float32)
                bqk[:, c * 4 + which * 2] = bc
                bqk[:, c * 4 + which * 2 + 1] = bc[p ^ 16]
        # wv: [p, kc*VW + col], col = VS*h + j
        wva = np.zeros((KT_AUG * 128, VW), dtype=np.float32)
        for h in range(HPG):
            rows = 2 * C + (8 * g + h) * 64 + np.arange(64)
            wva[:C, VS * h: VS * h + 64] = qkv_w[rows, :].T
            wva[C, VS * h: VS * h + 64] = qkv_b[rows]
            wva[C, VS * h + 64] = 1.0
        wv8 = np.empty((128, KT_AUG * VW), dtype=f8)
        wv16 = np.empty((128, KT_AUG * VW), dtype=np.float16)
        for kc in range(KT_AUG):
            wv8[:, kc * VW:(kc + 1) * VW] = wva[kc * 128:(kc + 1) * 128].astype(f8)
            wv16[:, kc * VW:(kc + 1) * VW] = wva[kc * 128:(kc + 1) * 128].astype(np.float16)
        ins_g.append((wqk, bqk, wv8, wv16))

    # rope tables
    inv_freq = (1.0 / (ROPE_BASE ** (np.arange(0, D, 2) / D))).astype(np.float64)
    t = np.arange(T, dtype=np.float64)
    ang = t[None, :] * inv_freq[f_p][:, None]          # (128, T)
    cs = np.cos(ang).astype(np.float16)
    sgn = np.where((p % 32) < 16, -1.0, 1.0)[:, None]
    css = (sgn * np.sin(ang)).astype(np.float16)

    return xt8s, xt16s, ins_g, cs, css


def _prep_wo(out_w, g):
    out_w = np.asarray(out_w, dtype=np.float32)
    wo = np.empty((128, NCHUNK * 1024), dtype=np.float16)
    for c in range(NCHUNK):
        rows = np.concatenate([(8 * g + 2 * c + hh) * 64 + np.arange(64) for hh in range(2)])
        wo[:, c * 1024:(c + 1) * 1024] = out_w[:, rows].astype(np.float16).T
    return wo


def _in_maps(x, qkv_w, qkv_b, out_w):
    xt8s, xt16s, ins_g, cs, css = _prep_inputs(x, qkv_w, qkv_b)
    wos = [_prep_wo(out_w, g) for g in range(2)]
    in_maps = []
    for core in range(N_CORES):
        b, g = core // 2, core % 2
        wqk, bqk, wv8, wv16 = ins_g[g]
        in_maps.append({
            "xt8": xt8s[b], "xt16": xt16s[b], "wqk": wqk,
            "wv8": wv8, "wv16": wv16, "wo": wos[g],
            "bqk": bqk, "cs": cs, "css": css,
        })
    return in_maps


def kernel(x, qkv_w, qkv_b, out_w, out_b):
    from concourse.bass_utils import run_bass_kernel_spmd

    if "nc" not in _CACHE:
        _CACHE["nc"] = _build_nc()
    nc = _CACHE["nc"]

    in_maps = _in_maps(x, qkv_w, qkv_b, out_w)
    out_b = np.asarray(out_b, dtype=np.float32)

    try:
        res = run_bass_kernel_spmd(nc, in_maps, core_ids=list(range(N_CORES)))
    except ModuleNotFoundError:
        # BASS_TRACE set but the NTFF profile hook isn't importable here
        import os
        os.environ["BASS_NEVER_TRACE"] = "1"
        res = run_bass_kernel_spmd(nc, in_maps, core_ids=list(range(N_CORES)))

    out = np.empty((B, T, C), dtype=np.float32)
    for b in range(B):
        pt = res.results[2 * b]["ot"] + res.results[2 * b + 1]["ot"]  # (C, T)
        out[b] = pt.T + out_b[None, :]
    return out


# revision 4
# speedup vs baseline: 1.0049x; 1.0013x over previous
"""Causal self-attention (B=4, T=2048, C=1024, H=16, D=64) on 8 TRN2 NeuronCores.

Sharding: core = (batch b, head-group g) with b = core // 2, g = core % 2.
Each core computes heads [8g, 8g+8) of batch b and produces the partial
out-projection (C, T) for its head group; the host sums the two head-group
partials per batch and adds the output bias.

v2 performance structure:
- fp8e4m3 DoubleRow matmuls (2 contraction k-tiles per instruction) for the
  qk-projection, v-projection, scores and att@V. Scores contract over d=64
  only, so their second DoubleRow slot is a zeroed region of the rq/rk
  tiles; att@V pairs adjacent s-tiles. The out-projection and the first
  t-window's attention (t < 512, where softmax averaging is too weak to
  wash out fp8 noise) stay fp16.
- Software pipeline over windows w=(c, tt): rope of windows[i+1] and
  scores+exp of windows[i] stream on PE/Act while attV+norm of windows[i-1]
  ride behind, so the in-order engines never head-of-line block the exp
  stream. V/out-projections are interleaved where they fit; chunk 3 runs
  tt descending so the final out-projection tail is the smallest window.
- exp for the late (most softmax-diffuse) windows is computed on DVE as a
  single scalar_tensor_tensor emitting fp8e4m3 BIT PATTERNS directly
  (the fp8 bit grid is linear in log2, so bits = 1.4427*score + 56.5).
- Softmax denominator: reciprocal on a DMA-transposed [128,8] layout
  (free-dim cost 8 instead of 1024), then DMA partition-broadcast.
- Causal masking only touches the 128-wide diagonal slab of each e-tile
  (gpsimd affine_select); fully-masked tiles are never computed.
- All input DMA is fused into ~13 large multi-dim transfers issued from the
  idle SP sequencer, ordered so RoPE/scores inputs land first.
"""

import numpy as np

B, T, C = 4, 2048, 1024
H, D = 16, 64
N_CORES = 8
HPG = H // 2            # heads per core (group)
NCHUNK = 4              # head-pair chunks per core
KT = 8                  # k-tiles of 128 over C
KT_AUG = 9              # + bias/ones k-tile
TT = 4                  # t-tiles of 512 over T
NT = 512                # t tile (matmul N)
VS = 66                 # v column stride per head (64 dims + ones + pad)
VW = HPG * VS           # 528 v columns per k-chunk block
ROPE_BASE = 10000.0

_CACHE = {}


def _build_nc():
    import concourse.bass as bass  # noqa: F401
    import concourse.tile as tile
    from concourse import bacc, mybir
    from contextlib import ExitStack

    f16 = mybir.dt.float16
    f32 = mybir.dt.float32
    f8 = mybir.dt.float8e4
    DR = mybir.MatmulPerfMode.DoubleRow

    nc = bacc.Bacc(
        "TRN2",
        target_bir_lowering=False,
        debug=False,
        enable_asserts=True,
        num_devices=N_CORES,
    )

    xt8_d = nc.dram_tensor("xt8", (KT_AUG * 128, T), f8, kind="ExternalInput").ap()
    xt16_d = nc.dram_tensor("xt16", (KT_AUG * 128, NT), f16, kind="ExternalInput").ap()
    wqk_d = nc.dram_tensor("wqk", (128, KT * 1024), f8, kind="ExternalInput").ap()
    wv8_d = nc.dram_tensor("wv8", (128, KT_AUG * VW), f8, kind="ExternalInput").ap()
    wv16_d = nc.dram_tensor("wv16", (128, KT_AUG * VW), f16, kind="ExternalInput").ap()
    wo_d = nc.dram_tensor("wo", (128, NCHUNK * 1024), f16, kind="ExternalInput").ap()
    bqk_d = nc.dram_tensor("bqk", (128, 16), f32, kind="ExternalInput").ap()
    cs_d = nc.dram_tensor("cs", (128, T), f16, kind="ExternalInput").ap()
    css_d = nc.dram_tensor("css", (128, T), f16, kind="ExternalInput").ap()
    ot_d = nc.dram_tensor("ot", (1024, T), f32, kind="ExternalOutput").ap()

    SHUF = list(range(16, 32)) + list(range(0, 16))

    with tile.TileContext(nc) as tc:
        with ExitStack() as ctx, nc.allow_low_precision("fp8/fp16 attention pipeline"):
            consts = ctx.enter_context(tc.tile_pool(name="consts", bufs=1))
            rtmp = ctx.enter_context(tc.tile_pool(name="rtmp", bufs=4))
            ep_pool = ctx.enter_context(tc.tile_pool(name="ep", bufs=13))
            ed_pool = ctx.enter_context(tc.tile_pool(name="ed", bufs=14))
            ed16_pool = ctx.enter_context(tc.tile_pool(name="ed16", bufs=5))
            small = ctx.enter_context(tc.tile_pool(name="small", bufs=3))
            osb = ctx.enter_context(tc.tile_pool(name="osb", bufs=4))
            ps_q = ctx.enter_context(tc.tile_pool(name="psq", bufs=2, space="PSUM"))
            ps_s = ctx.enter_context(tc.tile_pool(name="pss", bufs=3, space="PSUM"))

            # ---- resident tiles + input DMA ----
            # Fused input DMA (SP-issued, ~13 transfers): xt8 t-block 0 and the
            # chunk-0 qk weights land first so RoPE + first scores start early.
            xt8 = consts.tile([128, KT_AUG * T], f8)
            wqk = consts.tile([128, KT * 1024], f8)
            xt16 = consts.tile([128, KT_AUG * NT], f16)
            wv16 = consts.tile([128, KT_AUG * VW], f16)
            wv8 = consts.tile([128, KT_AUG * VW], f8)
            wo = consts.tile([128, NCHUNK * 1024], f16)
            bqk = consts.tile([128, 16], f32)
            cs = consts.tile([128, T], f16)
            css = consts.tile([128, T], f16)

            xt8_r = xt8[:].rearrange("p (k t) -> p k t", k=KT_AUG)
            xt16_r3 = xt16[:].rearrange("p (k t) -> p k t", k=KT_AUG)

            def dma_xt(dst3, dram, nk, width, c0, w):
                # one transfer covering kc 0..nk of dram rows, cols [c0, c0+w)
                nc.sync.dma_start(
                    dst3[:, 0:nk, c0:c0 + w],
                    bass.AP(tensor=dram.tensor, offset=dram.offset + c0,
                            ap=[[width, 128], [128 * width, nk], [1, w]]))

            dma_xt(xt8_r, xt8_d, KT, T, 0, NT)                      # t-block 0
            nc.sync.dma_start(wqk[:, 0:2048], wqk_d[:, 0:2048])     # chunk 0
            nc.sync.dma_start(bqk[:], bqk_d[:])
            nc.sync.dma_start(cs[:], cs_d[:])
            nc.sync.dma_start(css[:], css_d[:])
            dma_xt(xt8_r, xt8_d, KT_AUG, T, NT, T - NT)             # t-blocks 1-3
            nc.sync.dma_start(xt8_r[:, 8:9, 0:NT],
                              bass.AP(tensor=xt8_d.tensor, offset=xt8_d.offset + 8 * 128 * T,
                                      ap=[[T, 128], [1, NT]]))      # aug t-block 0
            dma_xt(xt16_r3, xt16_d, KT_AUG, NT, 0, NT)
            nc.sync.dma_start(wv16[:], wv16_d[:])
            nc.sync.dma_start(wv8[:], wv8_d[:])
            nc.sync.dma_start(wqk[:, 2048:8192], wqk_d[:, 2048:8192])
            nc.sync.dma_start(wo[:], wo_d[:])

            b56 = consts.tile([128, 1], f32)
            nc.gpsimd.memset(b56[:], 56.5)
            v8 = consts.tile([128, 16 * VW], f8)
            v16 = consts.tile([128, 4 * VW], f16)
            y_all = consts.tile([128, NCHUNK * T], f16)

            # double-buffered q/k tiles: cols [0:T) = rope output (slot A),
            # cols [T:2T) = zeros (slot B of the DoubleRow zero-slot trick)
            rqb = [consts.tile([128, 2 * T], f8, name=f"rq{i}") for i in range(2)]
            rkb = [consts.tile([128, 2 * T], f8, name=f"rk{i}") for i in range(2)]
            for t_ in rqb + rkb:
                nc.gpsimd.memset(t_[:, T:2 * T], 0.0)

            wqk_r = wqk[:].rearrange("p (c k u) -> p c k u", c=NCHUNK, k=KT)
            wv8_r = wv8[:].rearrange("p (k w) -> p k w", k=KT_AUG)
            wv16_r = wv16[:].rearrange("p (k w) -> p k w", k=KT_AUG)
            v8_r = v8[:].rearrange("p (m w) -> p m w", m=16)
            v16_r = v16[:].rearrange("p (m w) -> p m w", m=4)

            def emit_vproj8(m_lo, m_hi):
                with nc.named_scope("vproj8"):
                    for m in range(m_lo, m_hi):  # 128-row t-slices
                        psa = ps_q.tile([128, 512], f32, tag="q")
                        psb = ps_s.tile([128, 1024], f32, tag="s")
                        for kp in range(4):
                            lhsT = xt8_r[:, 2 * kp:2 * kp + 2, m * 128:(m + 1) * 128]
                            nc.tensor.matmul(psa[:], lhsT, wv8_r[:, 2 * kp:2 * kp + 2, 0:512],
                                             start=(kp == 0), stop=False, perf_mode=DR)
                            nc.tensor.matmul(psb[:, 0:16], lhsT,
                                             wv8_r[:, 2 * kp:2 * kp + 2, 512:528],
                                             start=(kp == 0), stop=False, perf_mode=DR)
                        lhs8 = xt8_r[:, 8:9, m * 128:(m + 1) * 128]
                        nc.tensor.matmul(psa[:], lhs8, wv8_r[:, 8:9, 0:512],
                                         start=False, stop=True)
                        nc.tensor.matmul(psb[:, 0:16], lhs8, wv8_r[:, 8:9, 512:528],
                                         start=False, stop=True)
                        nc.vector.tensor_copy(v8[:, m * VW: m * VW + 512], psa[:])
                        nc.vector.tensor_copy(v8[:, m * VW + 512:(m + 1) * VW], psb[:, 0:16])

            def emit_vproj16():
                # accurate fp16 V for s < 512 (feeds the t<512 attention)
                with nc.named_scope("vproj16"):
                    for m in range(4):
                        psa = ps_q.tile([128, 512], f32, tag="q")
                        psb = ps_s.tile([128, 1024], f32, tag="s")
                        for kc in range(KT_AUG):
                            lhs = xt16[:, kc * NT + m * 128: kc * NT + (m + 1) * 128]
                            nc.tensor.matmul(psa[:], lhs, wv16_r[:, kc:kc + 1, 0:512],
                                             start=(kc == 0), stop=(kc == KT_AUG - 1))
                            nc.tensor.matmul(psb[:, 0:16], lhs, wv16_r[:, kc:kc + 1, 512:528],
                                             start=(kc == 0), stop=(kc == KT_AUG - 1))
                        nc.vector.tensor_copy(v16[:, m * VW: m * VW + 512], psa[:])
                        nc.vector.tensor_copy(v16[:, m * VW + 512:(m + 1) * VW], psb[:, 0:16])

            def emit_rope(c, tt):
                # q/k projection + RoPE for heads (2c, 2c+1), t-window tt
                rq = rqb[c % 2]
                rk = rkb[c % 2]
                with nc.named_scope("qkrope"):
                    if True:
                        t0 = tt * NT
                        for which, dst in ((0, rq), (1, rk)):
                            ps = ps_q.tile([128, 512], f32, tag="q")
                            u0 = which * 128
                            for kp in range(4):
                                nc.tensor.matmul(
                                    ps[:],
                                    wqk_r[:, c, 2 * kp:2 * kp + 2, u0:u0 + 128],
                                    xt8_r[:, 2 * kp:2 * kp + 2, t0:t0 + NT],
                                    start=(kp == 0), stop=(kp == 3), perf_mode=DR)
                            bcol = bqk[:, c * 4 + which * 2: c * 4 + which * 2 + 1]
                            bswp = bqk[:, c * 4 + which * 2 + 1: c * 4 + which * 2 + 2]
                            s_t = rtmp.tile([128, 512], f32, tag="st")
                            nc.vector.stream_shuffle(s_t[:], ps[:], SHUF)
                            x1 = rtmp.tile([128, 512], f16, tag="x1")
                            nc.vector.scalar_tensor_tensor(
                                out=x1[:], in0=ps[:], scalar=bcol, in1=cs[:, t0:t0 + NT],
                                op0=mybir.AluOpType.add, op1=mybir.AluOpType.mult)
                            x2 = rtmp.tile([128, 512], f16, tag="x2")
                            nc.vector.scalar_tensor_tensor(
                                out=x2[:], in0=s_t[:], scalar=bswp, in1=css[:, t0:t0 + NT],
                                op0=mybir.AluOpType.add, op1=mybir.AluOpType.mult)
                            nc.gpsimd.tensor_add(dst[:, t0:t0 + NT], x1[:], x2[:])

            def emit_scores(c, tt):
                # scores + exp (+ diagonal mask) for window (c, tt).
                # Returns the e-tiles for emit_attv.
                rq_r = rqb[c % 2][:].rearrange("p (s t) -> p s t", s=2)
                rk_r = rkb[c % 2][:].rearrange("p (s t) -> p s t", s=2)
                t0 = tt * NT
                n_full = t0 // 128
                epairs, ediags = [], []
                with nc.named_scope("scores"):
                    for ip in range(n_full // 2):
                        epair = ep_pool.tile([128, 2048], f8)
                        for j in range(2):
                            sc = 2 * ip + j
                            s0 = sc * 128
                            sp = ps_s.tile([128, 1024], f32, tag="s")
                            for h in range(2):
                                nc.tensor.matmul(
                                    sp[:, h * NT:(h + 1) * NT],
                                    rk_r[h * 64:(h + 1) * 64, :, s0:s0 + 128],
                                    rq_r[h * 64:(h + 1) * 64, :, t0:t0 + NT],
                                    start=True, stop=True, perf_mode=DR)
                            if tt == 3 and ip >= 4:
                                # fp8e4m3 bit grid is linear in log2, so one
                                # DVE op emits exp(s/8) bits: b = 1.4427*s+56.5
                                nc.vector.scalar_tensor_tensor(
                                    out=epair[:, j * 1024:(j + 1) * 1024].bitcast(mybir.dt.int8),
                                    in0=sp[:], scalar=1.4426950,
                                    in1=b56[:, 0:1].broadcast_to([128, 1024]),
                                    op0=mybir.AluOpType.mult,
                                    op1=mybir.AluOpType.add)
                            else:
                                nc.scalar.activation(
                                    epair[:, j * 1024:(j + 1) * 1024], sp[:],
                                    mybir.ActivationFunctionType.Exp,
                                    bias=0.0, scale=0.125)
                        epairs.append(epair)
                    for d in range(4):
                        sc = n_full + d
                        s0 = sc * 128
                        dlt = d * 128
                        sp = ps_s.tile([128, 1024], f32, tag="s")
                        for h in range(2):
                            nc.tensor.matmul(
                                sp[:, h * NT + dlt:(h + 1) * NT],
                                rk_r[h * 64:(h + 1) * 64, :, s0:s0 + 128],
                                rq_r[h * 64:(h + 1) * 64, :, t0 + dlt:t0 + NT],
                                start=True, stop=True, perf_mode=DR)
                        e_t = (ed16_pool.tile([128, 1024], f16, name="e16")
                               if tt == 0 else ed_pool.tile([128, 1024], f8, name="e8"))
                        s3 = sp[:].rearrange("p (a b) -> p a b", a=2)[:, :, dlt:]
                        e3 = e_t[:].rearrange("p (a b) -> p a b", a=2)[:, :, dlt:]
                        nc.scalar.activation(e3, s3, mybir.ActivationFunctionType.Exp,
                                             bias=0.0, scale=0.125)
                        # causal mask on the 128-wide diagonal slab only:
                        # within the slab keep iff col >= partition
                        nc.gpsimd.affine_select(
                            out=e3[:, :, 0:128], in_=e3[:, :, 0:128],
                            compare_op=mybir.AluOpType.is_ge,
                            fill=0.0, base=0,
                            pattern=[[0, 2], [1, 128]], channel_multiplier=-1)
                        ediags.append(e_t)
                return epairs, ediags

            def emit_attv(c, tt, epairs, ediags, tail=False):
                # att@V accumulation + normalization for window (c, tt),
                # one head at a time so the accumulator is a 2KB [65,512]
                # tile in the ps_q ring (frees PSUM for a 3rd score buffer)
                t0 = tt * NT
                n_full = t0 // 128
                with nc.named_scope("attv"):
                    vsrc = v16_r if tt == 0 else v8_r
                    for h in range(2):
                        vcol = VS * (2 * c + h)
                        yp = ps_q.tile([65, 512], f32, tag="q", name="yp")
                        for ip, epair in enumerate(epairs):
                            epair_r = epair[:].rearrange("p (s x) -> p s x", s=2)
                            nc.tensor.matmul(
                                yp[:], v8_r[:, 2 * ip:2 * ip + 2, vcol:vcol + 65],
                                epair_r[:, :, h * NT:(h + 1) * NT],
                                start=(ip == 0), stop=False, perf_mode=DR,
                                skip_group_check=True)
                        for d, e_t in enumerate(ediags):
                            sc = n_full + d
                            dlt = d * 128
                            nc.tensor.matmul(
                                yp[:, dlt:NT], vsrc[:, sc:sc + 1, vcol:vcol + 65],
                                e_t[:, h * NT + dlt:(h + 1) * NT],
                                start=(tt == 0 and d == 0), stop=(d == 3),
                                skip_group_check=True)

                        # normalization: y *= 1/den, reciprocal on a
                        # DMA-transposed [128,4] layout except on the tail
                        yc = small.tile([65, 512], f16, tag="yc")
                        nc.vector.tensor_copy(yc[:], yp[:])
                        rd = small.tile([1, 512], f16, tag="rd")
                        if tail:
                            nc.vector.reciprocal(rd[:], yc[64:65, :])
                        else:
                            dT = small.tile([128, 4], f16, tag="dT")
                            nc.sync.dma_start(out=dT[:], in_=yc[64:65, :])
                            rdT = small.tile([128, 4], f16, tag="rdT")
                            nc.vector.reciprocal(rdT[:], dT[:])
                            nc.sync.dma_start(out=rd[:], in_=rdT[:])
                        dbc = small.tile([64, 512], f16, tag="dbc")
                        rda = rd[0:1, :]
                        nc.sync.dma_start(out=dbc[:], in_=bass.AP(
                            tensor=rda.tensor, offset=rda.offset,
                            ap=[list(rda.ap)[0], [0, 64], [1, 512]]))
                        nc.vector.tensor_mul(
                            y_all[h * 64:(h + 1) * 64, c * T + t0: c * T + t0 + NT],
                            yc[0:64, :], dbc[:])

            def emit_oproj(tt, cs_=tuple(range(NCHUNK)), dram=None, act_copy=False,
                           wide=False):
                # output projection for t-window tt (all head-chunks of core)
                t0 = tt * NT
                dram = ot_d if dram is None else dram
                with nc.named_scope("oproj"):
                    for ct in range(8):
                        if wide and ct % 2 == 1:
                            po = ps_s.tile([128, 512], f32, tag="s", name="po")
                        else:
                            po = ps_q.tile([128, 512], f32, tag="q", name="po")
                        for c in cs_:
                            nc.tensor.matmul(po[:], wo[:, c * 1024 + ct * 128: c * 1024 + ct * 128 + 128],
                                             y_all[:, c * T + t0: c * T + t0 + NT],
                                             start=(c == cs_[0]), stop=(c == cs_[-1]))
                        ob = osb.tile([128, 512], f32)
                        if act_copy:
                            nc.scalar.activation(ob[:], po[:],
                                                 mybir.ActivationFunctionType.Copy)
                        else:
                            nc.vector.tensor_copy(ob[:], po[:])
                        oq = nc.sync
                        oq.dma_start(ot_d[ct * 128:(ct + 1) * 128, t0:t0 + NT], ob[:])

            # software pipeline over windows w=(c,tt): rope of windows[i+1]
            # and scores+exp of windows[i] stream on PE/Act while attV+norm of
            # windows[i-1] ride behind; v/out-projections interleave early/late.
            # Chunk 3 runs tt descending so the final oproj tail is smallest.
            windows = [(c, tt) for c in range(NCHUNK - 1) for tt in range(TT)]
            windows += [(NCHUNK - 1, tt) for tt in (3, 2, 1, 0)]
            emit_rope(*windows[0])
            pend = None
            for i, (c, tt) in enumerate(windows):
                if i + 1 < len(windows):
                    emit_rope(*windows[i + 1])
                et = emit_scores(c, tt)
                if i == 1:
                    emit_vproj16()
                elif i == 2:
                    emit_vproj8(0, 8)
                elif i == 3:
                    emit_vproj8(8, 16)
                if pend is not None:
                    pc, ptt, pet = pend
                    emit_attv(pc, ptt, *pet)
                    if pc == NCHUNK - 1:
                        emit_oproj(ptt, act_copy=(ptt <= 1))
                pend = (c, tt, et)
            pc, ptt, pet = pend
            emit_attv(pc, ptt, *pet, tail=True)
            emit_oproj(ptt, act_copy=True, wide=True)

    nc.compile()
    return nc


def _prep_inputs(x, qkv_w, qkv_b):
    """Build the 8 per-core input maps (all host-side numpy)."""
    import ml_dtypes
    f8 = ml_dtypes.float8_e4m3

    x = np.asarray(x, dtype=np.float32)
    qkv_w = np.asarray(qkv_w, dtype=np.float32)
    qkv_b = np.asarray(qkv_b, dtype=np.float32)

    # xt per batch: (KT_AUG*128, T) with row 1024 = ones, rest of aug block 0
    xt8s, xt16s = [], []
    for b in range(B):
        xa = np.zeros((KT_AUG * 128, T), dtype=np.float32)
        xa[:C] = x[b].T
        xa[C] = 1.0
        xt8s.append(xa.astype(f8))
        xt16s.append(xa[:, :NT].astype(np.float16))

    r = np.arange(64)
    d_r = 2 * ((r // 32) * 16 + (r % 16)) + ((r % 32) >= 16)  # row -> head dim
    p = np.arange(128)
    f_p = ((p // 32) % 2) * 16 + (p % 16)

    ins_g = []
    for g in range(2):
        # wqk: [p, c*2048 + kc*256 + which*128 + m] (chunk-major)
        wqk = np.empty((128, KT * 1024), dtype=f8)
        bqk = np.empty((128, 16), dtype=np.float32)
        for c in range(NCHUNK):
            for which in range(2):  # 0=q, 1=k
                rows = np.concatenate([
                    which * C + (8 * g + 2 * c + hh) * 64 + d_r for hh in range(2)
                ])  # 128 feature rows
                blk = qkv_w[rows, :]          # (128 feat, 1024 k)
                for kc in range(KT):
                    sl = slice(c * 2048 + kc * 256 + which * 128,
                               c * 2048 + kc * 256 + which * 128 + 128)
                    wqk[:, sl] = blk[:, kc * 128:(kc + 1) * 128].T.astype(f8)
                bc = qkv_b[rows].astype(np.float32)
                bqk[:, c * 4 + which * 2] = bc
                bqk[:, c * 4 + which * 2 + 1] = bc[p ^ 16]
        # wv: [p, kc*VW + col], col = VS*h + j
        wva = np.zeros((KT_AUG * 128, VW), dtype=np.float32)
        for h in range(HPG):
            rows = 2 * C + (8 * g + h) * 64 + np.arange(64)
            wva[:C, VS * h: VS * h + 64] = qkv_w[rows, :].T
            wva[C, VS * h: VS * h + 64] = qkv_b[rows]
            wva[C, VS * h + 64] = 1.0
        wv8 = np.empty((128, KT_AUG * VW), dtype=f8)
        wv16 = np.empty((128, KT_AUG * VW), dtype=np.float16)
        for kc in range(KT_AUG):
            wv8[:, kc * VW:(kc + 1) * VW] = wva[kc * 128:(kc + 1) * 128].astype(f8)
            wv16[:, kc * VW:(kc + 1) * VW] = wva[kc * 128:(kc + 1) * 128].astype(np.float16)
        ins_g.append((wqk, bqk, wv8, wv16))

    # rope tables
    inv_freq = (1.0 / (ROPE_BASE ** (np.arange(0, D, 2) / D))).astype(np.float64)
    t = np.arange(T, dtype=np.float64)
    ang = t[None, :] * inv_freq[f_p][:, None]          # (128, T)
    cs = np.cos(ang).astype(np.float16)
    sgn = np.where((p % 32) < 16, -1.0, 1.0)[:, None]
    css = (sgn * np.sin(ang)).astype(np.float16)

    return xt8s, xt16s, ins_g, cs, css


def _prep_wo(out_w, g):
    out_w = np.asarray(out_w, dtype=np.float32)
    wo = np.empty((128, NCHUNK * 1024), dtype=np.float16)
    for c in range(NCHUNK):
        rows = np.concatenate([(8 * g + 2 * c + hh) * 64 + np.arange(64) for hh in range(2)])
        wo[:, c * 1024:(c + 1) * 1024] = out_w[:, rows].astype(np.float16).T
    return wo


def _in_maps(x, qkv_w, qkv_b, out_w):
    xt8s, xt16s, ins_g, cs, css = _prep_inputs(x, qkv_w, qkv_b)
    wos = [_prep_wo(out_w, g) for g in range(2)]
    in_maps = []
    for core in range(N_CORES):
        b, g = core // 2, core % 2
        wqk, bqk, wv8, wv16 = ins_g[g]
        in_maps.append({
            "xt8": xt8s[b], "xt16": xt16s[b], "wqk": wqk,
            "wv8": wv8, "wv16": wv16, "wo": wos[g],
            "bqk": bqk, "cs": cs, "css": css,
        })
    return in_maps


def kernel(x, qkv_w, qkv_b, out_w, out_b):
    from concourse.bass_utils import run_bass_kernel_spmd

    if "nc" not in _CACHE:
        _CACHE["nc"] = _build_nc()
    nc = _CACHE["nc"]

    in_maps = _in_maps(x, qkv_w, qkv_b, out_w)
    out_b = np.asarray(out_b, dtype=np.float32)

    try:
        res = run_bass_kernel_spmd(nc, in_maps, core_ids=list(range(N_CORES)))
    except ModuleNotFoundError:
        # BASS_TRACE set but the NTFF profile hook isn't importable here
        import os
        os.environ["BASS_NEVER_TRACE"] = "1"
        res = run_bass_kernel_spmd(nc, in_maps, core_ids=list(range(N_CORES)))

    out = np.empty((B, T, C), dtype=np.float32)
    for b in range(B):
        pt = res.results[2 * b]["ot"] + res.results[2 * b + 1]["ot"]  # (C, T)
        out[b] = pt.T + out_b[None, :]
    return out


# revision 5
# speedup vs baseline: 1.0075x; 1.0026x over previous
"""Causal self-attention (B=4, T=2048, C=1024, H=16, D=64) on 8 TRN2 NeuronCores.

Sharding: core = (batch b, head-group g) with b = core // 2, g = core % 2.
Each core computes heads [8g, 8g+8) of batch b and produces the partial
out-projection (C, T) for its head group; the host sums the two head-group
partials per batch and adds the output bias.

v2 performance structure:
- fp8e4m3 DoubleRow matmuls (2 contraction k-tiles per instruction) for the
  qk-projection, v-projection, scores and att@V. Scores contract over d=64
  only, so their second DoubleRow slot is a zeroed region of the rq/rk
  tiles; att@V pairs adjacent s-tiles. The out-projection and the first
  t-window's attention (t < 512, where softmax averaging is too weak to
  wash out fp8 noise) stay fp16.
- Software pipeline over windows w=(c, tt): rope of windows[i+1] and
  scores+exp of windows[i] stream on PE/Act while attV+norm of windows[i-1]
  ride behind, so the in-order engines never head-of-line block the exp
  stream. V/out-projections are interleaved where they fit; chunk 3 runs
  tt descending so the final out-projection tail is the smallest window.
- exp for the late (most softmax-diffuse) windows is computed on DVE as a
  single scalar_tensor_tensor emitting fp8e4m3 BIT PATTERNS directly
  (the fp8 bit grid is linear in log2, so bits = 1.4427*score + 56.5).
- Softmax denominator: reciprocal on a DMA-transposed [128,4] layout
  (free-dim cost 4 instead of 512), then DMA partition-broadcast. The
  att@V accumulator is per-head [65,512] in the ps_q ring, freeing PSUM
  for a 3-deep score ring that decouples exp from PE's in-order detours.
- Causal masking only touches the 128-wide diagonal slab of each e-tile
  (gpsimd affine_select); fully-masked tiles are never computed.
- All input DMA is fused into ~13 large multi-dim transfers issued from the
  idle SP sequencer, ordered so RoPE/scores inputs land first.
"""

import numpy as np

B, T, C = 4, 2048, 1024
H, D = 16, 64
N_CORES = 8
HPG = H // 2            # heads per core (group)
NCHUNK = 4              # head-pair chunks per core
KT = 8                  # k-tiles of 128 over C
KT_AUG = 9              # + bias/ones k-tile
TT = 4                  # t-tiles of 512 over T
NT = 512                # t tile (matmul N)
VS = 66                 # v column stride per head (64 dims + ones + pad)
VW = HPG * VS           # 528 v columns per k-chunk block
ROPE_BASE = 10000.0

_CACHE = {}


def _build_nc():
    import concourse.bass as bass  # noqa: F401
    import concourse.tile as tile
    from concourse import bacc, mybir
    from contextlib import ExitStack

    f16 = mybir.dt.float16
    f32 = mybir.dt.float32
    f8 = mybir.dt.float8e4
    DR = mybir.MatmulPerfMode.DoubleRow

    nc = bacc.Bacc(
        "TRN2",
        target_bir_lowering=False,
        debug=False,
        enable_asserts=True,
        num_devices=N_CORES,
    )

    xt8_d = nc.dram_tensor("xt8", (KT_AUG * 128, T), f8, kind="ExternalInput").ap()
    xt16_d = nc.dram_tensor("xt16", (KT_AUG * 128, NT), f16, kind="ExternalInput").ap()
    wqk_d = nc.dram_tensor("wqk", (128, KT * 1024), f8, kind="ExternalInput").ap()
    wv8_d = nc.dram_tensor("wv8", (128, KT_AUG * VW), f8, kind="ExternalInput").ap()
    wv16_d = nc.dram_tensor("wv16", (128, KT_AUG * VW), f16, kind="ExternalInput").ap()
    wo_d = nc.dram_tensor("wo", (128, NCHUNK * 1024), f16, kind="ExternalInput").ap()
    bqk_d = nc.dram_tensor("bqk", (128, 16), f32, kind="ExternalInput").ap()
    cs_d = nc.dram_tensor("cs", (128, T), f16, kind="ExternalInput").ap()
    css_d = nc.dram_tensor("css", (128, T), f16, kind="ExternalInput").ap()
    ot_d = nc.dram_tensor("ot", (1024, T), f32, kind="ExternalOutput").ap()

    SHUF = list(range(16, 32)) + list(range(0, 16))

    with tile.TileContext(nc) as tc:
        with ExitStack() as ctx, nc.allow_low_precision("fp8/fp16 attention pipeline"):
            consts = ctx.enter_context(tc.tile_pool(name="consts", bufs=1))
            rtmp = ctx.enter_context(tc.tile_pool(name="rtmp", bufs=4))
            ep_pool = ctx.enter_context(tc.tile_pool(name="ep", bufs=13))
            ed_pool = ctx.enter_context(tc.tile_pool(name="ed", bufs=14))
            ed16_pool = ctx.enter_context(tc.tile_pool(name="ed16", bufs=5))
            small = ctx.enter_context(tc.tile_pool(name="small", bufs=3))
            osb = ctx.enter_context(tc.tile_pool(name="osb", bufs=4))
            ps_q = ctx.enter_context(tc.tile_pool(name="psq", bufs=2, space="PSUM"))
            ps_s = ctx.enter_context(tc.tile_pool(name="pss", bufs=3, space="PSUM"))

            # ---- resident tiles + input DMA ----
            # Fused input DMA (SP-issued, ~13 transfers): xt8 t-block 0 and the
            # chunk-0 qk weights land first so RoPE + first scores start early.
            xt8 = consts.tile([128, KT_AUG * T], f8)
            wqk = consts.tile([128, KT * 1024], f8)
            xt16 = consts.tile([128, KT_AUG * NT], f16)
            wv16 = consts.tile([128, KT_AUG * VW], f16)
            wv8 = consts.tile([128, KT_AUG * VW], f8)
            wo = consts.tile([128, NCHUNK * 1024], f16)
            bqk = consts.tile([128, 16], f32)
            cs = consts.tile([128, T], f16)
            css = consts.tile([128, T], f16)

            xt8_r = xt8[:].rearrange("p (k t) -> p k t", k=KT_AUG)
            xt16_r3 = xt16[:].rearrange("p (k t) -> p k t", k=KT_AUG)

            def dma_xt(dst3, dram, nk, width, c0, w):
                # one transfer covering kc 0..nk of dram rows, cols [c0, c0+w)
                nc.sync.dma_start(
                    dst3[:, 0:nk, c0:c0 + w],
                    bass.AP(tensor=dram.tensor, offset=dram.offset + c0,
                            ap=[[width, 128], [128 * width, nk], [1, w]]))

            dma_xt(xt8_r, xt8_d, KT, T, 0, NT)                      # t-block 0
            nc.sync.dma_start(wqk[:, 0:2048], wqk_d[:, 0:2048])     # chunk 0
            nc.sync.dma_start(bqk[:], bqk_d[:])
            nc.sync.dma_start(cs[:], cs_d[:])
            nc.sync.dma_start(css[:], css_d[:])
            dma_xt(xt8_r, xt8_d, KT_AUG, T, NT, T - NT)             # t-blocks 1-3
            nc.sync.dma_start(xt8_r[:, 8:9, 0:NT],
                              bass.AP(tensor=xt8_d.tensor, offset=xt8_d.offset + 8 * 128 * T,
                                      ap=[[T, 128], [1, NT]]))      # aug t-block 0
            dma_xt(xt16_r3, xt16_d, KT_AUG, NT, 0, NT)
            nc.sync.dma_start(wv16[:], wv16_d[:])
            nc.sync.dma_start(wv8[:], wv8_d[:])
            nc.sync.dma_start(wqk[:, 2048:8192], wqk_d[:, 2048:8192])
            nc.sync.dma_start(wo[:], wo_d[:])

            b56 = consts.tile([128, 1], f32)
            nc.gpsimd.memset(b56[:], 56.5)
            v8 = consts.tile([128, 16 * VW], f8)
            v16 = consts.tile([128, 4 * VW], f16)
            y_all = consts.tile([128, NCHUNK * T], f16)

            # double-buffered q/k tiles: cols [0:T) = rope output (slot A),
            # cols [T:2T) = zeros (slot B of the DoubleRow zero-slot trick)
            rqb = [consts.tile([128, 2 * T], f8, name=f"rq{i}") for i in range(2)]
            rkb = [consts.tile([128, 2 * T], f8, name=f"rk{i}") for i in range(2)]
            for t_ in rqb + rkb:
                nc.gpsimd.memset(t_[:, T:2 * T], 0.0)

            wqk_r = wqk[:].rearrange("p (c k u) -> p c k u", c=NCHUNK, k=KT)
            wv8_r = wv8[:].rearrange("p (k w) -> p k w", k=KT_AUG)
            wv16_r = wv16[:].rearrange("p (k w) -> p k w", k=KT_AUG)
            v8_r = v8[:].rearrange("p (m w) -> p m w", m=16)
            v16_r = v16[:].rearrange("p (m w) -> p m w", m=4)

            def emit_vproj8(m_lo, m_hi):
                with nc.named_scope("vproj8"):
                    for m in range(m_lo, m_hi):  # 128-row t-slices
                        psa = ps_q.tile([128, 512], f32, tag="q")
                        psb = ps_s.tile([128, 1024], f32, tag="s")
                        for kp in range(4):
                            lhsT = xt8_r[:, 2 * kp:2 * kp + 2, m * 128:(m + 1) * 128]
                            nc.tensor.matmul(psa[:], lhsT, wv8_r[:, 2 * kp:2 * kp + 2, 0:512],
                                             start=(kp == 0), stop=False, perf_mode=DR)
                            nc.tensor.matmul(psb[:, 0:16], lhsT,
                                             wv8_r[:, 2 * kp:2 * kp + 2, 512:528],
                                             start=(kp == 0), stop=False, perf_mode=DR)
                        lhs8 = xt8_r[:, 8:9, m * 128:(m + 1) * 128]
                        nc.tensor.matmul(psa[:], lhs8, wv8_r[:, 8:9, 0:512],
                                         start=False, stop=True)
                        nc.tensor.matmul(psb[:, 0:16], lhs8, wv8_r[:, 8:9, 512:528],
                                         start=False, stop=True)
                        nc.vector.tensor_copy(v8[:, m * VW: m * VW + 512], psa[:])
                        nc.vector.tensor_copy(v8[:, m * VW + 512:(m + 1) * VW], psb[:, 0:16])

            def emit_vproj16():
                # accurate fp16 V for s < 512 (feeds the t<512 attention)
                with nc.named_scope("vproj16"):
                    for m in range(4):
                        psa = ps_q.tile([128, 512], f32, tag="q")
                        psb = ps_s.tile([128, 1024], f32, tag="s")
                        for kc in range(KT_AUG):
                            lhs = xt16[:, kc * NT + m * 128: kc * NT + (m + 1) * 128]
                            nc.tensor.matmul(psa[:], lhs, wv16_r[:, kc:kc + 1, 0:512],
                                             start=(kc == 0), stop=(kc == KT_AUG - 1))
                            nc.tensor.matmul(psb[:, 0:16], lhs, wv16_r[:, kc:kc + 1, 512:528],
                                             start=(kc == 0), stop=(kc == KT_AUG - 1))
                        nc.vector.tensor_copy(v16[:, m * VW: m * VW + 512], psa[:])
                        nc.vector.tensor_copy(v16[:, m * VW + 512:(m + 1) * VW], psb[:, 0:16])

            def emit_rope(c, tt):
                # q/k projection + RoPE for heads (2c, 2c+1), t-window tt
                rq = rqb[c % 2]
                rk = rkb[c % 2]
                with nc.named_scope("qkrope"):
                    if True:
                        t0 = tt * NT
                        for which, dst in ((0, rq), (1, rk)):
                            ps = ps_q.tile([128, 512], f32, tag="q")
                            u0 = which * 128
                            for kp in range(4):
                                nc.tensor.matmul(
                                    ps[:],
                                    wqk_r[:, c, 2 * kp:2 * kp + 2, u0:u0 + 128],
                                    xt8_r[:, 2 * kp:2 * kp + 2, t0:t0 + NT],
                                    start=(kp == 0), stop=(kp == 3), perf_mode=DR)
                            bcol = bqk[:, c * 4 + which * 2: c * 4 + which * 2 + 1]
                            bswp = bqk[:, c * 4 + which * 2 + 1: c * 4 + which * 2 + 2]
                            s_t = rtmp.tile([128, 512], f32, tag="st")
                            nc.vector.stream_shuffle(s_t[:], ps[:], SHUF)
                            x1 = rtmp.tile([128, 512], f16, tag="x1")
                            nc.vector.scalar_tensor_tensor(
                                out=x1[:], in0=ps[:], scalar=bcol, in1=cs[:, t0:t0 + NT],
                                op0=mybir.AluOpType.add, op1=mybir.AluOpType.mult)
                            x2 = rtmp.tile([128, 512], f16, tag="x2")
                            nc.vector.scalar_tensor_tensor(
                                out=x2[:], in0=s_t[:], scalar=bswp, in1=css[:, t0:t0 + NT],
                                op0=mybir.AluOpType.add, op1=mybir.AluOpType.mult)
                            nc.gpsimd.tensor_add(dst[:, t0:t0 + NT], x1[:], x2[:])

            def emit_scores(c, tt):
                # scores + exp (+ diagonal mask) for window (c, tt).
                # Returns the e-tiles for emit_attv.
                rq_r = rqb[c % 2][:].rearrange("p (s t) -> p s t", s=2)
                rk_r = rkb[c % 2][:].rearrange("p (s t) -> p s t", s=2)
                t0 = tt * NT
                n_full = t0 // 128
                epairs, ediags = [], []
                with nc.named_scope("scores"):
                    for ip in range(n_full // 2):
                        epair = ep_pool.tile([128, 2048], f8)
                        for j in range(2):
                            sc = 2 * ip + j
                            s0 = sc * 128
                            sp = ps_s.tile([128, 1024], f32, tag="s")
                            for h in range(2):
                                nc.tensor.matmul(
                                    sp[:, h * NT:(h + 1) * NT],
                                    rk_r[h * 64:(h + 1) * 64, :, s0:s0 + 128],
                                    rq_r[h * 64:(h + 1) * 64, :, t0:t0 + NT],
                                    start=True, stop=True, perf_mode=DR)
                            if tt == 3 and ip >= 4:
                                # fp8e4m3 bit grid is linear in log2, so one
                                # DVE op emits exp(s/8) bits: b = 1.4427*s+56.5
                                nc.vector.scalar_tensor_tensor(
                                    out=epair[:, j * 1024:(j + 1) * 1024].bitcast(mybir.dt.int8),
                                    in0=sp[:], scalar=1.4426950,
                                    in1=b56[:, 0:1].broadcast_to([128, 1024]),
                                    op0=mybir.AluOpType.mult,
                                    op1=mybir.AluOpType.add)
                            else:
                                nc.scalar.activation(
                                    epair[:, j * 1024:(j + 1) * 1024], sp[:],
                                    mybir.ActivationFunctionType.Exp,
                                    bias=0.0, scale=0.125)
                        epairs.append(epair)
                    for d in range(4):
                        sc = n_full + d
                        s0 = sc * 128
                        dlt = d * 128
                        sp = ps_s.tile([128, 1024], f32, tag="s")
                        for h in range(2):
                            nc.tensor.matmul(
                                sp[:, h * NT + dlt:(h + 1) * NT],
                                rk_r[h * 64:(h + 1) * 64, :, s0:s0 + 128],
                                rq_r[h * 64:(h + 1) * 64, :, t0 + dlt:t0 + NT],
                                start=True, stop=True, perf_mode=DR)
                        e_t = (ed16_pool.tile([128, 1024], f16, name="e16")
                               if tt == 0 else ed_pool.tile([128, 1024], f8, name="e8"))
                        s3 = sp[:].rearrange("p (a b) -> p a b", a=2)[:, :, dlt:]
                        e3 = e_t[:].rearrange("p (a b) -> p a b", a=2)[:, :, dlt:]
                        nc.scalar.activation(e3, s3, mybir.ActivationFunctionType.Exp,
                                             bias=0.0, scale=0.125)
                        # causal mask on the 128-wide diagonal slab only:
                        # within the slab keep iff col >= partition
                        nc.gpsimd.affine_select(
                            out=e3[:, :, 0:128], in_=e3[:, :, 0:128],
                            compare_op=mybir.AluOpType.is_ge,
                            fill=0.0, base=0,
                            pattern=[[0, 2], [1, 128]], channel_multiplier=-1)
                        ediags.append(e_t)
                return epairs, ediags

            def emit_attv(c, tt, epairs, ediags, tail=False):
                # att@V accumulation + normalization for window (c, tt),
                # one head at a time so the accumulator is a 2KB [65,512]
                # tile in the ps_q ring (frees PSUM for a 3rd score buffer)
                t0 = tt * NT
                n_full = t0 // 128
                with nc.named_scope("attv"):
                    vsrc = v16_r if tt == 0 else v8_r
                    for h in range(2):
                        vcol = VS * (2 * c + h)
                        yp = ps_q.tile([65, 512], f32, tag="q", name="yp")
                        for ip, epair in enumerate(epairs):
                            epair_r = epair[:].rearrange("p (s x) -> p s x", s=2)
                            nc.tensor.matmul(
                                yp[:], v8_r[:, 2 * ip:2 * ip + 2, vcol:vcol + 65],
                                epair_r[:, :, h * NT:(h + 1) * NT],
                                start=(ip == 0), stop=False, perf_mode=DR,
                                skip_group_check=True)
                        for d, e_t in enumerate(ediags):
                            sc = n_full + d
                            dlt = d * 128
                            nc.tensor.matmul(
                                yp[:, dlt:NT], vsrc[:, sc:sc + 1, vcol:vcol + 65],
                                e_t[:, h * NT + dlt:(h + 1) * NT],
                                start=(tt == 0 and d == 0), stop=(d == 3),
                                skip_group_check=True)

                        # normalization: y *= 1/den, reciprocal on a
                        # DMA-transposed [128,4] layout except on the tail
                        yc = small.tile([65, 512], f16, tag="yc")
                        nc.vector.tensor_copy(yc[:], yp[:])
                        rd = small.tile([1, 512], f16, tag="rd")
                        if tail:
                            nc.vector.reciprocal(rd[:], yc[64:65, :])
                        else:
                            dT = small.tile([128, 4], f16, tag="dT")
                            nc.sync.dma_start(out=dT[:], in_=yc[64:65, :])
                            rdT = small.tile([128, 4], f16, tag="rdT")
                            nc.vector.reciprocal(rdT[:], dT[:])
                            nc.sync.dma_start(out=rd[:], in_=rdT[:])
                        dbc = small.tile([64, 512], f16, tag="dbc")
                        rda = rd[0:1, :]
                        nc.sync.dma_start(out=dbc[:], in_=bass.AP(
                            tensor=rda.tensor, offset=rda.offset,
                            ap=[list(rda.ap)[0], [0, 64], [1, 512]]))
                        nc.vector.tensor_mul(
                            y_all[h * 64:(h + 1) * 64, c * T + t0: c * T + t0 + NT],
                            yc[0:64, :], dbc[:])

            def emit_oproj(tt, cs_=tuple(range(NCHUNK)), dram=None, act_copy=False,
                           wide=False):
                # output projection for t-window tt (all head-chunks of core)
                t0 = tt * NT
                dram = ot_d if dram is None else dram
                with nc.named_scope("oproj"):
                    for ct in range(8):
                        if wide and ct % 2 == 1:
                            po = ps_s.tile([128, 512], f32, tag="s", name="po")
                        else:
                            po = ps_q.tile([128, 512], f32, tag="q", name="po")
                        for c in cs_:
                            nc.tensor.matmul(po[:], wo[:, c * 1024 + ct * 128: c * 1024 + ct * 128 + 128],
                                             y_all[:, c * T + t0: c * T + t0 + NT],
                                             start=(c == cs_[0]), stop=(c == cs_[-1]))
                        ob = osb.tile([128, 512], f32)
                        if act_copy:
                            nc.scalar.activation(ob[:], po[:],
                                                 mybir.ActivationFunctionType.Copy)
                        else:
                            nc.vector.tensor_copy(ob[:], po[:])
                        oq = nc.sync
                        oq.dma_start(ot_d[ct * 128:(ct + 1) * 128, t0:t0 + NT], ob[:])

            # software pipeline over windows w=(c,tt): rope of windows[i+1]
            # and scores+exp of windows[i] stream on PE/Act while attV+norm of
            # windows[i-1] ride behind; v/out-projections interleave early/late.
            # Chunk 3 runs tt descending so the final oproj tail is smallest.
            windows = [(c, tt) for c in range(NCHUNK - 1) for tt in range(TT)]
            windows += [(NCHUNK - 1, tt) for tt in (3, 2, 1, 0)]
            emit_rope(*windows[0])
            pend = None
            for i, (c, tt) in enumerate(windows):
                if i + 1 < len(windows):
                    emit_rope(*windows[i + 1])
                et = emit_scores(c, tt)
                if i == 1:
                    emit_vproj16()
                elif i == 2:
                    emit_vproj8(0, 8)
                elif i == 3:
                    emit_vproj8(8, 16)
                if pend is not None:
                    pc, ptt, pet = pend
                    emit_attv(pc, ptt, *pet)
                    if pc == NCHUNK - 1:
                        emit_oproj(ptt)
                pend = (c, tt, et)
            pc, ptt, pet = pend
            emit_attv(pc, ptt, *pet, tail=True)
            emit_oproj(ptt, act_copy=True, wide=True)

    nc.compile()
    return nc


def _prep_inputs(x, qkv_w, qkv_b):
    """Build the 8 per-core input maps (all host-side numpy)."""
    import ml_dtypes
    f8 = ml_dtypes.float8_e4m3

    x = np.asarray(x, dtype=np.float32)
    qkv_w = np.asarray(qkv_w, dtype=np.float32)
    qkv_b = np.asarray(qkv_b, dtype=np.float32)

    # xt per batch: (KT_AUG*128, T) with row 1024 = ones, rest of aug block 0
    xt8s, xt16s = [], []
    for b in range(B):
        xa = np.zeros((KT_AUG * 128, T), dtype=np.float32)
        xa[:C] = x[b].T
        xa[C] = 1.0
        xt8s.append(xa.astype(f8))
        xt16s.append(xa[:, :NT].astype(np.float16))

    r = np.arange(64)
    d_r = 2 * ((r // 32) * 16 + (r % 16)) + ((r % 32) >= 16)  # row -> head dim
    p = np.arange(128)
    f_p = ((p // 32) % 2) * 16 + (p % 16)

    ins_g = []
    for g in range(2):
        # wqk: [p, c*2048 + kc*256 + which*128 + m] (chunk-major)
        wqk = np.empty((128, KT * 1024), dtype=f8)
        bqk = np.empty((128, 16), dtype=np.float32)
        for c in range(NCHUNK):
            for which in range(2):  # 0=q, 1=k
                rows = np.concatenate([
                    which * C + (8 * g + 2 * c + hh) * 64 + d_r for hh in range(2)
                ])  # 128 feature rows
                blk = qkv_w[rows, :]          # (128 feat, 1024 k)
                for kc in range(KT):
                    sl = slice(c * 2048 + kc * 256 + which * 128,
                               c * 2048 + kc * 256 + which * 128 + 128)
                    wqk[:, sl] = blk[:, kc * 128:(kc + 1) * 128].T.astype(f8)
                bc = qkv_b[rows].astype(np.float32)
                bqk[:, c * 4 + which * 2] = bc
                bqk[:, c * 4 + which * 2 + 1] = bc[p ^ 16]
        # wv: [p, kc*VW + col], col = VS*h + j
        wva = np.zeros((KT_AUG * 128, VW), dtype=np.float32)
        for h in range(HPG):
            rows = 2 * C + (8 * g + h) * 64 + np.arange(64)
            wva[:C, VS * h: VS * h + 64] = qkv_w[rows, :].T
            wva[C, VS * h: VS * h + 64] = qkv_b[rows]
            wva[C, VS * h + 64] = 1.0
        wv8 = np.empty((128, KT_AUG * VW), dtype=f8)
        wv16 = np.empty((128, KT_AUG * VW), dtype=np.float16)
        for kc in range(KT_AUG):
            wv8[:, kc * VW:(kc + 1) * VW] = wva[kc * 128:(kc + 1) * 128].astype(f8)
            wv16[:, kc * VW:(kc + 1) * VW] = wva[kc * 128:(kc + 1) * 128].astype(np.float16)
        ins_g.append((wqk, bqk, wv8, wv16))

    # rope tables
    inv_freq = (1.0 / (ROPE_BASE ** (np.arange(0, D, 2) / D))).astype(np.float64)
    t = np.arange(T, dtype=np.float64)
    ang = t[None, :] * inv_freq[f_p][:, None]          # (128, T)
    cs = np.cos(ang).astype(np.float16)
    sgn = np.where((p % 32) < 16, -1.0, 1.0)[:, None]
    css = (sgn * np.sin(ang)).astype(np.float16)

    return xt8s, xt16s, ins_g, cs, css


def _prep_wo(out_w, g):
    out_w = np.asarray(out_w, dtype=np.float32)
    wo = np.empty((128, NCHUNK * 1024), dtype=np.float16)
    for c in range(NCHUNK):
        rows = np.concatenate([(8 * g + 2 * c + hh) * 64 + np.arange(64) for hh in range(2)])
        wo[:, c * 1024:(c + 1) * 1024] = out_w[:, rows].astype(np.float16).T
    return wo


def _in_maps(x, qkv_w, qkv_b, out_w):
    xt8s, xt16s, ins_g, cs, css = _prep_inputs(x, qkv_w, qkv_b)
    wos = [_prep_wo(out_w, g) for g in range(2)]
    in_maps = []
    for core in range(N_CORES):
        b, g = core // 2, core % 2
        wqk, bqk, wv8, wv16 = ins_g[g]
        in_maps.append({
            "xt8": xt8s[b], "xt16": xt16s[b], "wqk": wqk,
            "wv8": wv8, "wv16": wv16, "wo": wos[g],
            "bqk": bqk, "cs": cs, "css": css,
        })
    return in_maps


def kernel(x, qkv_w, qkv_b, out_w, out_b):
    from concourse.bass_utils import run_bass_kernel_spmd

    if "nc" not in _CACHE:
        _CACHE["nc"] = _build_nc()
    nc = _CACHE["nc"]

    in_maps = _in_maps(x, qkv_w, qkv_b, out_w)
    out_b = np.asarray(out_b, dtype=np.float32)

    try:
        res = run_bass_kernel_spmd(nc, in_maps, core_ids=list(range(N_CORES)))
    except ModuleNotFoundError:
        # BASS_TRACE set but the NTFF profile hook isn't importable here
        import os
        os.environ["BASS_NEVER_TRACE"] = "1"
        res = run_bass_kernel_spmd(nc, in_maps, core_ids=list(range(N_CORES)))

    out = np.empty((B, T, C), dtype=np.float32)
    for b in range(B):
        pt = res.results[2 * b]["ot"] + res.results[2 * b + 1]["ot"]  # (C, T)
        out[b] = pt.T + out_b[None, :]
    return out


# revision 6
# speedup vs baseline: 1.0141x; 1.0065x over previous
"""Causal self-attention (B=4, T=2048, C=1024, H=16, D=64) on 8 TRN2 NeuronCores.

Sharding: core = (batch b, head-group g) with b = core // 2, g = core % 2.
Each core computes heads [8g, 8g+8) of batch b and produces the partial
out-projection (C, T) for its head group; the host sums the two head-group
partials per batch and adds the output bias.

v2 performance structure:
- fp8e4m3 DoubleRow matmuls (2 contraction k-tiles per instruction) for the
  qk-projection, v-projection, scores and att@V. Scores contract over d=64
  only, so their second DoubleRow slot is a zeroed region of the rq/rk
  tiles; att@V pairs adjacent s-tiles. The out-projection and the first
  t-window's attention (t < 512, where softmax averaging is too weak to
  wash out fp8 noise) stay fp16.
- Software pipeline over windows w=(c, tt): rope of windows[i+1] and
  scores+exp of windows[i] stream on PE/Act while attV+norm of windows[i-1]
  ride behind, so the in-order engines never head-of-line block the exp
  stream. V/out-projections are interleaved where they fit; chunk 3 runs
  tt descending so the final out-projection tail is the smallest window.
- exp for the late (most softmax-diffuse) windows is computed on DVE as a
  single scalar_tensor_tensor emitting fp8e4m3 BIT PATTERNS directly
  (the fp8 bit grid is linear in log2, so bits = 1.4427*score + 56.5).
- Softmax denominator: reciprocal on a DMA-transposed [128,4] layout
  (free-dim cost 4 instead of 512), then DMA partition-broadcast. The
  att@V accumulator is per-head [65,512] in the ps_q ring, freeing PSUM
  for a 3-deep score ring that decouples exp from PE's in-order detours.
- Causal masking only touches the 128-wide diagonal slab of each e-tile
  (gpsimd affine_select); fully-masked tiles are never computed.
- All input DMA is fused into ~13 large multi-dim transfers issued from the
  idle SP sequencer, ordered so RoPE/scores inputs land first.
"""

import numpy as np

B, T, C = 4, 2048, 1024
H, D = 16, 64
N_CORES = 8
HPG = H // 2            # heads per core (group)
NCHUNK = 4              # head-pair chunks per core
KT = 8                  # k-tiles of 128 over C
KT_AUG = 9              # + bias/ones k-tile
TT = 4                  # t-tiles of 512 over T
NT = 512                # t tile (matmul N)
VS = 66                 # v column stride per head (64 dims + ones + pad)
VW = HPG * VS           # 528 v columns per k-chunk block
ROPE_BASE = 10000.0

_CACHE = {}


def _build_nc():
    import concourse.bass as bass  # noqa: F401
    import concourse.tile as tile
    from concourse import bacc, mybir
    from contextlib import ExitStack

    f16 = mybir.dt.float16
    f32 = mybir.dt.float32
    f8 = mybir.dt.float8e4
    DR = mybir.MatmulPerfMode.DoubleRow

    nc = bacc.Bacc(
        "TRN2",
        target_bir_lowering=False,
        debug=False,
        enable_asserts=True,
        num_devices=N_CORES,
    )

    xt8_d = nc.dram_tensor("xt8", (KT_AUG * 128, T), f8, kind="ExternalInput").ap()
    xt16_d = nc.dram_tensor("xt16", (KT_AUG * 128, NT), f16, kind="ExternalInput").ap()
    wqk_d = nc.dram_tensor("wqk", (128, KT * 1024), f8, kind="ExternalInput").ap()
    wv8_d = nc.dram_tensor("wv8", (128, KT_AUG * VW), f8, kind="ExternalInput").ap()
    wv16_d = nc.dram_tensor("wv16", (128, KT_AUG * VW), f16, kind="ExternalInput").ap()
    wo_d = nc.dram_tensor("wo", (128, NCHUNK * 1024), f16, kind="ExternalInput").ap()
    bqk_d = nc.dram_tensor("bqk", (128, 16), f32, kind="ExternalInput").ap()
    cs_d = nc.dram_tensor("cs", (128, T), f16, kind="ExternalInput").ap()
    css_d = nc.dram_tensor("css", (128, T), f16, kind="ExternalInput").ap()
    ot_d = nc.dram_tensor("ot", (1024, T), f32, kind="ExternalOutput").ap()

    SHUF = list(range(16, 32)) + list(range(0, 16))

    with tile.TileContext(nc) as tc:
        with ExitStack() as ctx, nc.allow_low_precision("fp8/fp16 attention pipeline"):
            consts = ctx.enter_context(tc.tile_pool(name="consts", bufs=1))
            rtmp = ctx.enter_context(tc.tile_pool(name="rtmp", bufs=4))
            ep_pool = ctx.enter_context(tc.tile_pool(name="ep", bufs=13))
            ed_pool = ctx.enter_context(tc.tile_pool(name="ed", bufs=14))
            ed16_pool = ctx.enter_context(tc.tile_pool(name="ed16", bufs=5))
            small = ctx.enter_context(tc.tile_pool(name="small", bufs=3))
            osb = ctx.enter_context(tc.tile_pool(name="osb", bufs=4))
            ps_q = ctx.enter_context(tc.tile_pool(name="psq", bufs=2, space="PSUM"))
            ps_s = ctx.enter_context(tc.tile_pool(name="pss", bufs=3, space="PSUM"))

            # ---- resident tiles + input DMA ----
            # Fused input DMA (SP-issued, ~13 transfers): xt8 t-block 0 and the
            # chunk-0 qk weights land first so RoPE + first scores start early.
            xt8 = consts.tile([128, KT_AUG * T], f8)
            wqk = consts.tile([128, KT * 1024], f8)
            xt16 = consts.tile([128, KT_AUG * NT], f16)
            wv16 = consts.tile([128, KT_AUG * VW], f16)
            wv8 = consts.tile([128, KT_AUG * VW], f8)
            wo = consts.tile([128, NCHUNK * 1024], f16)
            bqk = consts.tile([128, 16], f32)
            cs = consts.tile([128, T], f16)
            css = consts.tile([128, T], f16)

            xt8_r = xt8[:].rearrange("p (k t) -> p k t", k=KT_AUG)
            xt16_r3 = xt16[:].rearrange("p (k t) -> p k t", k=KT_AUG)

            def dma_xt(dst3, dram, nk, width, c0, w):
                # one transfer covering kc 0..nk of dram rows, cols [c0, c0+w)
                nc.sync.dma_start(
                    dst3[:, 0:nk, c0:c0 + w],
                    bass.AP(tensor=dram.tensor, offset=dram.offset + c0,
                            ap=[[width, 128], [128 * width, nk], [1, w]]))

            dma_xt(xt8_r, xt8_d, KT, T, 0, NT)                      # t-block 0
            nc.sync.dma_start(wqk[:, 0:2048], wqk_d[:, 0:2048])     # chunk 0
            nc.sync.dma_start(bqk[:], bqk_d[:])
            nc.sync.dma_start(cs[:], cs_d[:])
            nc.sync.dma_start(css[:], css_d[:])
            dma_xt(xt8_r, xt8_d, KT_AUG, T, NT, T - NT)             # t-blocks 1-3
            nc.sync.dma_start(xt8_r[:, 8:9, 0:NT],
                              bass.AP(tensor=xt8_d.tensor, offset=xt8_d.offset + 8 * 128 * T,
                                      ap=[[T, 128], [1, NT]]))      # aug t-block 0
            dma_xt(xt16_r3, xt16_d, KT_AUG, NT, 0, NT)
            nc.sync.dma_start(wv16[:], wv16_d[:])
            nc.sync.dma_start(wv8[:], wv8_d[:])
            nc.sync.dma_start(wqk[:, 2048:8192], wqk_d[:, 2048:8192])
            nc.sync.dma_start(wo[:], wo_d[:])

            b56 = consts.tile([128, 1], f32)
            nc.gpsimd.memset(b56[:], 56.5)
            v8 = consts.tile([128, 16 * VW], f8)
            v16 = consts.tile([128, 4 * VW], f16)
            y_all = consts.tile([128, NCHUNK * T], f16)

            # double-buffered q/k tiles: cols [0:T) = rope output (slot A),
            # cols [T:2T) = zeros (slot B of the DoubleRow zero-slot trick)
            rqb = [consts.tile([128, 2 * T], f8, name=f"rq{i}") for i in range(2)]
            rkb = [consts.tile([128, 2 * T], f8, name=f"rk{i}") for i in range(2)]
            for t_ in rqb + rkb:
                nc.gpsimd.memset(t_[:, T:2 * T], 0.0)

            wqk_r = wqk[:].rearrange("p (c k u) -> p c k u", c=NCHUNK, k=KT)
            wv8_r = wv8[:].rearrange("p (k w) -> p k w", k=KT_AUG)
            wv16_r = wv16[:].rearrange("p (k w) -> p k w", k=KT_AUG)
            v8_r = v8[:].rearrange("p (m w) -> p m w", m=16)
            v16_r = v16[:].rearrange("p (m w) -> p m w", m=4)

            def emit_vproj8(m_lo, m_hi):
                with nc.named_scope("vproj8"):
                    for m in range(m_lo, m_hi):  # 128-row t-slices
                        psa = ps_q.tile([128, 512], f32, tag="q")
                        psb = ps_s.tile([128, 1024], f32, tag="s")
                        for kp in range(4):
                            lhsT = xt8_r[:, 2 * kp:2 * kp + 2, m * 128:(m + 1) * 128]
                            nc.tensor.matmul(psa[:], lhsT, wv8_r[:, 2 * kp:2 * kp + 2, 0:512],
                                             start=(kp == 0), stop=False, perf_mode=DR)
                            nc.tensor.matmul(psb[:, 0:16], lhsT,
                                             wv8_r[:, 2 * kp:2 * kp + 2, 512:528],
                                             start=(kp == 0), stop=False, perf_mode=DR)
                        lhs8 = xt8_r[:, 8:9, m * 128:(m + 1) * 128]
                        nc.tensor.matmul(psa[:], lhs8, wv8_r[:, 8:9, 0:512],
                                         start=False, stop=True)
                        nc.tensor.matmul(psb[:, 0:16], lhs8, wv8_r[:, 8:9, 512:528],
                                         start=False, stop=True)
                        nc.vector.tensor_copy(v8[:, m * VW: m * VW + 512], psa[:])
                        nc.vector.tensor_copy(v8[:, m * VW + 512:(m + 1) * VW], psb[:, 0:16])

            def emit_vproj16():
                # accurate fp16 V for s < 512 (feeds the t<512 attention)
                with nc.named_scope("vproj16"):
                    for m in range(4):
                        psa = ps_q.tile([128, 512], f32, tag="q")
                        psb = ps_s.tile([128, 1024], f32, tag="s")
                        for kc in range(KT_AUG):
                            lhs = xt16[:, kc * NT + m * 128: kc * NT + (m + 1) * 128]
                            nc.tensor.matmul(psa[:], lhs, wv16_r[:, kc:kc + 1, 0:512],
                                             start=(kc == 0), stop=(kc == KT_AUG - 1))
                            nc.tensor.matmul(psb[:, 0:16], lhs, wv16_r[:, kc:kc + 1, 512:528],
                                             start=(kc == 0), stop=(kc == KT_AUG - 1))
                        nc.vector.tensor_copy(v16[:, m * VW: m * VW + 512], psa[:])
                        nc.vector.tensor_copy(v16[:, m * VW + 512:(m + 1) * VW], psb[:, 0:16])

            def emit_rope(c, tt):
                # q/k projection + RoPE for heads (2c, 2c+1), t-window tt
                rq = rqb[c % 2]
                rk = rkb[c % 2]
                with nc.named_scope("qkrope"):
                    if True:
                        t0 = tt * NT
                        for which, dst in ((0, rq), (1, rk)):
                            ps = ps_q.tile([128, 512], f32, tag="q")
                            u0 = which * 128
                            for kp in range(4):
                                nc.tensor.matmul(
                                    ps[:],
                                    wqk_r[:, c, 2 * kp:2 * kp + 2, u0:u0 + 128],
                                    xt8_r[:, 2 * kp:2 * kp + 2, t0:t0 + NT],
                                    start=(kp == 0), stop=(kp == 3), perf_mode=DR)
                            bcol = bqk[:, c * 4 + which * 2: c * 4 + which * 2 + 1]
                            bswp = bqk[:, c * 4 + which * 2 + 1: c * 4 + which * 2 + 2]
                            s_t = rtmp.tile([128, 512], f32, tag="st")
                            nc.vector.stream_shuffle(s_t[:], ps[:], SHUF)
                            x1 = rtmp.tile([128, 512], f16, tag="x1")
                            nc.vector.scalar_tensor_tensor(
                                out=x1[:], in0=ps[:], scalar=bcol, in1=cs[:, t0:t0 + NT],
                                op0=mybir.AluOpType.add, op1=mybir.AluOpType.mult)
                            x2 = rtmp.tile([128, 512], f16, tag="x2")
                            nc.vector.scalar_tensor_tensor(
                                out=x2[:], in0=s_t[:], scalar=bswp, in1=css[:, t0:t0 + NT],
                                op0=mybir.AluOpType.add, op1=mybir.AluOpType.mult)
                            nc.gpsimd.tensor_add(dst[:, t0:t0 + NT], x1[:], x2[:])

            def emit_scores(c, tt):
                # scores + exp (+ diagonal mask) for window (c, tt).
                # Returns the e-tiles for emit_attv.
                rq_r = rqb[c % 2][:].rearrange("p (s t) -> p s t", s=2)
                rk_r = rkb[c % 2][:].rearrange("p (s t) -> p s t", s=2)
                t0 = tt * NT
                n_full = t0 // 128
                epairs, ediags = [], []
                with nc.named_scope("scores"):
                    for ip in range(n_full // 2):
                        epair = ep_pool.tile([128, 2048], f8)
                        for j in range(2):
                            sc = 2 * ip + j
                            s0 = sc * 128
                            sp = ps_s.tile([128, 1024], f32, tag="s")
                            for h in range(2):
                                nc.tensor.matmul(
                                    sp[:, h * NT:(h + 1) * NT],
                                    rk_r[h * 64:(h + 1) * 64, :, s0:s0 + 128],
                                    rq_r[h * 64:(h + 1) * 64, :, t0:t0 + NT],
                                    start=True, stop=True, perf_mode=DR)
                            if tt == 3 and ip >= 4:
                                # fp8e4m3 bit grid is linear in log2, so one
                                # DVE op emits exp(s/8) bits: b = 1.4427*s+56.5
                                nc.vector.scalar_tensor_tensor(
                                    out=epair[:, j * 1024:(j + 1) * 1024].bitcast(mybir.dt.int8),
                                    in0=sp[:], scalar=1.4426950,
                                    in1=b56[:, 0:1].broadcast_to([128, 1024]),
                                    op0=mybir.AluOpType.mult,
                                    op1=mybir.AluOpType.add)
                            else:
                                nc.scalar.activation(
                                    epair[:, j * 1024:(j + 1) * 1024], sp[:],
                                    mybir.ActivationFunctionType.Exp,
                                    bias=0.0, scale=0.125)
                        epairs.append(epair)
                    for d in range(4):
                        sc = n_full + d
                        s0 = sc * 128
                        dlt = d * 128
                        sp = ps_s.tile([128, 1024], f32, tag="s")
                        for h in range(2):
                            nc.tensor.matmul(
                                sp[:, h * NT + dlt:(h + 1) * NT],
                                rk_r[h * 64:(h + 1) * 64, :, s0:s0 + 128],
                                rq_r[h * 64:(h + 1) * 64, :, t0 + dlt:t0 + NT],
                                start=True, stop=True, perf_mode=DR)
                        e_t = (ed16_pool.tile([128, 1024], f16, name="e16")
                               if tt == 0 else ed_pool.tile([128, 1024], f8, name="e8"))
                        s3 = sp[:].rearrange("p (a b) -> p a b", a=2)[:, :, dlt:]
                        e3 = e_t[:].rearrange("p (a b) -> p a b", a=2)[:, :, dlt:]
                        nc.scalar.activation(e3, s3, mybir.ActivationFunctionType.Exp,
                                             bias=0.0, scale=0.125)
                        # causal mask on the 128-wide diagonal slab only:
                        # within the slab keep iff col >= partition
                        nc.gpsimd.affine_select(
                            out=e3[:, :, 0:128], in_=e3[:, :, 0:128],
                            compare_op=mybir.AluOpType.is_ge,
                            fill=0.0, base=0,
                            pattern=[[0, 2], [1, 128]], channel_multiplier=-1)
                        ediags.append(e_t)
                return epairs, ediags

            def emit_attv(c, tt, epairs, ediags, tail=False):
                # att@V accumulation + normalization for window (c, tt),
                # one head at a time so the accumulator is a 2KB [65,512]
                # tile in the ps_q ring (frees PSUM for a 3rd score buffer)
                t0 = tt * NT
                n_full = t0 // 128
                with nc.named_scope("attv"):
                    vsrc = v16_r if tt == 0 else v8_r
                    for h in range(2):
                        vcol = VS * (2 * c + h)
                        yp = ps_q.tile([65, 512], f32, tag="q", name="yp")
                        for ip, epair in enumerate(epairs):
                            epair_r = epair[:].rearrange("p (s x) -> p s x", s=2)
                            nc.tensor.matmul(
                                yp[:], v8_r[:, 2 * ip:2 * ip + 2, vcol:vcol + 65],
                                epair_r[:, :, h * NT:(h + 1) * NT],
                                start=(ip == 0), stop=False, perf_mode=DR,
                                skip_group_check=True)
                        for d, e_t in enumerate(ediags):
                            sc = n_full + d
                            dlt = d * 128
                            nc.tensor.matmul(
                                yp[:, dlt:NT], vsrc[:, sc:sc + 1, vcol:vcol + 65],
                                e_t[:, h * NT + dlt:(h + 1) * NT],
                                start=(tt == 0 and d == 0), stop=(d == 3),
                                skip_group_check=True)

                        # normalization: y *= 1/den, reciprocal on a
                        # DMA-transposed [128,4] layout except on the tail
                        yc = small.tile([65, 512], f16, tag="yc")
                        nc.vector.tensor_copy(yc[:], yp[:])
                        rd = small.tile([1, 512], f16, tag="rd")
                        if tail:
                            nc.vector.reciprocal(rd[:], yc[64:65, :])
                        else:
                            dT = small.tile([128, 4], f16, tag="dT")
                            nc.sync.dma_start(out=dT[:], in_=yc[64:65, :])
                            rdT = small.tile([128, 4], f16, tag="rdT")
                            nc.vector.reciprocal(rdT[:], dT[:])
                            nc.sync.dma_start(out=rd[:], in_=rdT[:])
                        dbc = small.tile([64, 512], f16, tag="dbc")
                        rda = rd[0:1, :]
                        nc.sync.dma_start(out=dbc[:], in_=bass.AP(
                            tensor=rda.tensor, offset=rda.offset,
                            ap=[list(rda.ap)[0], [0, 64], [1, 512]]))
                        nc.vector.tensor_mul(
                            y_all[h * 64:(h + 1) * 64, c * T + t0: c * T + t0 + NT],
                            yc[0:64, :], dbc[:])

            def emit_oproj(tt, cs_=tuple(range(NCHUNK)), dram=None, act_copy=False,
                           wide=False):
                # output projection for t-window tt (all head-chunks of core)
                t0 = tt * NT
                dram = ot_d if dram is None else dram
                with nc.named_scope("oproj"):
                    for ct in range(8):
                        if wide and ct % 2 == 1:
                            po = ps_s.tile([128, 512], f32, tag="s", name="po")
                        else:
                            po = ps_q.tile([128, 512], f32, tag="q", name="po")
                        for c in cs_:
                            nc.tensor.matmul(po[:], wo[:, c * 1024 + ct * 128: c * 1024 + ct * 128 + 128],
                                             y_all[:, c * T + t0: c * T + t0 + NT],
                                             start=(c == cs_[0]), stop=(c == cs_[-1]))
                        ob = osb.tile([128, 512], f32)
                        if act_copy:
                            nc.scalar.activation(ob[:], po[:],
                                                 mybir.ActivationFunctionType.Copy)
                        else:
                            nc.vector.tensor_copy(ob[:], po[:])
                        oq = nc.sync
                        oq.dma_start(ot_d[ct * 128:(ct + 1) * 128, t0:t0 + NT], ob[:])

            # software pipeline over windows w=(c,tt): rope of windows[i+1]
            # and scores+exp of windows[i] stream on PE/Act while attV+norm of
            # windows[i-1] ride behind; v/out-projections interleave early/late.
            # Chunk 3 runs tt descending so the final oproj tail is smallest.
            windows = [(c, tt) for c in range(NCHUNK - 1) for tt in range(TT)]
            windows += [(NCHUNK - 1, tt) for tt in (3, 2, 1, 0)]
            emit_rope(*windows[0])
            pend = None
            for i, (c, tt) in enumerate(windows):
                if i + 1 < len(windows):
                    emit_rope(*windows[i + 1])
                et = emit_scores(c, tt)
                if i == 1:
                    emit_vproj16()
                elif i == 2:
                    emit_vproj8(0, 8)
                elif i == 3:
                    emit_vproj8(8, 16)
                if pend is not None:
                    pc, ptt, pet = pend
                    emit_attv(pc, ptt, *pet)
                    if pc == NCHUNK - 1:
                        # the last windows have no scores left to feed, so
                        # their po tiles can use the freed ps_s slots too
                        emit_oproj(ptt, wide=(ptt <= 1), act_copy=(ptt == 1))
                pend = (c, tt, et)
            pc, ptt, pet = pend
            emit_attv(pc, ptt, *pet, tail=True)
            emit_oproj(ptt, act_copy=True, wide=True)

    nc.compile()
    return nc


def _prep_inputs(x, qkv_w, qkv_b):
    """Build the 8 per-core input maps (all host-side numpy)."""
    import ml_dtypes
    f8 = ml_dtypes.float8_e4m3

    x = np.asarray(x, dtype=np.float32)
    qkv_w = np.asarray(qkv_w, dtype=np.float32)
    qkv_b = np.asarray(qkv_b, dtype=np.float32)

    # xt per batch: (KT_AUG*128, T) with row 1024 = ones, rest of aug block 0
    xt8s, xt16s = [], []
    for b in range(B):
        xa = np.zeros((KT_AUG * 128, T), dtype=np.float32)
        xa[:C] = x[b].T
        xa[C] = 1.0
        xt8s.append(xa.astype(f8))
        xt16s.append(xa[:, :NT].astype(np.float16))

    r = np.arange(64)
    d_r = 2 * ((r // 32) * 16 + (r % 16)) + ((r % 32) >= 16)  # row -> head dim
    p = np.arange(128)
    f_p = ((p // 32) % 2) * 16 + (p % 16)

    ins_g = []
    for g in range(2):
        # wqk: [p, c*2048 + kc*256 + which*128 + m] (chunk-major)
        wqk = np.empty((128, KT * 1024), dtype=f8)
        bqk = np.empty((128, 16), dtype=np.float32)
        for c in range(NCHUNK):
            for which in range(2):  # 0=q, 1=k
                rows = np.concatenate([
                    which * C + (8 * g + 2 * c + hh) * 64 + d_r for hh in range(2)
                ])  # 128 feature rows
                blk = qkv_w[rows, :]          # (128 feat, 1024 k)
                for kc in range(KT):
                    sl = slice(c * 2048 + kc * 256 + which * 128,
                               c * 2048 + kc * 256 + which * 128 + 128)
                    wqk[:, sl] = blk[:, kc * 128:(kc + 1) * 128].T.astype(f8)
                bc = qkv_b[rows].astype(np.float32)
                bqk[:, c * 4 + which * 2] = bc
                bqk[:, c * 4 + which * 2 + 1] = bc[p ^ 16]
        # wv: [p, kc*VW + col], col = VS*h + j
        wva = np.zeros((KT_AUG * 128, VW), dtype=np.float32)
        for h in range(HPG):
            rows = 2 * C + (8 * g + h) * 64 + np.arange(64)
            wva[:C, VS * h: VS * h + 64] = qkv_w[rows, :].T
            wva[C, VS * h: VS * h + 64] = qkv_b[rows]
            wva[C, VS * h + 64] = 1.0
        wv8 = np.empty((128, KT_AUG * VW), dtype=f8)
        wv16 = np.empty((128, KT_AUG * VW), dtype=np.float16)
        for kc in range(KT_AUG):
            wv8[:, kc * VW:(kc + 1) * VW] = wva[kc * 128:(kc + 1) * 128].astype(f8)
            wv16[:, kc * VW:(kc + 1) * VW] = wva[kc * 128:(kc + 1) * 128].astype(np.float16)
        ins_g.append((wqk, bqk, wv8, wv16))

    # rope tables
    inv_freq = (1.0 / (ROPE_BASE ** (np.arange(0, D, 2) / D))).astype(np.float64)
    t = np.arange(T, dtype=np.float64)
    ang = t[None, :] * inv_freq[f_p][:, None]          # (128, T)
    cs = np.cos(ang).astype(np.float16)
    sgn = np.where((p % 32) < 16, -1.0, 1.0)[:, None]
    css = (sgn * np.sin(ang)).astype(np.float16)

    return xt8s, xt16s, ins_g, cs, css


def _prep_wo(out_w, g):
    out_w = np.asarray(out_w, dtype=np.float32)
    wo = np.empty((128, NCHUNK * 1024), dtype=np.float16)
    for c in range(NCHUNK):
        rows = np.concatenate([(8 * g + 2 * c + hh) * 64 + np.arange(64) for hh in range(2)])
        wo[:, c * 1024:(c + 1) * 1024] = out_w[:, rows].astype(np.float16).T
    return wo


def _in_maps(x, qkv_w, qkv_b, out_w):
    xt8s, xt16s, ins_g, cs, css = _prep_inputs(x, qkv_w, qkv_b)
    wos = [_prep_wo(out_w, g) for g in range(2)]
    in_maps = []
    for core in range(N_CORES):
        b, g = core // 2, core % 2
        wqk, bqk, wv8, wv16 = ins_g[g]
        in_maps.append({
            "xt8": xt8s[b], "xt16": xt16s[b], "wqk": wqk,
            "wv8": wv8, "wv16": wv16, "wo": wos[g],
            "bqk": bqk, "cs": cs, "css": css,
        })
    return in_maps


def kernel(x, qkv_w, qkv_b, out_w, out_b):
    from concourse.bass_utils import run_bass_kernel_spmd

    if "nc" not in _CACHE:
        _CACHE["nc"] = _build_nc()
    nc = _CACHE["nc"]

    in_maps = _in_maps(x, qkv_w, qkv_b, out_w)
    out_b = np.asarray(out_b, dtype=np.float32)

    try:
        res = run_bass_kernel_spmd(nc, in_maps, core_ids=list(range(N_CORES)))
    except ModuleNotFoundError:
        # BASS_TRACE set but the NTFF profile hook isn't importable here
        import os
        os.environ["BASS_NEVER_TRACE"] = "1"
        res = run_bass_kernel_spmd(nc, in_maps, core_ids=list(range(N_CORES)))

    out = np.empty((B, T, C), dtype=np.float32)
    for b in range(B):
        pt = res.results[2 * b]["ot"] + res.results[2 * b + 1]["ot"]  # (C, T)
        out[b] = pt.T + out_b[None, :]
    return out


# revision 7
# speedup vs baseline: 1.0314x; 1.0171x over previous
"""Causal self-attention (B=4, T=2048, C=1024, H=16, D=64) on 8 TRN2 NeuronCores.

Sharding: core = (batch b, head-group g) with b = core // 2, g = core % 2.
Each core computes heads [8g, 8g+8) of batch b and produces the partial
out-projection (C, T) for its head group; the host sums the two head-group
partials per batch and adds the output bias.

v2 performance structure:
- fp8e4m3 DoubleRow matmuls (2 contraction k-tiles per instruction) for the
  qk-projection, v-projection, scores and att@V. Scores contract over d=64
  only, so their second DoubleRow slot is a zeroed region of the rq/rk
  tiles; att@V pairs adjacent s-tiles. The out-projection and the first
  t-window's attention (t < 512, where softmax averaging is too weak to
  wash out fp8 noise) stay fp16.
- Software pipeline over windows w=(c, tt): rope of windows[i+1] and
  scores+exp of windows[i] stream on PE/Act while attV+norm of windows[i-1]
  ride behind, so the in-order engines never head-of-line block the exp
  stream. V/out-projections are interleaved where they fit; chunk 3 runs
  tt descending so the final out-projection tail is the smallest window.
- exp for the late (most softmax-diffuse) windows is computed on DVE as a
  single scalar_tensor_tensor emitting fp8e4m3 BIT PATTERNS directly
  (the fp8 bit grid is linear in log2, so bits = 1.4427*score + 56.5).
- Softmax denominator: reciprocal on a DMA-transposed [128,4] layout
  (free-dim cost 4 instead of 512), then DMA partition-broadcast. The
  att@V accumulator is per-head [65,512] in the ps_q ring, freeing PSUM
  for a 3-deep score ring that decouples exp from PE's in-order detours.
- Causal masking only touches the 128-wide diagonal slab of each e-tile
  (gpsimd affine_select); fully-masked tiles are never computed.
- All input DMA is fused into ~13 large multi-dim transfers issued from the
  idle SP sequencer, ordered so RoPE/scores inputs land first.
"""

import numpy as np

B, T, C = 4, 2048, 1024
H, D = 16, 64
N_CORES = 8
HPG = H // 2            # heads per core (group)
NCHUNK = 4              # head-pair chunks per core
KT = 8                  # k-tiles of 128 over C
KT_AUG = 9              # + bias/ones k-tile
TT = 4                  # t-tiles of 512 over T
NT = 512                # t tile (matmul N)
VS = 66                 # v column stride per head (64 dims + ones + pad)
VW = HPG * VS           # 528 v columns per k-chunk block
ROPE_BASE = 10000.0

_CACHE = {}


def _build_nc():
    import concourse.bass as bass  # noqa: F401
    import concourse.tile as tile
    from concourse import bacc, mybir
    from contextlib import ExitStack

    f16 = mybir.dt.float16
    f32 = mybir.dt.float32
    f8 = mybir.dt.float8e4
    DR = mybir.MatmulPerfMode.DoubleRow

    nc = bacc.Bacc(
        "TRN2",
        target_bir_lowering=False,
        debug=False,
        enable_asserts=True,
        num_devices=N_CORES,
    )

    xt8_d = nc.dram_tensor("xt8", (KT_AUG * 128, T), f8, kind="ExternalInput").ap()
    xt16_d = nc.dram_tensor("xt16", (KT_AUG * 128, NT), f16, kind="ExternalInput").ap()
    wqk_d = nc.dram_tensor("wqk", (128, KT * 1024), f8, kind="ExternalInput").ap()
    wv8_d = nc.dram_tensor("wv8", (128, KT_AUG * VW), f8, kind="ExternalInput").ap()
    wv16_d = nc.dram_tensor("wv16", (128, KT_AUG * VW), f16, kind="ExternalInput").ap()
    wo_d = nc.dram_tensor("wo", (128, NCHUNK * 1024), f16, kind="ExternalInput").ap()
    bqk_d = nc.dram_tensor("bqk", (128, 16), f32, kind="ExternalInput").ap()
    cs_d = nc.dram_tensor("cs", (128, T), f16, kind="ExternalInput").ap()
    css_d = nc.dram_tensor("css", (128, T), f16, kind="ExternalInput").ap()
    ot_d = nc.dram_tensor("ot", (1024, T), f32, kind="ExternalOutput").ap()

    SHUF = list(range(16, 32)) + list(range(0, 16))

    with tile.TileContext(nc) as tc:
        with ExitStack() as ctx, nc.allow_low_precision("fp8/fp16 attention pipeline"):
            consts = ctx.enter_context(tc.tile_pool(name="consts", bufs=1))
            rtmp = ctx.enter_context(tc.tile_pool(name="rtmp", bufs=4))
            ep_pool = ctx.enter_context(tc.tile_pool(name="ep", bufs=14))
            ed_pool = ctx.enter_context(tc.tile_pool(name="ed", bufs=14))
            ed16_pool = ctx.enter_context(tc.tile_pool(name="ed16", bufs=5))
            small = ctx.enter_context(tc.tile_pool(name="small", bufs=3))
            osb = ctx.enter_context(tc.tile_pool(name="osb", bufs=6))
            ps_q = ctx.enter_context(tc.tile_pool(name="psq", bufs=2, space="PSUM"))
            ps_s = ctx.enter_context(tc.tile_pool(name="pss", bufs=3, space="PSUM"))

            # ---- resident tiles + input DMA ----
            # Fused input DMA (SP-issued, ~13 transfers): xt8 t-block 0 and the
            # chunk-0 qk weights land first so RoPE + first scores start early.
            xt8 = consts.tile([128, KT_AUG * T], f8)
            wqk = consts.tile([128, KT * 1024], f8)
            xt16 = consts.tile([128, KT_AUG * NT], f16)
            wv16 = consts.tile([128, KT_AUG * VW], f16)
            wv8 = consts.tile([128, KT_AUG * VW], f8)
            wo = consts.tile([128, NCHUNK * 1024], f16)
            bqk = consts.tile([128, 16], f32)
            cs = consts.tile([128, T], f16)
            css = consts.tile([128, T], f16)

            xt8_r = xt8[:].rearrange("p (k t) -> p k t", k=KT_AUG)
            xt16_r3 = xt16[:].rearrange("p (k t) -> p k t", k=KT_AUG)

            def dma_xt(dst3, dram, nk, width, c0, w):
                # one transfer covering kc 0..nk of dram rows, cols [c0, c0+w)
                nc.sync.dma_start(
                    dst3[:, 0:nk, c0:c0 + w],
                    bass.AP(tensor=dram.tensor, offset=dram.offset + c0,
                            ap=[[width, 128], [128 * width, nk], [1, w]]))

            dma_xt(xt8_r, xt8_d, KT, T, 0, NT)                      # t-block 0
            nc.sync.dma_start(wqk[:, 0:2048], wqk_d[:, 0:2048])     # chunk 0
            nc.sync.dma_start(bqk[:], bqk_d[:])
            nc.sync.dma_start(cs[:], cs_d[:])
            nc.sync.dma_start(css[:], css_d[:])
            dma_xt(xt8_r, xt8_d, KT_AUG, T, NT, T - NT)             # t-blocks 1-3
            nc.sync.dma_start(xt8_r[:, 8:9, 0:NT],
                              bass.AP(tensor=xt8_d.tensor, offset=xt8_d.offset + 8 * 128 * T,
                                      ap=[[T, 128], [1, NT]]))      # aug t-block 0
            dma_xt(xt16_r3, xt16_d, KT_AUG, NT, 0, NT)
            nc.sync.dma_start(wv16[:], wv16_d[:])
            nc.sync.dma_start(wv8[:], wv8_d[:])
            nc.sync.dma_start(wqk[:, 2048:8192], wqk_d[:, 2048:8192])
            nc.sync.dma_start(wo[:], wo_d[:])

            b56 = consts.tile([128, 1], f32)
            nc.gpsimd.memset(b56[:], 56.5)
            v8 = consts.tile([128, 16 * VW], f8)
            v16 = consts.tile([128, 4 * VW], f16)
            y_all = consts.tile([128, NCHUNK * T], f16)

            # double-buffered q/k tiles: cols [0:T) = rope output (slot A),
            # cols [T:2T) = zeros (slot B of the DoubleRow zero-slot trick)
            rqb = [consts.tile([128, 2 * T], f8, name=f"rq{i}") for i in range(2)]
            rkb = [consts.tile([128, 2 * T], f8, name=f"rk{i}") for i in range(2)]
            for t_ in rqb + rkb:
                nc.gpsimd.memset(t_[:, T:2 * T], 0.0)

            wqk_r = wqk[:].rearrange("p (c k u) -> p c k u", c=NCHUNK, k=KT)
            wv8_r = wv8[:].rearrange("p (k w) -> p k w", k=KT_AUG)
            wv16_r = wv16[:].rearrange("p (k w) -> p k w", k=KT_AUG)
            v8_r = v8[:].rearrange("p (m w) -> p m w", m=16)
            v16_r = v16[:].rearrange("p (m w) -> p m w", m=4)

            def emit_vproj8(m_lo, m_hi):
                with nc.named_scope("vproj8"):
                    for m in range(m_lo, m_hi):  # 128-row t-slices
                        psa = ps_q.tile([128, 512], f32, tag="q")
                        psb = ps_s.tile([128, 1024], f32, tag="s")
                        for kp in range(4):
                            lhsT = xt8_r[:, 2 * kp:2 * kp + 2, m * 128:(m + 1) * 128]
                            nc.tensor.matmul(psa[:], lhsT, wv8_r[:, 2 * kp:2 * kp + 2, 0:512],
                                             start=(kp == 0), stop=False, perf_mode=DR)
                            nc.tensor.matmul(psb[:, 0:16], lhsT,
                                             wv8_r[:, 2 * kp:2 * kp + 2, 512:528],
                                             start=(kp == 0), stop=False, perf_mode=DR)
                        lhs8 = xt8_r[:, 8:9, m * 128:(m + 1) * 128]
                        nc.tensor.matmul(psa[:], lhs8, wv8_r[:, 8:9, 0:512],
                                         start=False, stop=True)
                        nc.tensor.matmul(psb[:, 0:16], lhs8, wv8_r[:, 8:9, 512:528],
                                         start=False, stop=True)
                        nc.vector.tensor_copy(v8[:, m * VW: m * VW + 512], psa[:])
                        nc.vector.tensor_copy(v8[:, m * VW + 512:(m + 1) * VW], psb[:, 0:16])

            def emit_vproj16():
                # accurate fp16 V for s < 512 (feeds the t<512 attention)
                with nc.named_scope("vproj16"):
                    for m in range(4):
                        psa = ps_q.tile([128, 512], f32, tag="q")
                        psb = ps_s.tile([128, 1024], f32, tag="s")
                        for kc in range(KT_AUG):
                            lhs = xt16[:, kc * NT + m * 128: kc * NT + (m + 1) * 128]
                            nc.tensor.matmul(psa[:], lhs, wv16_r[:, kc:kc + 1, 0:512],
                                             start=(kc == 0), stop=(kc == KT_AUG - 1))
                            nc.tensor.matmul(psb[:, 0:16], lhs, wv16_r[:, kc:kc + 1, 512:528],
                                             start=(kc == 0), stop=(kc == KT_AUG - 1))
                        nc.vector.tensor_copy(v16[:, m * VW: m * VW + 512], psa[:])
                        nc.vector.tensor_copy(v16[:, m * VW + 512:(m + 1) * VW], psb[:, 0:16])

            def emit_rope(c, tt):
                # q/k projection + RoPE for heads (2c, 2c+1), t-window tt
                rq = rqb[c % 2]
                rk = rkb[c % 2]
                with nc.named_scope("qkrope"):
                    if True:
                        t0 = tt * NT
                        for which, dst in ((0, rq), (1, rk)):
                            ps = ps_q.tile([128, 512], f32, tag="q")
                            u0 = which * 128
                            for kp in range(4):
                                nc.tensor.matmul(
                                    ps[:],
                                    wqk_r[:, c, 2 * kp:2 * kp + 2, u0:u0 + 128],
                                    xt8_r[:, 2 * kp:2 * kp + 2, t0:t0 + NT],
                                    start=(kp == 0), stop=(kp == 3), perf_mode=DR)
                            bcol = bqk[:, c * 4 + which * 2: c * 4 + which * 2 + 1]
                            bswp = bqk[:, c * 4 + which * 2 + 1: c * 4 + which * 2 + 2]
                            s_t = rtmp.tile([128, 512], f32, tag="st")
                            nc.vector.stream_shuffle(s_t[:], ps[:], SHUF)
                            x1 = rtmp.tile([128, 512], f16, tag="x1")
                            nc.vector.scalar_tensor_tensor(
                                out=x1[:], in0=ps[:], scalar=bcol, in1=cs[:, t0:t0 + NT],
                                op0=mybir.AluOpType.add, op1=mybir.AluOpType.mult)
                            x2 = rtmp.tile([128, 512], f16, tag="x2")
                            nc.vector.scalar_tensor_tensor(
                                out=x2[:], in0=s_t[:], scalar=bswp, in1=css[:, t0:t0 + NT],
                                op0=mybir.AluOpType.add, op1=mybir.AluOpType.mult)
                            nc.gpsimd.tensor_add(dst[:, t0:t0 + NT], x1[:], x2[:])

            def emit_scores(c, tt):
                # scores + exp (+ diagonal mask) for window (c, tt).
                # Returns the e-tiles for emit_attv.
                rq_r = rqb[c % 2][:].rearrange("p (s t) -> p s t", s=2)
                rk_r = rkb[c % 2][:].rearrange("p (s t) -> p s t", s=2)
                t0 = tt * NT
                n_full = t0 // 128
                epairs, ediags = [], []
                with nc.named_scope("scores"):
                    for ip in range(n_full // 2):
                        epair = ep_pool.tile([128, 2048], f8)
                        for j in range(2):
                            sc = 2 * ip + j
                            s0 = sc * 128
                            sp = ps_s.tile([128, 1024], f32, tag="s")
                            for h in range(2):
                                nc.tensor.matmul(
                                    sp[:, h * NT:(h + 1) * NT],
                                    rk_r[h * 64:(h + 1) * 64, :, s0:s0 + 128],
                                    rq_r[h * 64:(h + 1) * 64, :, t0:t0 + NT],
                                    start=True, stop=True, perf_mode=DR)
                            if tt == 3 and ip >= 4:
                                # fp8e4m3 bit grid is linear in log2, so one
                                # DVE op emits exp(s/8) bits: b = 1.4427*s+56.5
                                nc.vector.scalar_tensor_tensor(
                                    out=epair[:, j * 1024:(j + 1) * 1024].bitcast(mybir.dt.int8),
                                    in0=sp[:], scalar=1.4426950,
                                    in1=b56[:, 0:1].broadcast_to([128, 1024]),
                                    op0=mybir.AluOpType.mult,
                                    op1=mybir.AluOpType.add)
                            else:
                                nc.scalar.activation(
                                    epair[:, j * 1024:(j + 1) * 1024], sp[:],
                                    mybir.ActivationFunctionType.Exp,
                                    bias=0.0, scale=0.125)
                        epairs.append(epair)
                    for d in range(4):
                        sc = n_full + d
                        s0 = sc * 128
                        dlt = d * 128
                        sp = ps_s.tile([128, 1024], f32, tag="s")
                        for h in range(2):
                            nc.tensor.matmul(
                                sp[:, h * NT + dlt:(h + 1) * NT],
                                rk_r[h * 64:(h + 1) * 64, :, s0:s0 + 128],
                                rq_r[h * 64:(h + 1) * 64, :, t0 + dlt:t0 + NT],
                                start=True, stop=True, perf_mode=DR)
                        e_t = (ed16_pool.tile([128, 1024], f16, name="e16")
                               if tt == 0 else ed_pool.tile([128, 1024], f8, name="e8"))
                        s3 = sp[:].rearrange("p (a b) -> p a b", a=2)[:, :, dlt:]
                        e3 = e_t[:].rearrange("p (a b) -> p a b", a=2)[:, :, dlt:]
                        nc.scalar.activation(e3, s3, mybir.ActivationFunctionType.Exp,
                                             bias=0.0, scale=0.125)
                        # causal mask on the 128-wide diagonal slab only:
                        # within the slab keep iff col >= partition
                        nc.gpsimd.affine_select(
                            out=e3[:, :, 0:128], in_=e3[:, :, 0:128],
                            compare_op=mybir.AluOpType.is_ge,
                            fill=0.0, base=0,
                            pattern=[[0, 2], [1, 128]], channel_multiplier=-1)
                        ediags.append(e_t)
                return epairs, ediags

            def emit_attv(c, tt, epairs, ediags, tail=False):
                # att@V accumulation + normalization for window (c, tt),
                # one head at a time so the accumulator is a 2KB [65,512]
                # tile in the ps_q ring (frees PSUM for a 3rd score buffer)
                t0 = tt * NT
                n_full = t0 // 128
                with nc.named_scope("attv"):
                    vsrc = v16_r if tt == 0 else v8_r
                    for h in range(2):
                        vcol = VS * (2 * c + h)
                        yp = ps_q.tile([65, 512], f32, tag="q", name="yp")
                        for ip, epair in enumerate(epairs):
                            epair_r = epair[:].rearrange("p (s x) -> p s x", s=2)
                            nc.tensor.matmul(
                                yp[:], v8_r[:, 2 * ip:2 * ip + 2, vcol:vcol + 65],
                                epair_r[:, :, h * NT:(h + 1) * NT],
                                start=(ip == 0), stop=False, perf_mode=DR,
                                skip_group_check=True)
                        for d, e_t in enumerate(ediags):
                            sc = n_full + d
                            dlt = d * 128
                            nc.tensor.matmul(
                                yp[:, dlt:NT], vsrc[:, sc:sc + 1, vcol:vcol + 65],
                                e_t[:, h * NT + dlt:(h + 1) * NT],
                                start=(tt == 0 and d == 0), stop=(d == 3),
                                skip_group_check=True)

                        # normalization: y *= 1/den, reciprocal on a
                        # DMA-transposed [128,4] layout except on the tail
                        yc = small.tile([65, 512], f16, tag="yc")
                        nc.vector.tensor_copy(yc[:], yp[:])
                        rd = small.tile([1, 512], f16, tag="rd")
                        if tail:
                            nc.vector.reciprocal(rd[:], yc[64:65, :])
                        else:
                            dT = small.tile([128, 4], f16, tag="dT")
                            nc.sync.dma_start(out=dT[:], in_=yc[64:65, :])
                            rdT = small.tile([128, 4], f16, tag="rdT")
                            nc.vector.reciprocal(rdT[:], dT[:])
                            nc.sync.dma_start(out=rd[:], in_=rdT[:])
                        dbc = small.tile([64, 512], f16, tag="dbc")
                        rda = rd[0:1, :]
                        nc.sync.dma_start(out=dbc[:], in_=bass.AP(
                            tensor=rda.tensor, offset=rda.offset,
                            ap=[list(rda.ap)[0], [0, 64], [1, 512]]))
                        nc.vector.tensor_mul(
                            y_all[h * 64:(h + 1) * 64, c * T + t0: c * T + t0 + NT],
                            yc[0:64, :], dbc[:])

            def emit_oproj(tt, cs_=tuple(range(NCHUNK)), dram=None, act_copy=False,
                           wide=False):
                # output projection for t-window tt (all head-chunks of core)
                t0 = tt * NT
                dram = ot_d if dram is None else dram
                with nc.named_scope("oproj"):
                    for ct in range(8):
                        if wide and ct % 2 == 1:
                            po = ps_s.tile([128, 512], f32, tag="s", name="po")
                        else:
                            po = ps_q.tile([128, 512], f32, tag="q", name="po")
                        for c in cs_:
                            nc.tensor.matmul(po[:], wo[:, c * 1024 + ct * 128: c * 1024 + ct * 128 + 128],
                                             y_all[:, c * T + t0: c * T + t0 + NT],
                                             start=(c == cs_[0]), stop=(c == cs_[-1]))
                        ob = osb.tile([128, 512], f32)
                        if act_copy:
                            nc.scalar.activation(ob[:], po[:],
                                                 mybir.ActivationFunctionType.Copy)
                        else:
                            nc.vector.tensor_copy(ob[:], po[:])
                        oq = nc.sync
                        oq.dma_start(ot_d[ct * 128:(ct + 1) * 128, t0:t0 + NT], ob[:])

            # software pipeline over windows w=(c,tt): rope of windows[i+1]
            # and scores+exp of windows[i] stream on PE/Act while attV+norm of
            # windows[i-1] ride behind; v/out-projections interleave early/late.
            # Chunk 3 runs tt descending so the final oproj tail is smallest.
            windows = [(c, tt) for c in range(NCHUNK - 1) for tt in range(TT)]
            windows += [(NCHUNK - 1, tt) for tt in (3, 2, 1, 0)]
            emit_rope(*windows[0])
            pend = None
            for i, (c, tt) in enumerate(windows):
                if i + 1 < len(windows):
                    emit_rope(*windows[i + 1])
                et = emit_scores(c, tt)
                if i == 1:
                    emit_vproj16()
                elif i == 2:
                    emit_vproj8(0, 8)
                elif i == 3:
                    emit_vproj8(8, 16)
                if pend is not None:
                    pc, ptt, pet = pend
                    emit_attv(pc, ptt, *pet)
                    if pc == NCHUNK - 1:
                        # the last windows have no scores left to feed, so
                        # their po tiles can use the freed ps_s slots too
                        emit_oproj(ptt, wide=(ptt <= 1), act_copy=(ptt == 1))
                pend = (c, tt, et)
            pc, ptt, pet = pend
            emit_attv(pc, ptt, *pet, tail=True)
            emit_oproj(ptt, act_copy=True, wide=True)

    nc.compile()
    return nc


def _prep_inputs(x, qkv_w, qkv_b):
    """Build the 8 per-core input maps (all host-side numpy)."""
    import ml_dtypes
    f8 = ml_dtypes.float8_e4m3

    x = np.asarray(x, dtype=np.float32)
    qkv_w = np.asarray(qkv_w, dtype=np.float32)
    qkv_b = np.asarray(qkv_b, dtype=np.float32)

    # xt per batch: (KT_AUG*128, T) with row 1024 = ones, rest of aug block 0
    xt8s, xt16s = [], []
    for b in range(B):
        xa = np.zeros((KT_AUG * 128, T), dtype=np.float32)
        xa[:C] = x[b].T
        xa[C] = 1.0
        xt8s.append(xa.astype(f8))
        xt16s.append(xa[:, :NT].astype(np.float16))

    r = np.arange(64)
    d_r = 2 * ((r // 32) * 16 + (r % 16)) + ((r % 32) >= 16)  # row -> head dim
    p = np.arange(128)
    f_p = ((p // 32) % 2) * 16 + (p % 16)

    ins_g = []
    for g in range(2):
        # wqk: [p, c*2048 + kc*256 + which*128 + m] (chunk-major)
        wqk = np.empty((128, KT * 1024), dtype=f8)
        bqk = np.empty((128, 16), dtype=np.float32)
        for c in range(NCHUNK):
            for which in range(2):  # 0=q, 1=k
                rows = np.concatenate([
                    which * C + (8 * g + 2 * c + hh) * 64 + d_r for hh in range(2)
                ])  # 128 feature rows
                blk = qkv_w[rows, :]          # (128 feat, 1024 k)
                for kc in range(KT):
                    sl = slice(c * 2048 + kc * 256 + which * 128,
                               c * 2048 + kc * 256 + which * 128 + 128)
                    wqk[:, sl] = blk[:, kc * 128:(kc + 1) * 128].T.astype(f8)
                bc = qkv_b[rows].astype(np.float32)
                bqk[:, c * 4 + which * 2] = bc
                bqk[:, c * 4 + which * 2 + 1] = bc[p ^ 16]
        # wv: [p, kc*VW + col], col = VS*h + j
        wva = np.zeros((KT_AUG * 128, VW), dtype=np.float32)
        for h in range(HPG):
            rows = 2 * C + (8 * g + h) * 64 + np.arange(64)
            wva[:C, VS * h: VS * h + 64] = qkv_w[rows, :].T
            wva[C, VS * h: VS * h + 64] = qkv_b[rows]
            wva[C, VS * h + 64] = 1.0
        wv8 = np.empty((128, KT_AUG * VW), dtype=f8)
        wv16 = np.empty((128, KT_AUG * VW), dtype=np.float16)
        for kc in range(KT_AUG):
            wv8[:, kc * VW:(kc + 1) * VW] = wva[kc * 128:(kc + 1) * 128].astype(f8)
            wv16[:, kc * VW:(kc + 1) * VW] = wva[kc * 128:(kc + 1) * 128].astype(np.float16)
        ins_g.append((wqk, bqk, wv8, wv16))

    # rope tables
    inv_freq = (1.0 / (ROPE_BASE ** (np.arange(0, D, 2) / D))).astype(np.float64)
    t = np.arange(T, dtype=np.float64)
    ang = t[None, :] * inv_freq[f_p][:, None]          # (128, T)
    cs = np.cos(ang).astype(np.float16)
    sgn = np.where((p % 32) < 16, -1.0, 1.0)[:, None]
    css = (sgn * np.sin(ang)).astype(np.float16)

    return xt8s, xt16s, ins_g, cs, css


def _prep_wo(out_w, g):
    out_w = np.asarray(out_w, dtype=np.float32)
    wo = np.empty((128, NCHUNK * 1024), dtype=np.float16)
    for c in range(NCHUNK):
        rows = np.concatenate([(8 * g + 2 * c + hh) * 64 + np.arange(64) for hh in range(2)])
        wo[:, c * 1024:(c + 1) * 1024] = out_w[:, rows].astype(np.float16).T
    return wo


def _in_maps(x, qkv_w, qkv_b, out_w):
    xt8s, xt16s, ins_g, cs, css = _prep_inputs(x, qkv_w, qkv_b)
    wos = [_prep_wo(out_w, g) for g in range(2)]
    in_maps = []
    for core in range(N_CORES):
        b, g = core // 2, core % 2
        wqk, bqk, wv8, wv16 = ins_g[g]
        in_maps.append({
            "xt8": xt8s[b], "xt16": xt16s[b], "wqk": wqk,
            "wv8": wv8, "wv16": wv16, "wo": wos[g],
            "bqk": bqk, "cs": cs, "css": css,
        })
    return in_maps


def kernel(x, qkv_w, qkv_b, out_w, out_b):
    from concourse.bass_utils import run_bass_kernel_spmd

    if "nc" not in _CACHE:
        _CACHE["nc"] = _build_nc()
    nc = _CACHE["nc"]

    in_maps = _in_maps(x, qkv_w, qkv_b, out_w)
    out_b = np.asarray(out_b, dtype=np.float32)

    try:
        res = run_bass_kernel_spmd(nc, in_maps, core_ids=list(range(N_CORES)))
    except ModuleNotFoundError:
        # BASS_TRACE set but the NTFF profile hook isn't importable here
        import os
        os.environ["BASS_NEVER_TRACE"] = "1"
        res = run_bass_kernel_spmd(nc, in_maps, core_ids=list(range(N_CORES)))

    out = np.empty((B, T, C), dtype=np.float32)
    for b in range(B):
        pt = res.results[2 * b]["ot"] + res.results[2 * b + 1]["ot"]  # (C, T)
        out[b] = pt.T + out_b[None, :]
    return out


# revision 8
# speedup vs baseline: 1.0500x; 1.0180x over previous
"""Causal self-attention (B=4, T=2048, C=1024, H=16, D=64) on 8 TRN2 NeuronCores.

Sharding: core = (batch b, head-group g) with b = core // 2, g = core % 2.
Each core computes heads [8g, 8g+8) of batch b and produces the partial
out-projection (C, T) for its head group; the host sums the two head-group
partials per batch and adds the output bias.

v2 performance structure:
- fp8e4m3 DoubleRow matmuls (2 contraction k-tiles per instruction) for the
  qk-projection, v-projection, scores and att@V. Scores contract over d=64
  only, so their second DoubleRow slot is a zeroed region of the rq/rk
  tiles; att@V pairs adjacent s-tiles. The out-projection and the first
  t-window's attention (t < 512, where softmax averaging is too weak to
  wash out fp8 noise) stay fp16.
- Software pipeline over windows w=(c, tt): rope of windows[i+1] and
  scores+exp of windows[i] stream on PE/Act while attV+norm of windows[i-1]
  ride behind, so the in-order engines never head-of-line block the exp
  stream. V/out-projections are interleaved where they fit; chunk 3 runs
  tt descending so the final out-projection tail is the smallest window.
- exp for the late (most softmax-diffuse) windows is computed on DVE as a
  single scalar_tensor_tensor emitting fp8e4m3 BIT PATTERNS directly
  (the fp8 bit grid is linear in log2, so bits = 1.4427*score + 56.5).
- Softmax denominator: reciprocal on a DMA-transposed [128,4] layout
  (free-dim cost 4 instead of 512), then DMA partition-broadcast. The
  att@V accumulator is per-head [65,512] in the ps_q ring, freeing PSUM
  for a 3-deep score ring that decouples exp from PE's in-order detours.
- Causal masking only touches the 128-wide diagonal slab of each e-tile
  (gpsimd affine_select); fully-masked tiles are never computed.
- All input DMA is fused into ~13 large multi-dim transfers issued from the
  idle SP sequencer, ordered so RoPE/scores inputs land first.
"""

import numpy as np

B, T, C = 4, 2048, 1024
H, D = 16, 64
N_CORES = 8
HPG = H // 2            # heads per core (group)
NCHUNK = 4              # head-pair chunks per core
KT = 8                  # k-tiles of 128 over C
KT_AUG = 9              # + bias/ones k-tile
TT = 4                  # t-tiles of 512 over T
NT = 512                # t tile (matmul N)
VS = 66                 # v column stride per head (64 dims + ones + pad)
VW = HPG * VS           # 528 v columns per k-chunk block
ROPE_BASE = 10000.0

_CACHE = {}


def _build_nc():
    import concourse.bass as bass  # noqa: F401
    import concourse.tile as tile
    from concourse import bacc, mybir
    from contextlib import ExitStack

    f16 = mybir.dt.float16
    f32 = mybir.dt.float32
    f8 = mybir.dt.float8e4
    DR = mybir.MatmulPerfMode.DoubleRow

    nc = bacc.Bacc(
        "TRN2",
        target_bir_lowering=False,
        debug=False,
        enable_asserts=True,
        num_devices=N_CORES,
    )

    xt8_d = nc.dram_tensor("xt8", (KT_AUG * 128, T), f8, kind="ExternalInput").ap()
    xt16_d = nc.dram_tensor("xt16", (KT_AUG * 128, NT), f16, kind="ExternalInput").ap()
    wqk_d = nc.dram_tensor("wqk", (128, KT * 1024), f8, kind="ExternalInput").ap()
    wv8_d = nc.dram_tensor("wv8", (128, KT_AUG * VW), f8, kind="ExternalInput").ap()
    wv16_d = nc.dram_tensor("wv16", (128, KT_AUG * VW), f16, kind="ExternalInput").ap()
    wo_d = nc.dram_tensor("wo", (128, NCHUNK * 1024), f16, kind="ExternalInput").ap()
    bqk_d = nc.dram_tensor("bqk", (128, 16), f32, kind="ExternalInput").ap()
    cs_d = nc.dram_tensor("cs", (128, T), f16, kind="ExternalInput").ap()
    css_d = nc.dram_tensor("css", (128, T), f16, kind="ExternalInput").ap()
    ot_d = nc.dram_tensor("ot", (1024, T), f32, kind="ExternalOutput").ap()

    SHUF = list(range(16, 32)) + list(range(0, 16))

    with tile.TileContext(nc) as tc:
        with ExitStack() as ctx, nc.allow_low_precision("fp8/fp16 attention pipeline"):
            consts = ctx.enter_context(tc.tile_pool(name="consts", bufs=1))
            rtmp = ctx.enter_context(tc.tile_pool(name="rtmp", bufs=5))
            ep_pool = ctx.enter_context(tc.tile_pool(name="ep", bufs=14))
            ed_pool = ctx.enter_context(tc.tile_pool(name="ed", bufs=14))
            ed16_pool = ctx.enter_context(tc.tile_pool(name="ed16", bufs=5))
            small = ctx.enter_context(tc.tile_pool(name="small", bufs=3))
            osb = ctx.enter_context(tc.tile_pool(name="osb", bufs=6))
            ps_q = ctx.enter_context(tc.tile_pool(name="psq", bufs=2, space="PSUM"))
            ps_s = ctx.enter_context(tc.tile_pool(name="pss", bufs=3, space="PSUM"))

            # ---- resident tiles + input DMA ----
            # Fused input DMA (SP-issued, ~13 transfers): xt8 t-block 0 and the
            # chunk-0 qk weights land first so RoPE + first scores start early.
            xt8 = consts.tile([128, KT_AUG * T], f8)
            wqk = consts.tile([128, KT * 1024], f8)
            xt16 = consts.tile([128, KT_AUG * NT], f16)
            wv16 = consts.tile([128, KT_AUG * VW], f16)
            wv8 = consts.tile([128, KT_AUG * VW], f8)
            wo = consts.tile([128, NCHUNK * 1024], f16)
            bqk = consts.tile([128, 16], f32)
            cs = consts.tile([128, T], f16)
            css = consts.tile([128, T], f16)

            xt8_r = xt8[:].rearrange("p (k t) -> p k t", k=KT_AUG)
            xt16_r3 = xt16[:].rearrange("p (k t) -> p k t", k=KT_AUG)

            def dma_xt(dst3, dram, nk, width, c0, w):
                # one transfer covering kc 0..nk of dram rows, cols [c0, c0+w)
                nc.sync.dma_start(
                    dst3[:, 0:nk, c0:c0 + w],
                    bass.AP(tensor=dram.tensor, offset=dram.offset + c0,
                            ap=[[width, 128], [128 * width, nk], [1, w]]))

            dma_xt(xt8_r, xt8_d, KT, T, 0, NT)                      # t-block 0
            nc.sync.dma_start(wqk[:, 0:2048], wqk_d[:, 0:2048])     # chunk 0
            dma_xt(xt8_r, xt8_d, KT_AUG, T, NT, T - NT)             # t-blocks 1-3
            nc.sync.dma_start(bqk[:], bqk_d[:])
            nc.sync.dma_start(cs[:], cs_d[:])
            nc.sync.dma_start(css[:], css_d[:])
            nc.sync.dma_start(xt8_r[:, 8:9, 0:NT],
                              bass.AP(tensor=xt8_d.tensor, offset=xt8_d.offset + 8 * 128 * T,
                                      ap=[[T, 128], [1, NT]]))      # aug t-block 0
            dma_xt(xt16_r3, xt16_d, KT_AUG, NT, 0, NT)
            nc.sync.dma_start(wv16[:], wv16_d[:])
            nc.sync.dma_start(wv8[:], wv8_d[:])
            nc.sync.dma_start(wqk[:, 2048:8192], wqk_d[:, 2048:8192])
            nc.sync.dma_start(wo[:], wo_d[:])

            b56 = consts.tile([128, 1], f32)
            nc.gpsimd.memset(b56[:], 56.5)
            v8 = consts.tile([128, 16 * VW], f8)
            v16 = consts.tile([128, 4 * VW], f16)
            y_all = consts.tile([128, NCHUNK * T], f16)

            # double-buffered q/k tiles: cols [0:T) = rope output (slot A),
            # cols [T:2T) = zeros (slot B of the DoubleRow zero-slot trick)
            rqb = [consts.tile([128, 2 * T], f8, name=f"rq{i}") for i in range(2)]
            rkb = [consts.tile([128, 2 * T], f8, name=f"rk{i}") for i in range(2)]
            for t_ in rqb + rkb:
                nc.gpsimd.memset(t_[:, T:2 * T], 0.0)

            wqk_r = wqk[:].rearrange("p (c k u) -> p c k u", c=NCHUNK, k=KT)
            wv8_r = wv8[:].rearrange("p (k w) -> p k w", k=KT_AUG)
            wv16_r = wv16[:].rearrange("p (k w) -> p k w", k=KT_AUG)
            v8_r = v8[:].rearrange("p (m w) -> p m w", m=16)
            v16_r = v16[:].rearrange("p (m w) -> p m w", m=4)

            def emit_vproj8(m_lo, m_hi):
                with nc.named_scope("vproj8"):
                    for m in range(m_lo, m_hi):  # 128-row t-slices
                        psa = ps_q.tile([128, 512], f32, tag="q")
                        psb = ps_s.tile([128, 1024], f32, tag="s")
                        for kp in range(4):
                            lhsT = xt8_r[:, 2 * kp:2 * kp + 2, m * 128:(m + 1) * 128]
                            nc.tensor.matmul(psa[:], lhsT, wv8_r[:, 2 * kp:2 * kp + 2, 0:512],
                                             start=(kp == 0), stop=False, perf_mode=DR)
                            nc.tensor.matmul(psb[:, 0:16], lhsT,
                                             wv8_r[:, 2 * kp:2 * kp + 2, 512:528],
                                             start=(kp == 0), stop=False, perf_mode=DR)
                        lhs8 = xt8_r[:, 8:9, m * 128:(m + 1) * 128]
                        nc.tensor.matmul(psa[:], lhs8, wv8_r[:, 8:9, 0:512],
                                         start=False, stop=True)
                        nc.tensor.matmul(psb[:, 0:16], lhs8, wv8_r[:, 8:9, 512:528],
                                         start=False, stop=True)
                        nc.vector.tensor_copy(v8[:, m * VW: m * VW + 512], psa[:])
                        nc.vector.tensor_copy(v8[:, m * VW + 512:(m + 1) * VW], psb[:, 0:16])

            def emit_vproj16():
                # accurate fp16 V for s < 512 (feeds the t<512 attention)
                with nc.named_scope("vproj16"):
                    for m in range(4):
                        psa = ps_q.tile([128, 512], f32, tag="q")
                        psb = ps_s.tile([128, 1024], f32, tag="s")
                        for kc in range(KT_AUG):
                            lhs = xt16[:, kc * NT + m * 128: kc * NT + (m + 1) * 128]
                            nc.tensor.matmul(psa[:], lhs, wv16_r[:, kc:kc + 1, 0:512],
                                             start=(kc == 0), stop=(kc == KT_AUG - 1))
                            nc.tensor.matmul(psb[:, 0:16], lhs, wv16_r[:, kc:kc + 1, 512:528],
                                             start=(kc == 0), stop=(kc == KT_AUG - 1))
                        nc.vector.tensor_copy(v16[:, m * VW: m * VW + 512], psa[:])
                        nc.vector.tensor_copy(v16[:, m * VW + 512:(m + 1) * VW], psb[:, 0:16])

            def emit_rope(c, tt):
                # q/k projection + RoPE for heads (2c, 2c+1), t-window tt
                rq = rqb[c % 2]
                rk = rkb[c % 2]
                with nc.named_scope("qkrope"):
                    if True:
                        t0 = tt * NT
                        for which, dst in ((0, rq), (1, rk)):
                            ps = ps_q.tile([128, 512], f32, tag="q")
                            u0 = which * 128
                            for kp in range(4):
                                nc.tensor.matmul(
                                    ps[:],
                                    wqk_r[:, c, 2 * kp:2 * kp + 2, u0:u0 + 128],
                                    xt8_r[:, 2 * kp:2 * kp + 2, t0:t0 + NT],
                                    start=(kp == 0), stop=(kp == 3), perf_mode=DR)
                            bcol = bqk[:, c * 4 + which * 2: c * 4 + which * 2 + 1]
                            bswp = bqk[:, c * 4 + which * 2 + 1: c * 4 + which * 2 + 2]
                            s_t = rtmp.tile([128, 512], f32, tag="st")
                            nc.vector.stream_shuffle(s_t[:], ps[:], SHUF)
                            x1 = rtmp.tile([128, 512], f16, tag="x1")
                            nc.vector.scalar_tensor_tensor(
                                out=x1[:], in0=ps[:], scalar=bcol, in1=cs[:, t0:t0 + NT],
                                op0=mybir.AluOpType.add, op1=mybir.AluOpType.mult)
                            x2 = rtmp.tile([128, 512], f16, tag="x2")
                            nc.vector.scalar_tensor_tensor(
                                out=x2[:], in0=s_t[:], scalar=bswp, in1=css[:, t0:t0 + NT],
                                op0=mybir.AluOpType.add, op1=mybir.AluOpType.mult)
                            nc.gpsimd.tensor_add(dst[:, t0:t0 + NT], x1[:], x2[:])

            def emit_scores(c, tt):
                # scores + exp (+ diagonal mask) for window (c, tt).
                # Returns the e-tiles for emit_attv.
                rq_r = rqb[c % 2][:].rearrange("p (s t) -> p s t", s=2)
                rk_r = rkb[c % 2][:].rearrange("p (s t) -> p s t", s=2)
                t0 = tt * NT
                n_full = t0 // 128
                epairs, ediags = [], []
                with nc.named_scope("scores"):
                    for ip in range(n_full // 2):
                        epair = ep_pool.tile([128, 2048], f8)
                        for j in range(2):
                            sc = 2 * ip + j
                            s0 = sc * 128
                            sp = ps_s.tile([128, 1024], f32, tag="s")
                            for h in range(2):
                                nc.tensor.matmul(
                                    sp[:, h * NT:(h + 1) * NT],
                                    rk_r[h * 64:(h + 1) * 64, :, s0:s0 + 128],
                                    rq_r[h * 64:(h + 1) * 64, :, t0:t0 + NT],
                                    start=True, stop=True, perf_mode=DR)
                            if tt == 3 and ip >= 4:
                                # fp8e4m3 bit grid is linear in log2, so one
                                # DVE op emits exp(s/8) bits: b = 1.4427*s+56.5
                                nc.vector.scalar_tensor_tensor(
                                    out=epair[:, j * 1024:(j + 1) * 1024].bitcast(mybir.dt.int8),
                                    in0=sp[:], scalar=1.4426950,
                                    in1=b56[:, 0:1].broadcast_to([128, 1024]),
                                    op0=mybir.AluOpType.mult,
                                    op1=mybir.AluOpType.add)
                            else:
                                nc.scalar.activation(
                                    epair[:, j * 1024:(j + 1) * 1024], sp[:],
                                    mybir.ActivationFunctionType.Exp,
                                    bias=0.0, scale=0.125)
                        epairs.append(epair)
                    for d in range(4):
                        sc = n_full + d
                        s0 = sc * 128
                        dlt = d * 128
                        sp = ps_s.tile([128, 1024], f32, tag="s")
                        for h in range(2):
                            nc.tensor.matmul(
                                sp[:, h * NT + dlt:(h + 1) * NT],
                                rk_r[h * 64:(h + 1) * 64, :, s0:s0 + 128],
                                rq_r[h * 64:(h + 1) * 64, :, t0 + dlt:t0 + NT],
                                start=True, stop=True, perf_mode=DR)
                        e_t = (ed16_pool.tile([128, 1024], f16, name="e16")
                               if tt == 0 else ed_pool.tile([128, 1024], f8, name="e8"))
                        s3 = sp[:].rearrange("p (a b) -> p a b", a=2)[:, :, dlt:]
                        e3 = e_t[:].rearrange("p (a b) -> p a b", a=2)[:, :, dlt:]
                        nc.scalar.activation(e3, s3, mybir.ActivationFunctionType.Exp,
                                             bias=0.0, scale=0.125)
                        # causal mask on the 128-wide diagonal slab only:
                        # within the slab keep iff col >= partition
                        nc.gpsimd.affine_select(
                            out=e3[:, :, 0:128], in_=e3[:, :, 0:128],
                            compare_op=mybir.AluOpType.is_ge,
                            fill=0.0, base=0,
                            pattern=[[0, 2], [1, 128]], channel_multiplier=-1)
                        ediags.append(e_t)
                return epairs, ediags

            def emit_attv(c, tt, epairs, ediags, tail=False):
                # att@V accumulation + normalization for window (c, tt),
                # one head at a time so the accumulator is a 2KB [65,512]
                # tile in the ps_q ring (frees PSUM for a 3rd score buffer)
                t0 = tt * NT
                n_full = t0 // 128
                with nc.named_scope("attv"):
                    vsrc = v16_r if tt == 0 else v8_r
                    for h in range(2):
                        vcol = VS * (2 * c + h)
                        yp = ps_q.tile([65, 512], f32, tag="q", name="yp")
                        for ip, epair in enumerate(epairs):
                            epair_r = epair[:].rearrange("p (s x) -> p s x", s=2)
                            nc.tensor.matmul(
                                yp[:], v8_r[:, 2 * ip:2 * ip + 2, vcol:vcol + 65],
                                epair_r[:, :, h * NT:(h + 1) * NT],
                                start=(ip == 0), stop=False, perf_mode=DR,
                                skip_group_check=True)
                        for d, e_t in enumerate(ediags):
                            sc = n_full + d
                            dlt = d * 128
                            nc.tensor.matmul(
                                yp[:, dlt:NT], vsrc[:, sc:sc + 1, vcol:vcol + 65],
                                e_t[:, h * NT + dlt:(h + 1) * NT],
                                start=(tt == 0 and d == 0), stop=(d == 3),
                                skip_group_check=True)

                        # normalization: y *= 1/den, reciprocal on a
                        # DMA-transposed [128,4] layout except on the tail
                        yc = small.tile([65, 512], f16, tag="yc")
                        nc.vector.tensor_copy(yc[:], yp[:])
                        rd = small.tile([1, 512], f16, tag="rd")
                        if tail:
                            nc.vector.reciprocal(rd[:], yc[64:65, :])
                        else:
                            dT = small.tile([128, 4], f16, tag="dT")
                            nc.sync.dma_start(out=dT[:], in_=yc[64:65, :])
                            rdT = small.tile([128, 4], f16, tag="rdT")
                            nc.vector.reciprocal(rdT[:], dT[:])
                            nc.sync.dma_start(out=rd[:], in_=rdT[:])
                        dbc = small.tile([64, 512], f16, tag="dbc")
                        rda = rd[0:1, :]
                        nc.sync.dma_start(out=dbc[:], in_=bass.AP(
                            tensor=rda.tensor, offset=rda.offset,
                            ap=[list(rda.ap)[0], [0, 64], [1, 512]]))
                        nc.vector.tensor_mul(
                            y_all[h * 64:(h + 1) * 64, c * T + t0: c * T + t0 + NT],
                            yc[0:64, :], dbc[:])

            def emit_oproj(tt, cs_=tuple(range(NCHUNK)), dram=None, act_copy=False,
                           wide=False):
                # output projection for t-window tt (all head-chunks of core)
                t0 = tt * NT
                dram = ot_d if dram is None else dram
                with nc.named_scope("oproj"):
                    for ct in range(8):
                        if wide and ct % 2 == 1:
                            po = ps_s.tile([128, 512], f32, tag="s", name="po")
                        else:
                            po = ps_q.tile([128, 512], f32, tag="q", name="po")
                        for c in cs_:
                            nc.tensor.matmul(po[:], wo[:, c * 1024 + ct * 128: c * 1024 + ct * 128 + 128],
                                             y_all[:, c * T + t0: c * T + t0 + NT],
                                             start=(c == cs_[0]), stop=(c == cs_[-1]))
                        ob = osb.tile([128, 512], f32)
                        if act_copy:
                            nc.scalar.activation(ob[:], po[:],
                                                 mybir.ActivationFunctionType.Copy)
                        else:
                            nc.vector.tensor_copy(ob[:], po[:])
                        oq = nc.sync
                        oq.dma_start(ot_d[ct * 128:(ct + 1) * 128, t0:t0 + NT], ob[:])

            # software pipeline over windows w=(c,tt): rope of windows[i+1]
            # and scores+exp of windows[i] stream on PE/Act while attV+norm of
            # windows[i-1] ride behind; v/out-projections interleave early/late.
            # Chunk 3 runs tt descending so the final oproj tail is smallest.
            windows = [(c, tt) for c in range(NCHUNK - 1) for tt in range(TT)]
            windows += [(NCHUNK - 1, tt) for tt in (3, 2, 1, 0)]
            emit_rope(*windows[0])
            pend = None
            for i, (c, tt) in enumerate(windows):
                if i + 1 < len(windows):
                    emit_rope(*windows[i + 1])
                et = emit_scores(c, tt)
                if i == 1:
                    emit_vproj16()
                elif i == 2:
                    emit_vproj8(0, 8)
                elif i == 3:
                    emit_vproj8(8, 16)
                if pend is not None:
                    pc, ptt, pet = pend
                    emit_attv(pc, ptt, *pet)
                    if pc == NCHUNK - 1:
                        # the last windows have no scores left to feed, so
                        # their po tiles can use the freed ps_s slots too
                        emit_oproj(ptt, wide=(ptt <= 1), act_copy=(ptt == 1))
                pend = (c, tt, et)
            pc, ptt, pet = pend
            emit_attv(pc, ptt, *pet, tail=True)
            emit_oproj(ptt, act_copy=True, wide=True)

    nc.compile()
    return nc


def _prep_inputs(x, qkv_w, qkv_b):
    """Build the 8 per-core input maps (all host-side numpy)."""
    import ml_dtypes
    f8 = ml_dtypes.float8_e4m3

    x = np.asarray(x, dtype=np.float32)
    qkv_w = np.asarray(qkv_w, dtype=np.float32)
    qkv_b = np.asarray(qkv_b, dtype=np.float32)

    # xt per batch: (KT_AUG*128, T) with row 1024 = ones, rest of aug block 0
    xt8s, xt16s = [], []
    for b in range(B):
        xa = np.zeros((KT_AUG * 128, T), dtype=np.float32)
        xa[:C] = x[b].T
        xa[C] = 1.0
        xt8s.append(xa.astype(f8))
        xt16s.append(xa[:, :NT].astype(np.float16))

    r = np.arange(64)
    d_r = 2 * ((r // 32) * 16 + (r % 16)) + ((r % 32) >= 16)  # row -> head dim
    p = np.arange(128)
    f_p = ((p // 32) % 2) * 16 + (p % 16)

    ins_g = []
    for g in range(2):
        # wqk: [p, c*2048 + kc*256 + which*128 + m] (chunk-major)
        wqk = np.empty((128, KT * 1024), dtype=f8)
        bqk = np.empty((128, 16), dtype=np.float32)
        for c in range(NCHUNK):
            for which in range(2):  # 0=q, 1=k
                rows = np.concatenate([
                    which * C + (8 * g + 2 * c + hh) * 64 + d_r for hh in range(2)
                ])  # 128 feature rows
                blk = qkv_w[rows, :]          # (128 feat, 1024 k)
                for kc in range(KT):
                    sl = slice(c * 2048 + kc * 256 + which * 128,
                               c * 2048 + kc * 256 + which * 128 + 128)
                    wqk[:, sl] = blk[:, kc * 128:(kc + 1) * 128].T.astype(f8)
                bc = qkv_b[rows].astype(np.float32)
                bqk[:, c * 4 + which * 2] = bc
                bqk[:, c * 4 + which * 2 + 1] = bc[p ^ 16]
        # wv: [p, kc*VW + col], col = VS*h + j
        wva = np.zeros((KT_AUG * 128, VW), dtype=np.float32)
        for h in range(HPG):
            rows = 2 * C + (8 * g + h) * 64 + np.arange(64)
            wva[:C, VS * h: VS * h + 64] = qkv_w[rows, :].T
            wva[C, VS * h: VS * h + 64] = qkv_b[rows]
            wva[C, VS * h + 64] = 1.0
        wv8 = np.empty((128, KT_AUG * VW), dtype=f8)
        wv16 = np.empty((128, KT_AUG * VW), dtype=np.float16)
        for kc in range(KT_AUG):
            wv8[:, kc * VW:(kc + 1) * VW] = wva[kc * 128:(kc + 1) * 128].astype(f8)
            wv16[:, kc * VW:(kc + 1) * VW] = wva[kc * 128:(kc + 1) * 128].astype(np.float16)
        ins_g.append((wqk, bqk, wv8, wv16))

    # rope tables
    inv_freq = (1.0 / (ROPE_BASE ** (np.arange(0, D, 2) / D))).astype(np.float64)
    t = np.arange(T, dtype=np.float64)
    ang = t[None, :] * inv_freq[f_p][:, None]          # (128, T)
    cs = np.cos(ang).astype(np.float16)
    sgn = np.where((p % 32) < 16, -1.0, 1.0)[:, None]
    css = (sgn * np.sin(ang)).astype(np.float16)

    return xt8s, xt16s, ins_g, cs, css


def _prep_wo(out_w, g):
    out_w = np.asarray(out_w, dtype=np.float32)
    wo = np.empty((128, NCHUNK * 1024), dtype=np.float16)
    for c in range(NCHUNK):
        rows = np.concatenate([(8 * g + 2 * c + hh) * 64 + np.arange(64) for hh in range(2)])
        wo[:, c * 1024:(c + 1) * 1024] = out_w[:, rows].astype(np.float16).T
    return wo


def _in_maps(x, qkv_w, qkv_b, out_w):
    xt8s, xt16s, ins_g, cs, css = _prep_inputs(x, qkv_w, qkv_b)
    wos = [_prep_wo(out_w, g) for g in range(2)]
    in_maps = []
    for core in range(N_CORES):
        b, g = core // 2, core % 2
        wqk, bqk, wv8, wv16 = ins_g[g]
        in_maps.append({
            "xt8": xt8s[b], "xt16": xt16s[b], "wqk": wqk,
            "wv8": wv8, "wv16": wv16, "wo": wos[g],
            "bqk": bqk, "cs": cs, "css": css,
        })
    return in_maps


def kernel(x, qkv_w, qkv_b, out_w, out_b):
    from concourse.bass_utils import run_bass_kernel_spmd

    if "nc" not in _CACHE:
        _CACHE["nc"] = _build_nc()
    nc = _CACHE["nc"]

    in_maps = _in_maps(x, qkv_w, qkv_b, out_w)
    out_b = np.asarray(out_b, dtype=np.float32)

    try:
        res = run_bass_kernel_spmd(nc, in_maps, core_ids=list(range(N_CORES)))
    except ModuleNotFoundError:
        # BASS_TRACE set but the NTFF profile hook isn't importable here
        import os
        os.environ["BASS_NEVER_TRACE"] = "1"
        res = run_bass_kernel_spmd(nc, in_maps, core_ids=list(range(N_CORES)))

    out = np.empty((B, T, C), dtype=np.float32)
    for b in range(B):
        pt = res.results[2 * b]["ot"] + res.results[2 * b + 1]["ot"]  # (C, T)
        out[b] = pt.T + out_b[None, :]
    return out


# revision 9
# speedup vs baseline: 1.1105x; 1.0576x over previous
"""Causal self-attention (B=4, T=2048, C=1024, H=16, D=64) on 8 TRN2 NeuronCores.

Sharding: core = (batch b, head-group g) with b = core // 2, g = core % 2.
Each core computes heads [8g, 8g+8) of batch b and produces the partial
out-projection (C, T) for its head group; the host sums the two head-group
partials per batch and adds the output bias.

v2 performance structure:
- fp8e4m3 DoubleRow matmuls (2 contraction k-tiles per instruction) for the
  qk-projection, v-projection, scores and att@V. Scores contract over d=64
  only, so their second DoubleRow slot is a zeroed region of the rq/rk
  tiles; att@V pairs adjacent s-tiles. The out-projection and the first
  t-window's attention (t < 512, where softmax averaging is too weak to
  wash out fp8 noise) stay fp16.
- Software pipeline over windows w=(c, tt): rope of windows[i+1] and
  scores+exp of windows[i] stream on PE/Act while attV+norm of windows[i-1]
  ride behind, so the in-order engines never head-of-line block the exp
  stream. V/out-projections are interleaved where they fit; chunk 3 runs
  tt descending so the final out-projection tail is the smallest window.
- exp for the late (most softmax-diffuse) windows is computed on DVE as a
  single scalar_tensor_tensor emitting fp8e4m3 BIT PATTERNS directly
  (the fp8 bit grid is linear in log2, so bits = 1.4427*score + 56.5).
- Softmax denominator: reciprocal on a DMA-transposed [128,4] layout
  (free-dim cost 4 instead of 512), then DMA partition-broadcast. The
  att@V accumulator is per-head [65,512] in the ps_q ring, freeing PSUM
  for a 3-deep score ring that decouples exp from PE's in-order detours.
- Causal masking only touches the 128-wide diagonal slab of each e-tile
  (gpsimd affine_select); fully-masked tiles are never computed.
- All input DMA is fused into ~13 large multi-dim transfers issued from the
  idle SP sequencer, ordered so RoPE/scores inputs land first.
"""

import numpy as np

B, T, C = 4, 2048, 1024
H, D = 16, 64
N_CORES = 8
HPG = H // 2            # heads per core (group)
NCHUNK = 4              # head-pair chunks per core
KT = 8                  # k-tiles of 128 over C
KT_AUG = 9              # + bias/ones k-tile
TT = 4                  # t-tiles of 512 over T
NT = 512                # t tile (matmul N)
VS = 66                 # v column stride per head (64 dims + ones + pad)
VW = HPG * VS           # 528 v columns per k-chunk block
ROPE_BASE = 10000.0

_CACHE = {}


def _build_nc():
    import concourse.bass as bass  # noqa: F401
    import concourse.tile as tile
    from concourse import bacc, mybir
    from contextlib import ExitStack

    f16 = mybir.dt.float16
    f32 = mybir.dt.float32
    f8 = mybir.dt.float8e4
    DR = mybir.MatmulPerfMode.DoubleRow

    nc = bacc.Bacc(
        "TRN2",
        target_bir_lowering=False,
        debug=False,
        enable_asserts=True,
        num_devices=N_CORES,
    )

    xt8_d = nc.dram_tensor("xt8", (KT_AUG * 128, T), f8, kind="ExternalInput").ap()
    xt16_d = nc.dram_tensor("xt16", (KT_AUG * 128, NT), f16, kind="ExternalInput").ap()
    wqk_d = nc.dram_tensor("wqk", (128, KT * 1024), f8, kind="ExternalInput").ap()
    wv8_d = nc.dram_tensor("wv8", (128, KT_AUG * VW), f8, kind="ExternalInput").ap()
    wv16_d = nc.dram_tensor("wv16", (128, KT_AUG * VW), f16, kind="ExternalInput").ap()
    wo_d = nc.dram_tensor("wo", (128, NCHUNK * 1024), f16, kind="ExternalInput").ap()
    bqk_d = nc.dram_tensor("bqk", (128, 16), f32, kind="ExternalInput").ap()
    cs_d = nc.dram_tensor("cs", (128, T), f16, kind="ExternalInput").ap()
    css_d = nc.dram_tensor("css", (128, T), f16, kind="ExternalInput").ap()
    ot_d = nc.dram_tensor("ot", (1024, T), f32, kind="ExternalOutput").ap()

    SHUF = list(range(16, 32)) + list(range(0, 16))

    with tile.TileContext(nc) as tc:
        with ExitStack() as ctx, nc.allow_low_precision("fp8/fp16 attention pipeline"):
            consts = ctx.enter_context(tc.tile_pool(name="consts", bufs=1))
            rtmp = ctx.enter_context(tc.tile_pool(name="rtmp", bufs=5))
            ep_pool = ctx.enter_context(tc.tile_pool(name="ep", bufs=14))
            ed_pool = ctx.enter_context(tc.tile_pool(name="ed", bufs=14))
            ed16_pool = ctx.enter_context(tc.tile_pool(name="ed16", bufs=5))
            small = ctx.enter_context(tc.tile_pool(name="small", bufs=3))
            osb = ctx.enter_context(tc.tile_pool(name="osb", bufs=6))
            ps_q = ctx.enter_context(tc.tile_pool(name="psq", bufs=2, space="PSUM"))
            ps_s = ctx.enter_context(tc.tile_pool(name="pss", bufs=3, space="PSUM"))

            # ---- resident tiles + input DMA ----
            # Fused input DMA (SP-issued, ~13 transfers): xt8 t-block 0 and the
            # chunk-0 qk weights land first so RoPE + first scores start early.
            xt8 = consts.tile([128, KT_AUG * T], f8)
            wqk = consts.tile([128, KT * 1024], f8)
            xt16 = consts.tile([128, KT_AUG * NT], f16)
            wv16 = consts.tile([128, KT_AUG * VW], f16)
            wv8 = consts.tile([128, KT_AUG * VW], f8)
            wo = consts.tile([128, NCHUNK * 1024], f16)
            bqk = consts.tile([128, 16], f32)
            cs = consts.tile([128, T], f16)
            css = consts.tile([128, T], f16)

            xt8_r = xt8[:].rearrange("p (k t) -> p k t", k=KT_AUG)
            xt16_r3 = xt16[:].rearrange("p (k t) -> p k t", k=KT_AUG)

            def dma_xt(dst3, dram, nk, width, c0, w):
                # one transfer covering kc 0..nk of dram rows, cols [c0, c0+w)
                nc.sync.dma_start(
                    dst3[:, 0:nk, c0:c0 + w],
                    bass.AP(tensor=dram.tensor, offset=dram.offset + c0,
                            ap=[[width, 128], [128 * width, nk], [1, w]]))

            dma_xt(xt8_r, xt8_d, KT, T, 0, NT)                      # t-block 0
            nc.sync.dma_start(wqk[:, 0:2048], wqk_d[:, 0:2048])     # chunk 0
            dma_xt(xt8_r, xt8_d, KT_AUG, T, NT, T - NT)             # t-blocks 1-3
            nc.sync.dma_start(bqk[:], bqk_d[:])
            nc.sync.dma_start(cs[:], cs_d[:])
            nc.sync.dma_start(css[:], css_d[:])
            nc.sync.dma_start(xt8_r[:, 8:9, 0:NT],
                              bass.AP(tensor=xt8_d.tensor, offset=xt8_d.offset + 8 * 128 * T,
                                      ap=[[T, 128], [1, NT]]))      # aug t-block 0
            dma_xt(xt16_r3, xt16_d, KT_AUG, NT, 0, NT)
            nc.sync.dma_start(wv16[:], wv16_d[:])
            nc.sync.dma_start(wv8[:], wv8_d[:])
            nc.sync.dma_start(wqk[:, 2048:8192], wqk_d[:, 2048:8192])
            nc.sync.dma_start(wo[:], wo_d[:])

            b56 = consts.tile([128, 1], f32)
            nc.gpsimd.memset(b56[:], 56.5)
            v8 = consts.tile([128, 16 * VW], f8)
            v16 = consts.tile([128, 4 * VW], f16)
            y_all = consts.tile([128, NCHUNK * T], f16)

            # double-buffered q/k tiles: cols [0:T) = rope output (slot A),
            # cols [T:2T) = zeros (slot B of the DoubleRow zero-slot trick)
            rqb = [consts.tile([128, 2 * T], f8, name=f"rq{i}") for i in range(2)]
            rkb = [consts.tile([128, 2 * T], f8, name=f"rk{i}") for i in range(2)]
            for t_ in rqb + rkb:
                nc.gpsimd.memset(t_[:, T:2 * T], 0.0)

            wqk_r = wqk[:].rearrange("p (c k u) -> p c k u", c=NCHUNK, k=KT)
            wv8_r = wv8[:].rearrange("p (k w) -> p k w", k=KT_AUG)
            wv16_r = wv16[:].rearrange("p (k w) -> p k w", k=KT_AUG)
            v8_r = v8[:].rearrange("p (m w) -> p m w", m=16)
            v16_r = v16[:].rearrange("p (m w) -> p m w", m=4)

            def emit_vproj8(m_lo, m_hi):
                with nc.named_scope("vproj8"):
                    for m in range(m_lo, m_hi):  # 128-row t-slices
                        psa = ps_q.tile([128, 512], f32, tag="q")
                        psb = ps_s.tile([128, 1024], f32, tag="s")
                        for kp in range(4):
                            lhsT = xt8_r[:, 2 * kp:2 * kp + 2, m * 128:(m + 1) * 128]
                            nc.tensor.matmul(psa[:], lhsT, wv8_r[:, 2 * kp:2 * kp + 2, 0:512],
                                             start=(kp == 0), stop=False, perf_mode=DR)
                            nc.tensor.matmul(psb[:, 0:16], lhsT,
                                             wv8_r[:, 2 * kp:2 * kp + 2, 512:528],
                                             start=(kp == 0), stop=False, perf_mode=DR)
                        lhs8 = xt8_r[:, 8:9, m * 128:(m + 1) * 128]
                        nc.tensor.matmul(psa[:], lhs8, wv8_r[:, 8:9, 0:512],
                                         start=False, stop=True)
                        nc.tensor.matmul(psb[:, 0:16], lhs8, wv8_r[:, 8:9, 512:528],
                                         start=False, stop=True)
                        nc.vector.tensor_copy(v8[:, m * VW: m * VW + 512], psa[:])
                        nc.vector.tensor_copy(v8[:, m * VW + 512:(m + 1) * VW], psb[:, 0:16])

            def emit_vproj16():
                # accurate fp16 V for s < 512 (feeds the t<512 attention)
                with nc.named_scope("vproj16"):
                    for m in range(4):
                        psa = ps_q.tile([128, 512], f32, tag="q")
                        psb = ps_s.tile([128, 1024], f32, tag="s")
                        for kc in range(KT_AUG):
                            lhs = xt16[:, kc * NT + m * 128: kc * NT + (m + 1) * 128]
                            nc.tensor.matmul(psa[:], lhs, wv16_r[:, kc:kc + 1, 0:512],
                                             start=(kc == 0), stop=(kc == KT_AUG - 1))
                            nc.tensor.matmul(psb[:, 0:16], lhs, wv16_r[:, kc:kc + 1, 512:528],
                                             start=(kc == 0), stop=(kc == KT_AUG - 1))
                        nc.vector.tensor_copy(v16[:, m * VW: m * VW + 512], psa[:])
                        nc.vector.tensor_copy(v16[:, m * VW + 512:(m + 1) * VW], psb[:, 0:16])

            def emit_rope(c, tt):
                # q/k projection + RoPE for heads (2c, 2c+1), t-window tt
                rq = rqb[c % 2]
                rk = rkb[c % 2]
                with nc.named_scope("qkrope"):
                    if True:
                        t0 = tt * NT
                        for which, dst in ((0, rq), (1, rk)):
                            ps = ps_q.tile([128, 512], f32, tag="q")
                            u0 = which * 128
                            for kp in range(4):
                                nc.tensor.matmul(
                                    ps[:],
                                    wqk_r[:, c, 2 * kp:2 * kp + 2, u0:u0 + 128],
                                    xt8_r[:, 2 * kp:2 * kp + 2, t0:t0 + NT],
                                    start=(kp == 0), stop=(kp == 3), perf_mode=DR)
                            bcol = bqk[:, c * 4 + which * 2: c * 4 + which * 2 + 1]
                            bswp = bqk[:, c * 4 + which * 2 + 1: c * 4 + which * 2 + 2]
                            s_t = rtmp.tile([128, 512], f32, tag="st")
                            nc.vector.stream_shuffle(s_t[:], ps[:], SHUF)
                            x1 = rtmp.tile([128, 512], f16, tag="x1")
                            nc.vector.scalar_tensor_tensor(
                                out=x1[:], in0=ps[:], scalar=bcol, in1=cs[:, t0:t0 + NT],
                                op0=mybir.AluOpType.add, op1=mybir.AluOpType.mult)
                            x2 = rtmp.tile([128, 512], f16, tag="x2")
                            nc.vector.scalar_tensor_tensor(
                                out=x2[:], in0=s_t[:], scalar=bswp, in1=css[:, t0:t0 + NT],
                                op0=mybir.AluOpType.add, op1=mybir.AluOpType.mult)
                            nc.gpsimd.tensor_add(dst[:, t0:t0 + NT], x1[:], x2[:])

            def emit_scores(c, tt):
                # scores + exp (+ diagonal mask) for window (c, tt).
                # Returns the e-tiles for emit_attv.
                rq_r = rqb[c % 2][:].rearrange("p (s t) -> p s t", s=2)
                rk_r = rkb[c % 2][:].rearrange("p (s t) -> p s t", s=2)
                t0 = tt * NT
                n_full = t0 // 128
                epairs, ediags = [], []
                with nc.named_scope("scores"):
                    for ip in range(n_full // 2):
                        epair = ep_pool.tile([128, 2048], f8)
                        for j in range(2):
                            sc = 2 * ip + j
                            s0 = sc * 128
                            sp = ps_s.tile([128, 1024], f32, tag="s")
                            for h in range(2):
                                nc.tensor.matmul(
                                    sp[:, h * NT:(h + 1) * NT],
                                    rk_r[h * 64:(h + 1) * 64, :, s0:s0 + 128],
                                    rq_r[h * 64:(h + 1) * 64, :, t0:t0 + NT],
                                    start=True, stop=True, perf_mode=DR)
                            if tt == 3 and ip >= 4:
                                # fp8e4m3 bit grid is linear in log2, so one
                                # DVE op emits exp(s/8) bits: b = 1.4427*s+56.5
                                nc.vector.scalar_tensor_tensor(
                                    out=epair[:, j * 1024:(j + 1) * 1024].bitcast(mybir.dt.int8),
                                    in0=sp[:], scalar=1.4426950,
                                    in1=b56[:, 0:1].broadcast_to([128, 1024]),
                                    op0=mybir.AluOpType.mult,
                                    op1=mybir.AluOpType.add)
                            else:
                                nc.scalar.activation(
                                    epair[:, j * 1024:(j + 1) * 1024], sp[:],
                                    mybir.ActivationFunctionType.Exp,
                                    bias=0.0, scale=0.125)
                        epairs.append(epair)
                    for d in range(4):
                        sc = n_full + d
                        s0 = sc * 128
                        dlt = d * 128
                        sp = ps_s.tile([128, 1024], f32, tag="s")
                        for h in range(2):
                            nc.tensor.matmul(
                                sp[:, h * NT + dlt:(h + 1) * NT],
                                rk_r[h * 64:(h + 1) * 64, :, s0:s0 + 128],
                                rq_r[h * 64:(h + 1) * 64, :, t0 + dlt:t0 + NT],
                                start=True, stop=True, perf_mode=DR)
                        e_t = (ed16_pool.tile([128, 1024], f16, name="e16")
                               if tt == 0 else ed_pool.tile([128, 1024], f8, name="e8"))
                        s3 = sp[:].rearrange("p (a b) -> p a b", a=2)[:, :, dlt:]
                        e3 = e_t[:].rearrange("p (a b) -> p a b", a=2)[:, :, dlt:]
                        nc.scalar.activation(e3, s3, mybir.ActivationFunctionType.Exp,
                                             bias=0.0, scale=0.125)
                        # causal mask on the 128-wide diagonal slab only:
                        # within the slab keep iff col >= partition
                        nc.gpsimd.affine_select(
                            out=e3[:, :, 0:128], in_=e3[:, :, 0:128],
                            compare_op=mybir.AluOpType.is_ge,
                            fill=0.0, base=0,
                            pattern=[[0, 2], [1, 128]], channel_multiplier=-1)
                        ediags.append(e_t)
                return epairs, ediags

            def emit_attv(c, tt, epairs, ediags, tail=False):
                # att@V accumulation + normalization for window (c, tt),
                # one head at a time so the accumulator is a 2KB [65,512]
                # tile in the ps_q ring (frees PSUM for a 3rd score buffer)
                t0 = tt * NT
                n_full = t0 // 128
                with nc.named_scope("attv"):
                    vsrc = v16_r if tt == 0 else v8_r
                    for h in range(2):
                        vcol = VS * (2 * c + h)
                        yp = ps_q.tile([65, 512], f32, tag="q", name="yp")
                        for ip, epair in enumerate(epairs):
                            epair_r = epair[:].rearrange("p (s x) -> p s x", s=2)
                            nc.tensor.matmul(
                                yp[:], v8_r[:, 2 * ip:2 * ip + 2, vcol:vcol + 65],
                                epair_r[:, :, h * NT:(h + 1) * NT],
                                start=(ip == 0), stop=False, perf_mode=DR,
                                skip_group_check=True)
                        for d, e_t in enumerate(ediags):
                            sc = n_full + d
                            dlt = d * 128
                            nc.tensor.matmul(
                                yp[:, dlt:NT], vsrc[:, sc:sc + 1, vcol:vcol + 65],
                                e_t[:, h * NT + dlt:(h + 1) * NT],
                                start=(tt == 0 and d == 0), stop=(d == 3),
                                skip_group_check=True)

                        # normalization: y *= 1/den, reciprocal on a
                        # DMA-transposed [128,4] layout except on the tail
                        yc = small.tile([65, 512], f16, tag="yc")
                        nc.vector.tensor_copy(yc[:], yp[:])
                        rd = small.tile([1, 512], f16, tag="rd")
                        if tail:
                            nc.vector.reciprocal(rd[:], yc[64:65, :])
                        else:
                            dT = small.tile([128, 4], f16, tag="dT")
                            nc.sync.dma_start(out=dT[:], in_=yc[64:65, :])
                            rdT = small.tile([128, 4], f16, tag="rdT")
                            nc.vector.reciprocal(rdT[:], dT[:])
                            nc.sync.dma_start(out=rd[:], in_=rdT[:])
                        dbc = small.tile([64, 512], f16, tag="dbc")
                        rda = rd[0:1, :]
                        nc.sync.dma_start(out=dbc[:], in_=bass.AP(
                            tensor=rda.tensor, offset=rda.offset,
                            ap=[list(rda.ap)[0], [0, 64], [1, 512]]))
                        nc.vector.tensor_mul(
                            y_all[h * 64:(h + 1) * 64, c * T + t0: c * T + t0 + NT],
                            yc[0:64, :], dbc[:])

            def emit_oproj(tt, cs_=tuple(range(NCHUNK)), dram=None, act_copy=False,
                           wide=False):
                # output projection for t-window tt (all head-chunks of core)
                t0 = tt * NT
                dram = ot_d if dram is None else dram
                with nc.named_scope("oproj"):
                    for ct in range(8):
                        if wide and ct % 2 == 1:
                            po = ps_s.tile([128, 512], f32, tag="s", name="po")
                        else:
                            po = ps_q.tile([128, 512], f32, tag="q", name="po")
                        for c in cs_:
                            nc.tensor.matmul(po[:], wo[:, c * 1024 + ct * 128: c * 1024 + ct * 128 + 128],
                                             y_all[:, c * T + t0: c * T + t0 + NT],
                                             start=(c == cs_[0]), stop=(c == cs_[-1]))
                        ob = osb.tile([128, 512], f32)
                        if act_copy:
                            nc.scalar.activation(ob[:], po[:],
                                                 mybir.ActivationFunctionType.Copy)
                        else:
                            nc.vector.tensor_copy(ob[:], po[:])
                        oq = nc.sync
                        oq.dma_start(ot_d[ct * 128:(ct + 1) * 128, t0:t0 + NT], ob[:])

            # software pipeline over windows w=(c,tt): rope of windows[i+1]
            # and scores+exp of windows[i] stream on PE/Act while attV+norm of
            # windows[i-1] ride behind; v/out-projections interleave early/late.
            # Chunk 3 runs tt descending so the final oproj tail is smallest.
            windows = [(c, tt) for c in range(NCHUNK - 1) for tt in range(TT)]
            windows += [(NCHUNK - 1, tt) for tt in (3, 2, 1, 0)]
            emit_rope(*windows[0])
            emit_rope(*windows[1])
            pend = None
            for i, (c, tt) in enumerate(windows):
                if i + 2 < len(windows):
                    emit_rope(*windows[i + 2])
                et = emit_scores(c, tt)
                if i == 1:
                    emit_vproj16()
                elif i == 2:
                    emit_vproj8(0, 8)
                elif i == 3:
                    emit_vproj8(8, 16)
                if pend is not None:
                    pc, ptt, pet = pend
                    emit_attv(pc, ptt, *pet)
                    if pc == NCHUNK - 1:
                        # the last windows have no scores left to feed, so
                        # their po tiles can use the freed ps_s slots too
                        emit_oproj(ptt, wide=(ptt <= 1), act_copy=(ptt == 1))
                pend = (c, tt, et)
            pc, ptt, pet = pend
            emit_attv(pc, ptt, *pet, tail=True)
            emit_oproj(ptt, act_copy=True, wide=True)

    nc.compile()
    return nc


def _prep_inputs(x, qkv_w, qkv_b):
    """Build the 8 per-core input maps (all host-side numpy)."""
    import ml_dtypes
    f8 = ml_dtypes.float8_e4m3

    x = np.asarray(x, dtype=np.float32)
    qkv_w = np.asarray(qkv_w, dtype=np.float32)
    qkv_b = np.asarray(qkv_b, dtype=np.float32)

    # xt per batch: (KT_AUG*128, T) with row 1024 = ones, rest of aug block 0
    xt8s, xt16s = [], []
    for b in range(B):
        xa = np.zeros((KT_AUG * 128, T), dtype=np.float32)
        xa[:C] = x[b].T
        xa[C] = 1.0
        xt8s.append(xa.astype(f8))
        xt16s.append(xa[:, :NT].astype(np.float16))

    r = np.arange(64)
    d_r = 2 * ((r // 32) * 16 + (r % 16)) + ((r % 32) >= 16)  # row -> head dim
    p = np.arange(128)
    f_p = ((p // 32) % 2) * 16 + (p % 16)

    ins_g = []
    for g in range(2):
        # wqk: [p, c*2048 + kc*256 + which*128 + m] (chunk-major)
        wqk = np.empty((128, KT * 1024), dtype=f8)
        bqk = np.empty((128, 16), dtype=np.float32)
        for c in range(NCHUNK):
            for which in range(2):  # 0=q, 1=k
                rows = np.concatenate([
                    which * C + (8 * g + 2 * c + hh) * 64 + d_r for hh in range(2)
                ])  # 128 feature rows
                blk = qkv_w[rows, :]          # (128 feat, 1024 k)
                for kc in range(KT):
                    sl = slice(c * 2048 + kc * 256 + which * 128,
                               c * 2048 + kc * 256 + which * 128 + 128)
                    wqk[:, sl] = blk[:, kc * 128:(kc + 1) * 128].T.astype(f8)
                bc = qkv_b[rows].astype(np.float32)
                bqk[:, c * 4 + which * 2] = bc
                bqk[:, c * 4 + which * 2 + 1] = bc[p ^ 16]
        # wv: [p, kc*VW + col], col = VS*h + j
        wva = np.zeros((KT_AUG * 128, VW), dtype=np.float32)
        for h in range(HPG):
            rows = 2 * C + (8 * g + h) * 64 + np.arange(64)
            wva[:C, VS * h: VS * h + 64] = qkv_w[rows, :].T
            wva[C, VS * h: VS * h + 64] = qkv_b[rows]
            wva[C, VS * h + 64] = 1.0
        wv8 = np.empty((128, KT_AUG * VW), dtype=f8)
        wv16 = np.empty((128, KT_AUG * VW), dtype=np.float16)
        for kc in range(KT_AUG):
            wv8[:, kc * VW:(kc + 1) * VW] = wva[kc * 128:(kc + 1) * 128].astype(f8)
            wv16[:, kc * VW:(kc + 1) * VW] = wva[kc * 128:(kc + 1) * 128].astype(np.float16)
        ins_g.append((wqk, bqk, wv8, wv16))

    # rope tables
    inv_freq = (1.0 / (ROPE_BASE ** (np.arange(0, D, 2) / D))).astype(np.float64)
    t = np.arange(T, dtype=np.float64)
    ang = t[None, :] * inv_freq[f_p][:, None]          # (128, T)
    cs = np.cos(ang).astype(np.float16)
    sgn = np.where((p % 32) < 16, -1.0, 1.0)[:, None]
    css = (sgn * np.sin(ang)).astype(np.float16)

    return xt8s, xt16s, ins_g, cs, css


def _prep_wo(out_w, g):
    out_w = np.asarray(out_w, dtype=np.float32)
    wo = np.empty((128, NCHUNK * 1024), dtype=np.float16)
    for c in range(NCHUNK):
        rows = np.concatenate([(8 * g + 2 * c + hh) * 64 + np.arange(64) for hh in range(2)])
        wo[:, c * 1024:(c + 1) * 1024] = out_w[:, rows].astype(np.float16).T
    return wo


def _in_maps(x, qkv_w, qkv_b, out_w):
    xt8s, xt16s, ins_g, cs, css = _prep_inputs(x, qkv_w, qkv_b)
    wos = [_prep_wo(out_w, g) for g in range(2)]
    in_maps = []
    for core in range(N_CORES):
        b, g = core // 2, core % 2
        wqk, bqk, wv8, wv16 = ins_g[g]
        in_maps.append({
            "xt8": xt8s[b], "xt16": xt16s[b], "wqk": wqk,
            "wv8": wv8, "wv16": wv16, "wo": wos[g],
            "bqk": bqk, "cs": cs, "css": css,
        })
    return in_maps


def kernel(x, qkv_w, qkv_b, out_w, out_b):
    from concourse.bass_utils import run_bass_kernel_spmd

    if "nc" not in _CACHE:
        _CACHE["nc"] = _build_nc()
    nc = _CACHE["nc"]

    in_maps = _in_maps(x, qkv_w, qkv_b, out_w)
    out_b = np.asarray(out_b, dtype=np.float32)

    try:
        res = run_bass_kernel_spmd(nc, in_maps, core_ids=list(range(N_CORES)))
    except ModuleNotFoundError:
        # BASS_TRACE set but the NTFF profile hook isn't importable here
        import os
        os.environ["BASS_NEVER_TRACE"] = "1"
        res = run_bass_kernel_spmd(nc, in_maps, core_ids=list(range(N_CORES)))

    out = np.empty((B, T, C), dtype=np.float32)
    for b in range(B):
        pt = res.results[2 * b]["ot"] + res.results[2 * b + 1]["ot"]  # (C, T)
        out[b] = pt.T + out_b[None, :]
    return out
